# revision 1
# baseline (speedup 1.0000x reference)
"""GridNetBlock (TF-GridNet) Trainium2 kernel: 8-core SPMD, 5 launches."""
import sys, os, contextlib
for _p in ("/opt/trn_rl_repo", "/root/.axon_site/_ro/trn_rl_repo"):
    if os.path.isdir(_p) and _p not in sys.path:
        sys.path.insert(0, _p)
import numpy as np
import concourse.bass as bass
import concourse.bacc as bacc
import concourse.tile as tile
from concourse import mybir
from concourse.masks import make_identity
from concourse.bass_utils import run_bass_kernel_spmd

F32 = mybir.dt.float32
BF16 = mybir.dt.bfloat16
AF = mybir.ActivationFunctionType
OP = mybir.AluOpType
AX = mybir.AxisListType

B, C, T, Q = 2, 64, 1000, 65
KS = 4
Qp, L1, Hh, HID, L2 = 68, 17, 128, 256, 250
NH, E, Dv = 4, 4, 16
EPS = 1e-5
NCORES = 8
TSH = T // 4
NP1 = TSH * Qp
G1 = (NP1 + 127) // 128   # 133
RW2 = (B * Qp) // NCORES  # 17
TP = 1003                 # causal padded time
NT1 = L1 * TSH            # 4250
NT2 = L2 * RW2            # 4250


def bap(t, tail):
    ap = list(t.ap)
    for n in tail:
        ap.append([0, n])
    return bass.AP(tensor=t.tensor, offset=t.offset, ap=ap)


def shift_ap(t, off, dims):
    return bass.AP(tensor=t.tensor, offset=t.offset + off, ap=[t.ap[0]] + dims)


def new_nc():
    return bacc.Bacc("TRN2", target_bir_lowering=False, debug=False,
                     enable_asserts=True, num_devices=NCORES)


def ln_posmajor(nc, pool, work, xpm, G, nred, eps_t):
    s1 = work.tile([128, G], F32, tag="lns1")
    nc.vector.tensor_reduce(out=s1[:], in_=xpm[:], axis=AX.X, op=OP.add)
    xsq = pool.tile([128, G, nred], BF16, tag="xut")
    nc.scalar.activation(out=xsq[:], in_=xpm[:], func=AF.Square)
    s2 = work.tile([128, G], F32, tag="lns2")
    nc.vector.tensor_reduce(out=s2[:], in_=xsq[:], axis=AX.X, op=OP.add)
    mu = work.tile([128, G], F32, tag="lnmu")
    nc.vector.tensor_scalar_mul(out=mu[:], in0=s1[:], scalar1=1.0 / nred)
    var = work.tile([128, G], F32, tag="lnvar")
    nc.vector.tensor_tensor(out=var[:], in0=mu[:], in1=mu[:], op=OP.mult)
    nc.vector.scalar_tensor_tensor(out=var[:], in0=s2[:], scalar=1.0 / nred,
                                   in1=var[:], op0=OP.mult, op1=OP.subtract)
    rs = work.tile([128, G], F32, tag="lnrs")
    nc.scalar.activation(out=rs[:], in_=var[:], func=AF.Sqrt, bias=eps_t[:])
    nc.vector.reciprocal(out=rs[:], in_=rs[:])
    zpm = pool.tile([128, G, nred], BF16, tag="xut")
    nc.vector.tensor_tensor(out=zpm[:], in0=xpm[:], in1=bap(mu, [nred]),
                            op=OP.subtract)
    nc.vector.tensor_tensor(out=zpm[:], in0=zpm[:], in1=bap(rs, [nred]),
                            op=OP.mult)
    return zpm


def ap3(t, off, d1, n1, d2, n2):
    """Strided 2-free-dim AP view of tile t at element offset off."""
    return bass.AP(tensor=t.tensor, offset=t.offset + off,
                   ap=[t.ap[0], [d1, n1], [d2, n2]])


def build_lstm_launch(which):
    """which: 'intra' or 'inter'. Returns compiled nc."""
    intra = which == "intra"
    ND = 2 if intra else 1
    MC = 4 if intra else 8
    KC = 1 if intra else 2
    L = L1 if intra else L2
    NB = TSH if intra else RW2        # lstm batch per core
    NT = L * NB                       # 4250
    G = G1
    ZC = G * 128

    nc = new_nc()
    x_pm = nc.dram_tensor("x_pm", [128, G, C], BF16, kind="ExternalInput")
    x_u = nc.dram_tensor("x_u", [128, 2, NT], F32, kind="ExternalInput")
    if intra:
        wih = nc.dram_tensor("wih", [65, 2, 4, 4, 128], BF16,
                             kind="ExternalInput")
        whh = nc.dram_tensor("whh", [128, 2, 4, 128], BF16,
                             kind="ExternalInput")
    else:
        wih = nc.dram_tensor("wih", [128, 8, 2, 128], BF16,
                             kind="ExternalInput")
        whh = nc.dram_tensor("whh", [128, 16, 128], BF16,
                             kind="ExternalInput")
        bih = nc.dram_tensor("bih", [128, 8], F32, kind="ExternalInput")
    ctw = nc.dram_tensor("ctw", [128, ND, 2, KC, 128], BF16,
                         kind="ExternalInput")
    ctb = nc.dram_tensor("ctb", [128, 2], F32, kind="ExternalInput")
    outu = nc.dram_tensor("outu", [128, 2, NT], F32, kind="ExternalOutput")

    ctx = contextlib.ExitStack()
    with tile.TileContext(nc) as tc, ctx:
        const = ctx.enter_context(tc.tile_pool(name="const", bufs=1))
        persist = ctx.enter_context(tc.tile_pool(name="persist", bufs=1))
        psum = ctx.enter_context(tc.tile_pool(name="psum", bufs=2,
                                              space="PSUM"))
        psumB = ctx.enter_context(tc.tile_pool(name="psumB", bufs=2,
                                               space="PSUM"))
        psumS = ctx.enter_context(tc.tile_pool(
            name="psumS", bufs=1 if intra else 2, space="PSUM"))

        eps_t = const.tile([128, 1], F32)
        nc.vector.memset(eps_t[:], EPS)
        ident = const.tile([128, 128], BF16)
        make_identity(nc, ident[:])

        if intra:
            wih_t = const.tile([65, 2, 4, 4, 128], BF16)
            whh_t = const.tile([128, 2, 4, 128], BF16)
        else:
            wih_t = const.tile([128, 8, 2, 128], BF16)
            whh_t = const.tile([128, 16, 128], BF16)
            bih_t = const.tile([128, 8], F32)
            nc.sync.dma_start(out=bih_t[:], in_=bih[:])
        nc.sync.dma_start(out=wih_t[:], in_=wih[:])
        nc.sync.dma_start(out=whh_t[:], in_=whh[:])
        ct_tl = const.tile([128, ND, 2, KC, 128], BF16)
        nc.sync.dma_start(out=ct_tl[:], in_=ctw[:])
        ctb_t = const.tile([128, 2], F32)
        nc.sync.dma_start(out=ctb_t[:], in_=ctb[:])

        # --- persistent tiles ---
        if intra:
            # z channel-major [65, ZC]; row 64 = ones (bias row)
            z_cm = persist.tile([65, ZC], BF16)
            hbufs = [persist.tile([128, L, NB], BF16, name=f"hb{d}")
                     for d in range(ND * KC)]
        else:
            # z doubled rows: p<64: z[c, t'-3]; p>=64: z[c, t'-2]
            z2d = persist.tile([128, RW2, 1000], BF16)
            hb2 = persist.tile([128, 2, L, NB], BF16)
        ou = persist.tile([128, 2, L, NB], F32)

        # --- LN over C (pos-major) + transpose to channel-major ---
        with tc.tile_pool(name="tmpA", bufs=1) as tmpA:
            xpm = tmpA.tile([128, G, C], BF16)
            nc.sync.dma_start(out=xpm[:], in_=x_pm[:])
            work = tmpA
            zpm = ln_posmajor(nc, tmpA, work, xpm, G, C, eps_t)
            if intra:
                z_dst = z_cm
                nc.gpsimd.memset(z_cm[64:65, :], 1.0)
            else:
                z_tmp = tmpA.tile([64, ZC], BF16)
                z_dst = z_tmp
            for g0 in range(0, G, 4):
                gn = min(4, G - g0)
                pt = psum.tile([64, 4, 128], BF16, tag="tps")
                for gg in range(gn):
                    nc.tensor.transpose(pt[:, gg, :], zpm[:, g0 + gg, :],
                                        ident[:])
                if (g0 // 4) % 2 == 0:
                    nc.scalar.copy(
                        out=z_dst[0:64, g0 * 128:(g0 + gn) * 128],
                        in_=pt[:, 0:gn, :].rearrange("p a b -> p (a b)"))
                else:
                    nc.vector.tensor_copy(
                        out=z_dst[0:64, g0 * 128:(g0 + gn) * 128],
                        in_=pt[:, 0:gn, :].rearrange("p a b -> p (a b)"))
            if not intra:
                # z_tmp [64, row*1000+t] -> z2d two shifted copies
                nc.vector.memset(z2d[0:64, :, 0:3], 0.0)
                nc.vector.memset(z2d[64:128, :, 0:2], 0.0)
                nc.sync.dma_start(
                    out=z2d[0:64, :, 3:1000],
                    in_=ap3(z_tmp, 0, 1000, RW2, 1, 997))
                nc.sync.dma_start(
                    out=z2d[64:128, :, 2:1000],
                    in_=ap3(z_tmp, 0, 1000, RW2, 1, 998))

        # --- gate precompute (inter only): pre2 [128, 8, RW2, L] bf16 ---
        rec = ctx.enter_context(tc.tile_pool(name="rec", bufs=1))
        if not intra:
            # l-major so per-step slices [128, 8, RW2] are contiguous
            pre2 = rec.tile([128, L, 8, RW2], BF16)
            LSUB = 30
            # l-chunk outer so the recurrence can start after one chunk
            for l0 in range(0, L, LSUB):
                ln_ = min(LSUB, L - l0)
                for m in range(8):
                    pp = psumB.tile([128, 512], F32, tag="ppre")
                    for kp in range(2):
                        rhs = ap3(z2d, 4 * l0 + 2 * kp, 4, ln_, 1000, RW2)
                        nc.tensor.matmul(pp[:, :RW2 * ln_],
                                         wih_t[:, m, kp, :], rhs,
                                         start=(kp == 0), stop=(kp == 1))
                    dst = pre2[:, l0:l0 + ln_, m, :]
                    src = pp[:, :RW2 * ln_].rearrange(
                        "p (l r) -> p l r", r=RW2)
                    if m % 2 == 0:
                        nc.vector.tensor_scalar_add(out=dst, in0=src,
                                                    scalar1=bih_t[:, m:m + 1])
                    else:
                        nc.scalar.activation(out=dst, in_=src,
                                             func=AF.Identity,
                                             bias=bih_t[:, m:m + 1])

        # --- recurrence (interleaved directions) ---
        NGC = MC // 4                      # hidden chunks (1 or 2)
        gsb = [rec.tile([128, MC, NB], BF16, name=f"gs{d}")
               for d in range(ND)]
        c_t = [rec.tile([128, NGC, NB], F32, name=f"ct{d}")
               for d in range(ND)]
        ig_t = [rec.tile([128, NGC, NB], BF16, name=f"ig{d}")
               for d in range(ND)]
        tc_t = [rec.tile([128, NGC, NB], BF16, name=f"tc{d}")
               for d in range(ND)]
        slot = 64 if NB <= 64 else 256

        def step(d, l, first):
            lp = l + 1 if (intra and d == 1) else l - 1
            ps = psumS.tile([128, MC, slot if intra else RW2], F32,
                            tag=f"lps{d}")
            if intra:
                for m in range(4):
                    st = Qp
                    for k in range(4):
                        o0 = 4 * l + k
                        rhs = bass.AP(
                            tensor=z_cm.tensor, offset=z_cm.offset + o0,
                            ap=[z_cm.ap[0], [st, NB]])
                        nc.tensor.matmul(ps[:, m, :NB],
                                         wih_t[:, d, m, k, :], rhs,
                                         start=(k == 0),
                                         stop=(k == 3 and first))
                    if not first:
                        nc.tensor.matmul(ps[:, m, :NB], whh_t[:, d, m, :],
                                         hbufs[d][:, lp, :],
                                         start=False, stop=True)
            else:
                if not first:
                    # inject pre2 into psum, then accumulate whh @ h
                    nc.tensor.matmul(
                        ps[:].rearrange("p m n -> p (m n)"), ident[:],
                        pre2[:, l, :, :].rearrange("p m n -> p (m n)"),
                        start=True, stop=False, skip_group_check=True)
                    for m in range(8):
                        for kc in range(2):
                            nc.tensor.matmul(ps[:, m, :NB],
                                             whh_t[:, m * 2 + kc, :],
                                             hb2[:, kc, lp, :],
                                             start=False, stop=(kc == 1),
                                             skip_group_check=True)
            # gates: i (NGC), f (NGC), o (NGC), g (NGC)
            gg = gsb[d]
            if first and not intra:
                sig_in = pre2[:, l, 0:3 * NGC, :]
                tanh_in = pre2[:, l, 3 * NGC:, :]
            else:
                sig_in = ps[:, 0:3 * NGC, :NB]
                tanh_in = ps[:, 3 * NGC:, :NB]
            nc.scalar.activation(out=gg[:, 0:3 * NGC, :], in_=sig_in,
                                 func=AF.Sigmoid)
            nc.scalar.activation(out=gg[:, 3 * NGC:, :], in_=tanh_in,
                                 func=AF.Tanh)
            i_g, f_g = gg[:, 0:NGC, :], gg[:, NGC:2 * NGC, :]
            o_g, g_g = gg[:, 2 * NGC:3 * NGC, :], gg[:, 3 * NGC:, :]
            if first:
                nc.vector.tensor_tensor(out=c_t[d][:], in0=i_g, in1=g_g,
                                        op=OP.mult)
            else:
                nc.vector.tensor_tensor(out=ig_t[d][:], in0=i_g, in1=g_g,
                                        op=OP.mult)
                nc.vector.tensor_tensor(out=c_t[d][:], in0=f_g, in1=c_t[d][:],
                                        op=OP.mult)
                nc.vector.tensor_tensor(out=c_t[d][:], in0=c_t[d][:],
                                        in1=ig_t[d][:], op=OP.add)
            nc.scalar.activation(out=tc_t[d][:], in_=c_t[d][:], func=AF.Tanh)
            if intra:
                nc.vector.tensor_tensor(out=hbufs[d][:, l, :], in0=o_g,
                                        in1=tc_t[d][:], op=OP.mult)
            else:
                nc.vector.tensor_tensor(out=hb2[:, :, l, :], in0=o_g,
                                        in1=tc_t[d][:], op=OP.mult)

        # --- ConvT + bias + residual, l-chunked, interleaved with steps ---
        xu_t = rec.tile([128, 2, NT], F32)
        nc.sync.dma_start(out=xu_t[:], in_=x_u[:])
        CL = 2 if intra else 30

        def convt_chunk(l0):
            ln_ = min(CL, L - l0)
            nn_ = ln_ * NB
            for mo in range(2):
                ps2 = psumB.tile([128, 512], F32, tag="pct")
                nch = 0
                for d in range(ND):
                    for k in range(KC):
                        hsl = (hbufs[d][:, l0:l0 + ln_, :] if intra
                               else hb2[:, k, l0:l0 + ln_, :])
                        nc.tensor.matmul(
                            ps2[:, :nn_], ct_tl[:, d, mo, k, :],
                            hsl.rearrange("p l t -> p (l t)"),
                            start=(nch == 0), stop=(nch == ND * KC - 1))
                        nch += 1
                nc.vector.scalar_tensor_tensor(
                    out=ou[:, mo, l0:l0 + ln_, :].rearrange(
                        "p l t -> p (l t)"),
                    in0=ps2[:, :nn_], scalar=ctb_t[:, mo:mo + 1],
                    in1=xu_t[:, mo, l0 * NB:l0 * NB + nn_],
                    op0=OP.add, op1=OP.add)

        pending = list(range(0, L, CL))
        for i in range(L):
            step(0, i, i == 0)
            if intra:
                step(1, L - 1 - i, i == 0)
            for l0 in list(pending):
                ln_ = min(CL, L - l0)
                ready = i >= l0 + ln_ - 1
                if intra:
                    ready = ready and i >= L - 1 - l0
                if ready:
                    convt_chunk(l0)
                    pending.remove(l0)
        for l0 in pending:
            convt_chunk(l0)
        nc.sync.dma_start(out=outu[:],
                          in_=ou[:].rearrange("p a l t -> p a (l t)"))
    nc.compile()
    return nc


# ---------------- Launch 3a: QKV conv + PReLU + LN ----------------

def build_l3a():
    nc = new_nc()
    icm = nc.dram_tensor("icm", [64, TSH, Qp], BF16, kind="ExternalInput")
    wall = nc.dram_tensor("wall", [64, 96], BF16, kind="ExternalInput")
    bs = nc.dram_tensor("bs", [96, 4], F32, kind="ExternalInput")
    # bs cols: bias, alpha, cnt_inv, gscale (per row)
    gmat = nc.dram_tensor("gmat", [96, 96], BF16, kind="ExternalInput")
    qkvo = nc.dram_tensor("qkvo", [96, TSH, Qp], BF16, kind="ExternalOutput")
    NTF = TSH * Qp  # 17000
    ctx = contextlib.ExitStack()
    with tile.TileContext(nc) as tc, ctx:
        const = ctx.enter_context(tc.tile_pool(name="const", bufs=1))
        big = ctx.enter_context(tc.tile_pool(name="big", bufs=1))
        work = ctx.enter_context(tc.tile_pool(name="work", bufs=2))
        psum = ctx.enter_context(tc.tile_pool(name="psum", bufs=2, space="PSUM"))
        eps_t = const.tile([96, 1], F32)
        nc.vector.memset(eps_t[:], EPS)
        ict = big.tile([64, NTF], BF16, tag="ict")
        nc.sync.dma_start(out=ict[:], in_=icm.rearrange("c t f -> c (t f)"))
        wt = const.tile([64, 96], BF16)
        nc.sync.dma_start(out=wt[:], in_=wall[:])
        bst = const.tile([96, 4], F32)
        nc.sync.dma_start(out=bst[:], in_=bs[:])
        gm = const.tile([96, 96], BF16)
        nc.sync.dma_start(out=gm[:], in_=gmat[:])

        qr = big.tile([96, NTF], F32, tag="qr")
        for n0 in range(0, NTF, 512):
            nn_ = min(512, NTF - n0)
            ps = psum.tile([96, 512], F32, tag="pc")
            nc.tensor.matmul(ps[:, :nn_], wt[:], ict[:, n0:n0 + nn_],
                             start=True, stop=True)
            nc.scalar.activation(out=qr[:, n0:n0 + nn_], in_=ps[:, :nn_],
                                 func=AF.Prelu, bias=bst[:, 0:1],
                                 alpha=bst[:, 1:2])
        # stats over (e,f) groups: reduce f, then group-collapse via gmat
        s1 = work.tile([96, TSH], F32, tag="s1")
        nc.vector.tensor_reduce(out=s1[:], in_=qr[:].rearrange(
            "p (t f) -> p t f", f=Qp), axis=AX.X, op=OP.add)
        sq = big.tile([96, NTF], BF16, tag="sq")
        nc.scalar.activation(out=sq[:], in_=qr[:], func=AF.Square)
        s2 = work.tile([96, TSH], F32, tag="s2")
        nc.vector.tensor_reduce(out=s2[:], in_=sq[:].rearrange(
            "p (t f) -> p t f", f=Qp), axis=AX.X, op=OP.add)
        s1b = work.tile([96, TSH], BF16, tag="s1b")
        nc.vector.tensor_copy(out=s1b[:], in_=s1[:])
        s2b = work.tile([96, TSH], BF16, tag="s2b")
        nc.vector.tensor_copy(out=s2b[:], in_=s2[:])
        mu = work.tile([96, TSH], F32, tag="mu")
        ps1 = psum.tile([96, TSH], F32, tag="pg1")
        nc.tensor.matmul(ps1[:], gm[:], s1b[:], start=True, stop=True)
        nc.vector.tensor_scalar_mul(out=mu[:], in0=ps1[:], scalar1=bst[:, 2:3])
        var = work.tile([96, TSH], F32, tag="var")
        ps2g = psum.tile([96, TSH], F32, tag="pg2")
        nc.tensor.matmul(ps2g[:], gm[:], s2b[:], start=True, stop=True)
        nc.vector.tensor_scalar_mul(out=var[:], in0=ps2g[:], scalar1=bst[:, 2:3])
        mu2 = work.tile([96, TSH], F32, tag="mu2")
        nc.vector.tensor_tensor(out=mu2[:], in0=mu[:], in1=mu[:], op=OP.mult)
        nc.vector.tensor_tensor(out=var[:], in0=var[:], in1=mu2[:],
                                op=OP.subtract)
        rs = work.tile([96, TSH], F32, tag="rs")
        nc.scalar.activation(out=rs[:], in_=var[:], func=AF.Sqrt, bias=eps_t[:])
        nc.vector.reciprocal(out=rs[:], in_=rs[:])
        nc.vector.tensor_scalar_mul(out=rs[:], in0=rs[:], scalar1=bst[:, 3:4])
        zh = big.tile([96, TSH, Qp], BF16, tag="zh")
        qr3 = qr[:].rearrange("p (t f) -> p t f", f=Qp)
        nc.vector.tensor_tensor(out=zh[:], in0=qr3, in1=bap(mu, [Qp]),
                                op=OP.subtract)
        nc.vector.tensor_tensor(out=zh[:], in0=zh[:], in1=bap(rs, [Qp]),
                                op=OP.mult)
        nc.vector.memset(zh[:, :, Q:Qp], 0.0)
        nc.sync.dma_start(out=qkvo[:], in_=zh[:])
    nc.compile()
    return nc


# ---------------- Launch 3b: attention per (h,b) ----------------

def build_l3b():
    nc = new_nc()
    # feature-major Q,K (host-transposed): [384, 1000] rows = 272QT+pad, etc
    qT = nc.dram_tensor("qT", [128, 3, T], BF16, kind="ExternalInput")
    kT = nc.dram_tensor("kT", [128, 3, T], BF16, kind="ExternalInput")
    vm = nc.dram_tensor("vm", [128, 8, Dv * Qp], BF16,
                        kind="ExternalInput")
    msk = nc.dram_tensor("msk", [128, 128], F32, kind="ExternalInput")
    avo = nc.dram_tensor("avo", [128, 8, Dv * Qp], BF16,
                         kind="ExternalOutput")
    DFv = Dv * Qp
    ctx = contextlib.ExitStack()
    with tile.TileContext(nc) as tc, ctx:
        const = ctx.enter_context(tc.tile_pool(name="const", bufs=1))
        big = ctx.enter_context(tc.tile_pool(name="big", bufs=1))
        work = ctx.enter_context(tc.tile_pool(name="work", bufs=3))
        psum = ctx.enter_context(tc.tile_pool(name="psum", bufs=2, space="PSUM"))
        psumB = ctx.enter_context(tc.tile_pool(name="psumB", bufs=1,
                                               space="PSUM"))
        ident = const.tile([128, 128], F32)
        make_identity(nc, ident[:])
        qt_t = big.tile([128, 3, T], BF16, tag="qt")
        nc.sync.dma_start(out=qt_t[:], in_=qT[:])
        kt_t = big.tile([128, 3, T], BF16, tag="kt")
        nc.sync.dma_start(out=kt_t[:], in_=kT[:])
        vm_t = big.tile([128, 8, DFv], BF16, tag="vm")
        nc.sync.dma_start(out=vm_t[:], in_=vm[:])
        msk_t = const.tile([128, 128], F32)
        nc.sync.dma_start(out=msk_t[:], in_=msk[:])

        for tcn in range(8):
            ns = min((tcn + 1) * 128, T)
            tch = min(128, T - tcn * 128)
            sc = big.tile([128, 1024], F32, tag="sc")
            for s0 in range(0, ns, 512):
                nn_ = min(512, ns - s0)
                ps = psum.tile([128, 512], F32, tag="psc")
                for kc in range(3):
                    nc.tensor.matmul(
                        ps[:tch, :nn_],
                        qt_t[:, kc, tcn * 128:tcn * 128 + tch],
                        kt_t[:, kc, s0:s0 + nn_],
                        start=(kc == 0), stop=(kc == 2))
                nc.vector.tensor_copy(out=sc[:tch, s0:s0 + nn_],
                                      in_=ps[:tch, :nn_])
            dw = ns - tcn * 128
            nc.vector.tensor_tensor(out=sc[:tch, tcn * 128:ns],
                                    in0=sc[:tch, tcn * 128:ns],
                                    in1=msk_t[:tch, :dw], op=OP.add)
            mx = work.tile([128, 1], F32, tag="mx")
            nc.vector.tensor_reduce(out=mx[:tch], in_=sc[:tch, :ns], axis=AX.X,
                                    op=OP.max)
            nc.vector.tensor_scalar_mul(out=mx[:tch], in0=mx[:tch],
                                        scalar1=-1.0)
            sme = work.tile([128, 1], F32, tag="sme")
            nc.scalar.activation(out=sc[:tch, :ns], in_=sc[:tch, :ns],
                                 func=AF.Exp, bias=mx[:tch],
                                 accum_out=sme[:tch])
            nc.vector.reciprocal(out=sme[:tch], in_=sme[:tch])
            av = psumB.tile([128, 3, 512], F32, tag="pav")
            for sb0 in range(0, tcn + 1, 4):
                sbn = min(4, tcn + 1 - sb0)
                pT = psum.tile([128, 4, 128], F32, tag="ptr")
                for j in range(sbn):
                    sb = sb0 + j
                    scb = min(128, ns - sb * 128)
                    nc.tensor.transpose(pT[:scb, j, :tch],
                                        sc[:tch, sb * 128:sb * 128 + scb],
                                        ident[:tch, :tch])
                aT = work.tile([128, 4, 128], BF16, tag="aT")
                nc.scalar.copy(out=aT[:, 0:sbn, :].rearrange("p a b -> p (a b)"),
                               in_=pT[:, 0:sbn, :].rearrange("p a b -> p (a b)"))
                for j in range(sbn):
                    sb = sb0 + j
                    scb = min(128, ns - sb * 128)
                    for n3 in range(3):
                        nn_ = min(512, DFv - n3 * 512)
                        nc.tensor.matmul(
                            av[:tch, n3, :nn_], aT[:scb, j, :tch],
                            vm_t[:scb, sb, n3 * 512:n3 * 512 + nn_],
                            start=(sb == 0), stop=(sb == tcn))
            avs = big.tile([128, DFv], BF16, tag="avs")
            av2 = bass.AP(tensor=av.tensor, offset=av.offset,
                          ap=[av.ap[0], [1, DFv]])
            nc.vector.tensor_scalar_mul(out=avs[:tch], in0=av2[:tch],
                                        scalar1=sme[:tch])
            nc.sync.dma_start(out=avo[:, tcn, :], in_=avs[:])
    nc.compile()
    return nc


# ---------------- Launch 3c: proj + out-LN + residual ----------------

def build_l3c():
    nc = new_nc()
    avf = nc.dram_tensor("avf", [64, TSH, Qp], BF16, kind="ExternalInput")
    icm = nc.dram_tensor("icm", [64, TSH, Qp], F32, kind="ExternalInput")
    pw = nc.dram_tensor("pw", [64, 64], BF16, kind="ExternalInput")
    pb = nc.dram_tensor("pb", [64, 3], F32, kind="ExternalInput")
    # pb cols: bias, gamma0*? , ... col0 bias, col1 alpha-scalar bcast
    outo = nc.dram_tensor("outo", [64, TSH, Q], F32, kind="ExternalOutput")
    NTF = TSH * Qp
    ctx = contextlib.ExitStack()
    with tile.TileContext(nc) as tc, ctx:
        const = ctx.enter_context(tc.tile_pool(name="const", bufs=1))
        big = ctx.enter_context(tc.tile_pool(name="big", bufs=1))
        work = ctx.enter_context(tc.tile_pool(name="work", bufs=1))
        psum = ctx.enter_context(tc.tile_pool(name="psum", bufs=2, space="PSUM"))
        eps_t = const.tile([128, 1], F32)
        nc.vector.memset(eps_t[:], EPS)
        ones_t = const.tile([64, 128], BF16)
        nc.vector.memset(ones_t[:], 1.0)
        avt = big.tile([64, NTF], BF16, tag="avt")
        nc.sync.dma_start(out=avt[:], in_=avf.rearrange("c t f -> c (t f)"))
        pwt = const.tile([64, 64], BF16)
        nc.sync.dma_start(out=pwt[:], in_=pw[:])
        pbt = const.tile([64, 3], F32)
        nc.sync.dma_start(out=pbt[:], in_=pb[:])

        P = big.tile([64, NTF], F32, tag="P")
        for n0 in range(0, NTF, 512):
            nn_ = min(512, NTF - n0)
            ps = psum.tile([64, 512], F32, tag="pp")
            nc.tensor.matmul(ps[:, :nn_], pwt[:], avt[:, n0:n0 + nn_],
                             start=True, stop=True)
            nc.scalar.activation(out=P[:, n0:n0 + nn_], in_=ps[:, :nn_],
                                 func=AF.Prelu, bias=pbt[:, 0:1],
                                 alpha=pbt[:, 1:2])
        P3 = P[:].rearrange("p (t f) -> p t f", f=Qp)
        nc.vector.memset(P3[:, :, Q:Qp], 0.0)
        s1 = work.tile([64, TSH], F32, tag="s1")
        nc.vector.tensor_reduce(out=s1[:], in_=P3, axis=AX.X, op=OP.add)
        sq = big.tile([64, NTF], BF16, tag="avt")
        nc.scalar.activation(out=sq[:], in_=P[:], func=AF.Square)
        s2 = work.tile([64, TSH], F32, tag="s2")
        nc.vector.tensor_reduce(out=s2[:], in_=sq[:].rearrange(
            "p (t f) -> p t f", f=Qp), axis=AX.X, op=OP.add)
        s1b = work.tile([64, TSH], BF16, tag="s1b")
        nc.vector.tensor_copy(out=s1b[:], in_=s1[:])
        s2b = work.tile([64, TSH], BF16, tag="s2b")
        nc.vector.tensor_copy(out=s2b[:], in_=s2[:])
        NCF = 64 * Q  # 4160
        mu = work.tile([128, TSH], F32, tag="mu")
        psg = psum.tile([128, TSH], F32, tag="pg")
        nc.tensor.matmul(psg[:], ones_t[:], s1b[:], start=True, stop=True)
        nc.vector.tensor_scalar_mul(out=mu[:], in0=psg[:], scalar1=1.0 / NCF)
        var = work.tile([128, TSH], F32, tag="var")
        psg2 = psum.tile([128, TSH], F32, tag="pg2")
        nc.tensor.matmul(psg2[:], ones_t[:], s2b[:], start=True, stop=True)
        nc.vector.tensor_scalar_mul(out=var[:], in0=psg2[:], scalar1=1.0 / NCF)
        mu2 = work.tile([128, TSH], F32, tag="mu2")
        nc.vector.tensor_tensor(out=mu2[:], in0=mu[:], in1=mu[:], op=OP.mult)
        nc.vector.tensor_tensor(out=var[:], in0=var[:], in1=mu2[:],
                                op=OP.subtract)
        rs = work.tile([128, TSH], F32, tag="rs")
        nc.scalar.activation(out=rs[:], in_=var[:], func=AF.Sqrt, bias=eps_t[:])
        nc.vector.reciprocal(out=rs[:], in_=rs[:])
        # out = (P - mu)*rs + inter
        o1 = big.tile([64, TSH, Qp], F32, tag="o1")
        nc.vector.tensor_tensor(out=o1[:], in0=P3, in1=bap(mu[0:64, :], [Qp]),
                                op=OP.subtract)
        nc.vector.tensor_tensor(out=o1[:], in0=o1[:], in1=bap(rs[0:64, :], [Qp]),
                                op=OP.mult)
        ict = big.tile([64, NTF], F32, tag="P")
        nc.sync.dma_start(out=ict[:], in_=icm.rearrange("c t f -> c (t f)"))
        nc.vector.tensor_tensor(out=o1[:], in0=o1[:],
                                in1=ict[:].rearrange("p (t f) -> p t f", f=Qp),
                                op=OP.add)
        nc.sync.dma_start(out=outo[:], in_=o1[:, :, :Q])
    nc.compile()
    return nc


# ======================= host side =======================

_CACHE = {}


def _posmajor(arr_pos_c, G):
    """[NPOS, nred] -> [128, G, nred] tiles, pos = g*128+p."""
    npos, nred = arr_pos_c.shape
    pad = np.zeros((G * 128, nred), arr_pos_c.dtype)
    pad[:npos] = arr_pos_c
    return np.ascontiguousarray(pad.reshape(G, 128, nred).transpose(1, 0, 2))


def _lstm_weight_prep(wih, whh, bih, bhh, ctw, ctb, gamma, beta, MC, KC):
    """Fold LN gamma/beta into wih/bias; build device layouts."""
    g = gamma.reshape(-1).astype(np.float64)   # [C]
    b = beta.reshape(-1).astype(np.float64)
    wih = np.asarray(wih, np.float64)          # [4H, C*KS]
    NH4 = wih.shape[0]
    w4 = wih.reshape(NH4, C, KS)
    wih_eff = w4 * g[None, :, None]
    bih_eff = (np.asarray(bih, np.float64) + np.asarray(bhh, np.float64)
               + (w4 * b[None, :, None]).sum((1, 2)))
    # device wih tile [MC, 4, 64, 128]: [m, k, c, gate-in-chunk]
    wt = np.zeros((MC, 4, 64, 128), np.float32)
    for m in range(MC):
        for k in range(4):
            wt[m, k] = wih_eff[m * 128:(m + 1) * 128, :, k].T
    # whh lhsT [MC*KC, 128, 128]: chunk (m,kc): whh[m*128:.., kc*128:..].T
    whh = np.asarray(whh, np.float64)
    wh = np.zeros((MC * KC, 128, 128), np.float32)
    for m in range(MC):
        for kc in range(KC):
            wh[m * KC + kc] = whh[m * 128:(m + 1) * 128,
                                  kc * 128:(kc + 1) * 128].T
    bih_t = np.zeros((128, MC), np.float32)
    for m in range(MC):
        bih_t[:, m] = bih_eff[m * 128:(m + 1) * 128]
    # convT: ctw [HIDd, 64, 4] -> [2, KC*128, 128] ; out rows (k',c) k'*64+c
    ctw = np.asarray(ctw, np.float64)
    KCc = ctw.shape[0] // 128
    ct = np.zeros((2, KCc * 128, 128), np.float32)
    for mo in range(2):
        for kp in range(2):
            for cc in range(64):
                j = kp * 64 + cc
                ct[mo, :, j] = ctw[:, cc, mo * 2 + kp]
    ctb_t = np.zeros((128, 2), np.float32)
    for mo in range(2):
        for kp in range(2):
            ctb_t[kp * 64:(kp + 1) * 64, mo] = np.asarray(ctb)
    return wt, wh, bih_t, ct, ctb_t


def _unf_rows(arr_c_t, L, off=0):
    """arr [64, NTIME] -> x_u [2, 128, L, NB] rows (k,c) k*64+c, cols (l, nb).
    value = arr[c, nb, 4l+k+off] where arr is [64, NB, NTIME-per-row]."""
    C_, NB, NT_ = arr_c_t.shape
    out = np.zeros((2, 128, L, NB), np.float32)
    for mo in range(2):
        for kp in range(2):
            k = mo * 2 + kp
            idx = 4 * np.arange(L) + k + off
            v = arr_c_t[:, :, :][:, :, idx]          # [64, NB, L]
            out[mo, kp * 64:(kp + 1) * 64] = v.transpose(0, 2, 1)
    return out


def _uniform(a):
    a = np.asarray(a)
    assert np.all(a == a.flat[0]), "nonuniform LN affine not supported"
    return float(a.flat[0])


def _prep_lstm_v2(wih, whh, bih, bhh, gamma, beta):
    """LN-folded, gate-reordered (i,f,o,g) weight arrays."""
    g = np.asarray(gamma, np.float64).reshape(-1)
    b = np.asarray(beta, np.float64).reshape(-1)
    NH4 = np.asarray(wih).shape[0]
    w4 = np.asarray(wih, np.float64).reshape(NH4, C, KS)
    wih_eff = w4 * g[None, :, None]
    bih_eff = (np.asarray(bih, np.float64) + np.asarray(bhh, np.float64)
               + (w4 * b[None, :, None]).sum((1, 2)))
    H = NH4 // 4
    perm = np.r_[0:H, H:2 * H, 3 * H:4 * H, 2 * H:3 * H]
    return wih_eff[perm], bih_eff[perm], np.asarray(whh, np.float64)[perm]


def kernel(**inputs):
    ii = {k: np.asarray(v) for k, v in inputs.items()}
    x = ii["x"].astype(np.float32)
    xp = np.zeros((B, C, T, Qp), np.float32)
    xp[:, :, :, :Q] = x

    if "l1" not in _CACHE:
        _CACHE["l1"] = build_lstm_launch("intra")
        _CACHE["l2"] = build_lstm_launch("inter")
        _CACHE["l3a"] = build_l3a()
        _CACHE["l3b"] = build_l3b()
        _CACHE["l3c"] = build_l3c()

    bf = lambda a: np.ascontiguousarray(a, dtype=np.float32).astype(
        mybir.dt.np(BF16))
    f32c = lambda a: np.ascontiguousarray(a, dtype=np.float32)

    # ---------- L1 ----------
    wts, whs = [], []
    for d in range(2):
        we, be, wp = _prep_lstm_v2(
            ii["intra_wih"][d], ii["intra_whh"][d], ii["intra_bih"][d],
            ii["intra_bhh"][d], ii["intra_gamma"], ii["intra_beta"])
        wt = np.zeros((65, 4, 4, 128), np.float32)
        wh = np.zeros((128, 4, 128), np.float32)
        for m in range(4):
            for k in range(4):
                wt[:64, m, k] = we[m * 128:(m + 1) * 128, :, k].T
            wt[64, m, 0] = be[m * 128:(m + 1) * 128]
            wh[:, m] = wp[m * 128:(m + 1) * 128].T
        wts.append(wt); whs.append(wh)
    _, _, _, ct1, ctb1 = _lstm_weight_prep(
        ii["intra_wih"][0], ii["intra_whh"][0], ii["intra_bih"][0],
        ii["intra_bhh"][0], ii["intra_ct_w"], ii["intra_ct_b"],
        ii["intra_gamma"], ii["intra_beta"], 4, 1)
    # intra ctw [256,64,4]: split fw rows 0:128, bw 128:256 across d
    ctw_i = np.asarray(ii["intra_ct_w"], np.float64)
    ct_d = np.zeros((2, 2, 128, 128), np.float32)
    for d in range(2):
        sub = ctw_i[d * 128:(d + 1) * 128]
        for mo in range(2):
            for kp in range(2):
                for cc in range(64):
                    ct_d[d, mo, :, kp * 64 + cc] = sub[:, cc, mo * 2 + kp]
    l1_w = {
        "wih": bf(np.stack(wts, axis=1)),
        "whh": bf(np.stack(whs, axis=1)),
        "ctw": bf(ct_d.reshape(2, 2, 1, 128, 128).transpose(3, 0, 1, 2, 4)),
        "ctb": f32c(ctb1),
    }
    l1_maps = []
    for core in range(NCORES):
        b = core // 4
        t0 = (core % 4) * TSH
        xs = xp[b, :, t0:t0 + TSH, :]                    # [C, TSH, Qp]
        x_pm = _posmajor(np.ascontiguousarray(
            xs.transpose(1, 2, 0)).reshape(NP1, C), G1)
        xu = _unf_rows(xs.transpose(0, 1, 2).reshape(C, TSH, Qp)
                       .transpose(0, 1, 2), L1)          # wait: per row=t
        # arr [64, NB=TSH, Qp]
        xu = _unf_rows(np.ascontiguousarray(xs.transpose(0, 1, 2)), L1)
        l1_maps.append({**l1_w, "x_pm": bf(x_pm),
                        "x_u": f32c(xu.reshape(2, 128, L1 * TSH)
                                    .transpose(1, 0, 2))})
    r1 = run_bass_kernel_spmd(_CACHE["l1"], l1_maps,
                              core_ids=list(range(NCORES))).results
    # reassemble intra [B, C, T, Qp]
    intra = np.zeros((B, C, T, Qp), np.float32)
    for core in range(NCORES):
        b = core // 4
        t0 = (core % 4) * TSH
        ou = r1[core]["outu"].transpose(1, 0, 2).reshape(2, 128, L1, TSH)
        for mo in range(2):
            for kp in range(2):
                k = mo * 2 + kp
                q_idx = 4 * np.arange(L1) + k
                intra[b, :, t0:t0 + TSH, q_idx] = \
                    ou[mo, kp * 64:(kp + 1) * 64].transpose(1, 0, 2)
    # ---------- L2 ----------
    _, _, _, ct2, ctb2 = _lstm_weight_prep(
        ii["inter_wih"], ii["inter_whh"], ii["inter_bih"], ii["inter_bhh"],
        ii["inter_ct_w"], ii["inter_ct_b"], ii["inter_gamma"],
        ii["inter_beta"], 8, 2)
    we2, be2, wp2 = _prep_lstm_v2(
        ii["inter_wih"], ii["inter_whh"], ii["inter_bih"], ii["inter_bhh"],
        ii["inter_gamma"], ii["inter_beta"])
    wih2 = np.zeros((128, 8, 2, 128), np.float32)
    whh2 = np.zeros((128, 16, 128), np.float32)
    bih2 = np.zeros((128, 8), np.float32)
    for m in range(8):
        rows = we2[m * 128:(m + 1) * 128]
        for kp in range(2):
            wih2[:64, m, kp] = rows[:, :, 2 * kp].T
            wih2[64:, m, kp] = rows[:, :, 2 * kp + 1].T
        for kc in range(2):
            whh2[:, m * 2 + kc] = wp2[m * 128:(m + 1) * 128,
                                      kc * 128:(kc + 1) * 128].T
        bih2[:, m] = be2[m * 128:(m + 1) * 128]
    l2_w = {"wih": bf(wih2), "whh": bf(whh2), "bih": f32c(bih2),
            "ctw": bf(ct2.reshape(2, 2, 128, 128).transpose(2, 0, 1, 3)
                      .reshape(128, 1, 2, 2, 128)),
            "ctb": f32c(ctb2)}
    l2_maps = []
    for core in range(NCORES):
        b = core // 4
        q0 = (core % 4) * RW2
        isl = intra[b, :, :, q0:q0 + RW2]                # [C, T, RW2]
        rows_ct = np.ascontiguousarray(isl.transpose(0, 2, 1))  # [C,RW2,T]
        x_pm = _posmajor(np.ascontiguousarray(
            rows_ct.transpose(1, 2, 0)).reshape(RW2 * T, C), G1)
        # x_u resid: value = intra[c, row, t=4l+k]
        xu = _unf_rows(rows_ct, L2, off=0)
        l2_maps.append({**l2_w, "x_pm": bf(x_pm),
                        "x_u": f32c(xu.reshape(2, 128, L2 * RW2)
                                    .transpose(1, 0, 2))})
    r2 = run_bass_kernel_spmd(_CACHE["l2"], l2_maps,
                              core_ids=list(range(NCORES))).results
    inter = np.zeros((B, C, T, Qp), np.float32)
    for core in range(NCORES):
        b = core // 4
        q0 = (core % 4) * RW2
        ou = r2[core]["outu"].transpose(1, 0, 2).reshape(2, 128, L2, RW2)
        for mo in range(2):
            for kp in range(2):
                k = mo * 2 + kp
                t_idx = 4 * np.arange(L2) + k
                inter[b, :, t_idx, q0:q0 + RW2] = \
                    ou[mo, kp * 64:(kp + 1) * 64].transpose(1, 0, 2)
    inter_r = np.zeros((B, C, T, Qp), np.float32)
    inter_r[:, :, :, :Q] = inter[:, :, :, :Q]            # real freqs only
    # ---------- L3a ----------
    qg = _uniform(ii["q_g"]); kg = _uniform(ii["k_g"]); vg = _uniform(ii["v_g"])
    assert _uniform(ii["q_bt"]) == 0 and _uniform(ii["k_bt"]) == 0
    assert _uniform(ii["v_bt"]) == 0
    wall = np.zeros((64, 96), np.float32)
    bias96 = np.zeros((96,), np.float32)
    alpha96 = np.zeros((96,), np.float32)
    cnt96 = np.zeros((96,), np.float32)
    gs96 = np.zeros((96,), np.float32)
    grp = np.zeros((96,), np.int32)
    for h in range(NH):
        wall[:, h * 4:h * 4 + 4] = np.asarray(ii["q_w"][h]).T
        wall[:, 16 + h * 4:16 + h * 4 + 4] = np.asarray(ii["k_w"][h]).T
        wall[:, 32 + h * 16:32 + h * 16 + 16] = np.asarray(ii["v_w"][h]).T
        bias96[h * 4:h * 4 + 4] = np.asarray(ii["q_b"][h])
        bias96[16 + h * 4:16 + h * 4 + 4] = np.asarray(ii["k_b"][h])
        alpha96[h * 4:h * 4 + 4] = float(ii["q_p"][h])
        alpha96[16 + h * 4:16 + h * 4 + 4] = float(ii["k_p"][h])
        alpha96[32 + h * 16:32 + h * 16 + 16] = float(ii["v_p"][h])
        cnt96[h * 4:h * 4 + 4] = 1.0 / (E * Q)
        cnt96[16 + h * 4:16 + h * 4 + 4] = 1.0 / (E * Q)
        cnt96[32 + h * 16:32 + h * 16 + 16] = 1.0 / (Dv * Q)
        gs96[h * 4:h * 4 + 4] = qg / np.sqrt(E * Q)
        gs96[16 + h * 4:16 + h * 4 + 4] = kg
        gs96[32 + h * 16:32 + h * 16 + 16] = vg
        grp[h * 4:h * 4 + 4] = h
        grp[16 + h * 4:16 + h * 4 + 4] = 4 + h
        grp[32 + h * 16:32 + h * 16 + 16] = 8 + h
    gmat = (grp[:, None] == grp[None, :]).astype(np.float32)
    bs96 = np.stack([bias96, alpha96, cnt96, gs96], axis=1)
    l3a_w = {"wall": bf(wall), "bs": f32c(bs96), "gmat": bf(gmat)}
    l3a_maps = []
    for core in range(NCORES):
        b = core // 4
        t0 = (core % 4) * TSH
        l3a_maps.append({**l3a_w,
                         "icm": bf(inter_r[b, :, t0:t0 + TSH, :])})
    r3a = run_bass_kernel_spmd(_CACHE["l3a"], l3a_maps,
                               core_ids=list(range(NCORES))).results
    qkv = np.zeros((B, 96, T, Qp), np.float32)
    for core in range(NCORES):
        b = core // 4
        t0 = (core % 4) * TSH
        qkv[b, :, t0:t0 + TSH, :] = r3a[core]["qkvo"].astype(
            np.float32).transpose(0, 1, 2)
    # ---------- L3b ----------
    mask = np.triu(np.full((128, 128), -1e9, np.float32), 1)
    l3b_maps = []
    for core in range(NCORES):
        h, b = core % 4, core // 4
        qh = qkv[b, h * 4:h * 4 + 4]                     # [4, T, Qp]
        kh = qkv[b, 16 + h * 4:16 + h * 4 + 4]
        vh = qkv[b, 32 + h * 16:32 + h * 16 + 16]        # [16, T, Qp]
        qT = np.zeros((384, T), np.float32)
        kT = np.zeros((384, T), np.float32)
        qT[:4 * Qp] = qh.transpose(0, 2, 1).reshape(4 * Qp, T)
        kT[:4 * Qp] = kh.transpose(0, 2, 1).reshape(4 * Qp, T)
        vm = np.zeros((8, 128, Dv * Qp), np.float32)
        vflat = vh.transpose(1, 0, 2).reshape(T, Dv * Qp)  # [s, (d,f)]
        vm.reshape(1024, Dv * Qp)[:T] = vflat
        l3b_maps.append({"qT": bf(qT.reshape(3, 128, T).transpose(1, 0, 2)),
                         "kT": bf(kT.reshape(3, 128, T).transpose(1, 0, 2)),
                         "vm": bf(vm.transpose(1, 0, 2)), "msk": f32c(mask)})
    r3b = run_bass_kernel_spmd(_CACHE["l3b"], l3b_maps,
                               core_ids=list(range(NCORES))).results
    # av: [B, (h,d) 64, T, Qp]
    av = np.zeros((B, 64, T, Qp), np.float32)
    for core in range(NCORES):
        h, b = core % 4, core // 4
        a_ = r3b[core]["avo"].astype(np.float32).transpose(1, 0, 2)\
            .reshape(1024, Dv, Qp)[:T]
        av[b, h * 16:(h + 1) * 16] = a_.transpose(1, 0, 2)
    # ---------- L3c ----------
    assert _uniform(ii["proj_g"]) == 1.0 and _uniform(ii["proj_bt"]) == 0.0
    pw = np.asarray(ii["proj_w"], np.float32).T          # lhsT [hd, c]
    pb3 = np.zeros((64, 3), np.float32)
    pb3[:, 0] = np.asarray(ii["proj_b"])
    pb3[:, 1] = float(ii["proj_p"])
    l3c_w = {"pw": bf(pw), "pb": f32c(pb3)}
    l3c_maps = []
    for core in range(NCORES):
        b = core // 4
        t0 = (core % 4) * TSH
        l3c_maps.append({**l3c_w, "avf": bf(av[b, :, t0:t0 + TSH, :]),
                         "icm": f32c(inter_r[b, :, t0:t0 + TSH, :])})
    r3c = run_bass_kernel_spmd(_CACHE["l3c"], l3c_maps,
                               core_ids=list(range(NCORES))).results
    out = np.zeros((B, C, T, Q), np.float32)
    for core in range(NCORES):
        b = core // 4
        t0 = (core % 4) * TSH
        out[b, :, t0:t0 + TSH, :] = r3c[core]["outo"]
    kernel.dbg = {"intra": intra, "inter": inter, "qkv": qkv, "av": av}
    return out



# revision 4
# speedup vs baseline: 15.5215x; 15.5215x over previous
"""GridNetBlock (TF-GridNet) Trainium2 kernel: 8-core SPMD.

v2: fully device-resident pipeline. The five Bass stage kernels from v1
are unchanged, but all inter-stage glue (unfold/reshard/transpose) now
runs on-device as jitted jnp ops, weights are uploaded once and cached,
and host<->device traffic per call is only x (f16 in) + output (f16
out). The axon tunnel moves ~33MB/s, so this is what dominates wall.
"""
import sys, os, contextlib
for _p in ("/opt/trn_rl_repo", "/root/.axon_site/_ro/trn_rl_repo"):
    if os.path.isdir(_p) and _p not in sys.path:
        sys.path.insert(0, _p)
import numpy as np
import jax
import jax.numpy as jnp
from jax.sharding import Mesh, PartitionSpec, NamedSharding
from jax.experimental.shard_map import shard_map
import concourse.bass as bass
import concourse.bacc as bacc
import concourse.tile as tile
from concourse import mybir
from concourse.masks import make_identity
from concourse.bass2jax import (_bass_exec_p, install_neuronx_cc_hook,
                                partition_id_tensor)

F32 = mybir.dt.float32
BF16 = mybir.dt.bfloat16
AF = mybir.ActivationFunctionType
OP = mybir.AluOpType
AX = mybir.AxisListType

B, C, T, Q = 2, 64, 1000, 65
KS = 4
Qp, L1, Hh, HID, L2 = 68, 17, 128, 256, 250
NH, E, Dv = 4, 4, 16
EPS = 1e-5
NCORES = 8
TSH = T // 4
NP1 = TSH * Qp
G1 = (NP1 + 127) // 128   # 133
RW2 = (B * Qp) // NCORES  # 17
NT1 = L1 * TSH            # 4250
NT2 = L2 * RW2            # 4250


def bap(t, tail):
    ap = list(t.ap)
    for n in tail:
        ap.append([0, n])
    return bass.AP(tensor=t.tensor, offset=t.offset, ap=ap)


def new_nc():
    return bacc.Bacc("TRN2", target_bir_lowering=False, debug=False,
                     enable_asserts=True, num_devices=NCORES)


def ln_posmajor(nc, pool, work, xpm, G, nred, eps_t):
    s1 = work.tile([128, G], F32, tag="lns1")
    nc.vector.tensor_reduce(out=s1[:], in_=xpm[:], axis=AX.X, op=OP.add)
    xsq = pool.tile([128, G, nred], BF16, tag="xut")
    nc.scalar.activation(out=xsq[:], in_=xpm[:], func=AF.Square)
    s2 = work.tile([128, G], F32, tag="lns2")
    nc.vector.tensor_reduce(out=s2[:], in_=xsq[:], axis=AX.X, op=OP.add)
    mu = work.tile([128, G], F32, tag="lnmu")
    nc.vector.tensor_scalar_mul(out=mu[:], in0=s1[:], scalar1=1.0 / nred)
    var = work.tile([128, G], F32, tag="lnvar")
    nc.vector.tensor_tensor(out=var[:], in0=mu[:], in1=mu[:], op=OP.mult)
    nc.vector.scalar_tensor_tensor(out=var[:], in0=s2[:], scalar=1.0 / nred,
                                   in1=var[:], op0=OP.mult, op1=OP.subtract)
    rs = work.tile([128, G], F32, tag="lnrs")
    nc.scalar.activation(out=rs[:], in_=var[:], func=AF.Sqrt, bias=eps_t[:])
    nc.vector.reciprocal(out=rs[:], in_=rs[:])
    zpm = pool.tile([128, G, nred], BF16, tag="xut")
    nc.vector.tensor_tensor(out=zpm[:], in0=xpm[:], in1=bap(mu, [nred]),
                            op=OP.subtract)
    nc.vector.tensor_tensor(out=zpm[:], in0=zpm[:], in1=bap(rs, [nred]),
                            op=OP.mult)
    return zpm


def ap3(t, off, d1, n1, d2, n2):
    """Strided 2-free-dim AP view of tile t at element offset off."""
    return bass.AP(tensor=t.tensor, offset=t.offset + off,
                   ap=[t.ap[0], [d1, n1], [d2, n2]])


def build_lstm_launch(which):
    """which: 'intra' or 'inter'. Returns compiled nc."""
    intra = which == "intra"
    ND = 2 if intra else 1
    MC = 4 if intra else 8
    KC = 1 if intra else 2
    L = L1 if intra else L2
    NB = TSH if intra else RW2        # lstm batch per core
    NT = L * NB                       # 4250
    G = G1
    ZC = G * 128

    nc = new_nc()
    x_pm = nc.dram_tensor("x_pm", [128, G, C], BF16, kind="ExternalInput")
    x_u = nc.dram_tensor("x_u", [128, 2, NT], F32, kind="ExternalInput")
    if intra:
        wih = nc.dram_tensor("wih", [65, 2, 4, 4, 128], BF16,
                             kind="ExternalInput")
        whh = nc.dram_tensor("whh", [128, 2, 4, 128], BF16,
                             kind="ExternalInput")
    else:
        wih = nc.dram_tensor("wih", [128, 8, 2, 128], BF16,
                             kind="ExternalInput")
        whh = nc.dram_tensor("whh", [128, 16, 128], BF16,
                             kind="ExternalInput")
        bih = nc.dram_tensor("bih", [128, 8], F32, kind="ExternalInput")
    ctw = nc.dram_tensor("ctw", [128, ND, 2, KC, 128], BF16,
                         kind="ExternalInput")
    ctb = nc.dram_tensor("ctb", [128, 2], F32, kind="ExternalInput")
    outu = nc.dram_tensor("outu", [128, 2, NT], F32, kind="ExternalOutput")

    ctx = contextlib.ExitStack()
    with tile.TileContext(nc) as tc, ctx:
        const = ctx.enter_context(tc.tile_pool(name="const", bufs=1))
        persist = ctx.enter_context(tc.tile_pool(name="persist", bufs=1))
        psum = ctx.enter_context(tc.tile_pool(name="psum", bufs=2,
                                              space="PSUM"))
        psumB = ctx.enter_context(tc.tile_pool(name="psumB", bufs=2,
                                               space="PSUM"))
        psumS = ctx.enter_context(tc.tile_pool(
            name="psumS", bufs=1 if intra else 2, space="PSUM"))

        eps_t = const.tile([128, 1], F32)
        nc.vector.memset(eps_t[:], EPS)
        ident = const.tile([128, 128], BF16)
        make_identity(nc, ident[:])

        if intra:
            wih_t = const.tile([65, 2, 4, 4, 128], BF16)
            whh_t = const.tile([128, 2, 4, 128], BF16)
        else:
            wih_t = const.tile([128, 8, 2, 128], BF16)
            whh_t = const.tile([128, 16, 128], BF16)
            bih_t = const.tile([128, 8], F32)
            nc.sync.dma_start(out=bih_t[:], in_=bih[:])
        nc.sync.dma_start(out=wih_t[:], in_=wih[:])
        nc.sync.dma_start(out=whh_t[:], in_=whh[:])
        ct_tl = const.tile([128, ND, 2, KC, 128], BF16)
        nc.sync.dma_start(out=ct_tl[:], in_=ctw[:])
        ctb_t = const.tile([128, 2], F32)
        nc.sync.dma_start(out=ctb_t[:], in_=ctb[:])

        # --- persistent tiles ---
        if intra:
            # z channel-major [65, ZC]; row 64 = ones (bias row)
            z_cm = persist.tile([65, ZC], BF16)
            hbufs = [persist.tile([128, L, NB], BF16, name=f"hb{d}")
                     for d in range(ND * KC)]
        else:
            # z doubled rows: p<64: z[c, t'-3]; p>=64: z[c, t'-2]
            z2d = persist.tile([128, RW2, 1000], BF16)
            hb2 = persist.tile([128, 2, L, NB], BF16)
        ou = persist.tile([128, 2, L, NB], F32)

        # --- LN over C (pos-major) + transpose to channel-major ---
        with tc.tile_pool(name="tmpA", bufs=1) as tmpA:
            xpm = tmpA.tile([128, G, C], BF16)
            nc.sync.dma_start(out=xpm[:], in_=x_pm[:])
            work = tmpA
            zpm = ln_posmajor(nc, tmpA, work, xpm, G, C, eps_t)
            if intra:
                z_dst = z_cm
                nc.gpsimd.memset(z_cm[64:65, :], 1.0)
            else:
                z_tmp = tmpA.tile([64, ZC], BF16)
                z_dst = z_tmp
            for g0 in range(0, G, 4):
                gn = min(4, G - g0)
                pt = psum.tile([64, 4, 128], BF16, tag="tps")
                for gg in range(gn):
                    nc.tensor.transpose(pt[:, gg, :], zpm[:, g0 + gg, :],
                                        ident[:])
                if (g0 // 4) % 2 == 0:
                    nc.scalar.copy(
                        out=z_dst[0:64, g0 * 128:(g0 + gn) * 128],
                        in_=pt[:, 0:gn, :].rearrange("p a b -> p (a b)"))
                else:
                    nc.vector.tensor_copy(
                        out=z_dst[0:64, g0 * 128:(g0 + gn) * 128],
                        in_=pt[:, 0:gn, :].rearrange("p a b -> p (a b)"))
            if not intra:
                # z_tmp [64, row*1000+t] -> z2d two shifted copies
                nc.vector.memset(z2d[0:64, :, 0:3], 0.0)
                nc.vector.memset(z2d[64:128, :, 0:2], 0.0)
                nc.sync.dma_start(
                    out=z2d[0:64, :, 3:1000],
                    in_=ap3(z_tmp, 0, 1000, RW2, 1, 997))
                nc.sync.dma_start(
                    out=z2d[64:128, :, 2:1000],
                    in_=ap3(z_tmp, 0, 1000, RW2, 1, 998))

        # --- gate precompute (inter only): pre2 [128, L, 8, RW2] bf16 ---
        rec = ctx.enter_context(tc.tile_pool(name="rec", bufs=1))
        if not intra:
            pre2 = rec.tile([128, L, 8, RW2], BF16)
            LSUB = 30
            for l0 in range(0, L, LSUB):
                ln_ = min(LSUB, L - l0)
                for m in range(8):
                    pp = psumB.tile([128, 512], F32, tag="ppre")
                    for kp in range(2):
                        rhs = ap3(z2d, 4 * l0 + 2 * kp, 4, ln_, 1000, RW2)
                        nc.tensor.matmul(pp[:, :RW2 * ln_],
                                         wih_t[:, m, kp, :], rhs,
                                         start=(kp == 0), stop=(kp == 1))
                    dst = pre2[:, l0:l0 + ln_, m, :]
                    src = pp[:, :RW2 * ln_].rearrange(
                        "p (l r) -> p l r", r=RW2)
                    if m % 2 == 0:
                        nc.vector.tensor_scalar_add(out=dst, in0=src,
                                                    scalar1=bih_t[:, m:m + 1])
                    else:
                        nc.scalar.activation(out=dst, in_=src,
                                             func=AF.Identity,
                                             bias=bih_t[:, m:m + 1])

        # --- recurrence (interleaved directions) ---
        NGC = MC // 4                      # hidden chunks (1 or 2)
        gsb = [rec.tile([128, MC, NB], BF16, name=f"gs{d}")
               for d in range(ND)]
        c_t = [rec.tile([128, NGC, NB], F32, name=f"ct{d}")
               for d in range(ND)]
        ig_t = [rec.tile([128, NGC, NB], BF16, name=f"ig{d}")
               for d in range(ND)]
        tc_t = [rec.tile([128, NGC, NB], BF16, name=f"tc{d}")
               for d in range(ND)]
        slot = 64 if NB <= 64 else 256

        def step(d, l, first):
            lp = l + 1 if (intra and d == 1) else l - 1
            ps = psumS.tile([128, MC, slot if intra else RW2], F32,
                            tag=f"lps{d}")
            if intra:
                for m in range(4):
                    st = Qp
                    for k in range(4):
                        o0 = 4 * l + k
                        rhs = bass.AP(
                            tensor=z_cm.tensor, offset=z_cm.offset + o0,
                            ap=[z_cm.ap[0], [st, NB]])
                        nc.tensor.matmul(ps[:, m, :NB],
                                         wih_t[:, d, m, k, :], rhs,
                                         start=(k == 0),
                                         stop=(k == 3 and first))
                    if not first:
                        nc.tensor.matmul(ps[:, m, :NB], whh_t[:, d, m, :],
                                         hbufs[d][:, lp, :],
                                         start=False, stop=True)
            else:
                if not first:
                    nc.tensor.matmul(
                        ps[:].rearrange("p m n -> p (m n)"), ident[:],
                        pre2[:, l, :, :].rearrange("p m n -> p (m n)"),
                        start=True, stop=False, skip_group_check=True)
                    for m in range(8):
                        for kc in range(2):
                            nc.tensor.matmul(ps[:, m, :NB],
                                             whh_t[:, m * 2 + kc, :],
                                             hb2[:, kc, lp, :],
                                             start=False, stop=(kc == 1),
                                             skip_group_check=True)
            # gates: i (NGC), f (NGC), o (NGC), g (NGC)
            gg = gsb[d]
            if first and not intra:
                sig_in = pre2[:, l, 0:3 * NGC, :]
                tanh_in = pre2[:, l, 3 * NGC:, :]
            else:
                sig_in = ps[:, 0:3 * NGC, :NB]
                tanh_in = ps[:, 3 * NGC:, :NB]
            nc.scalar.activation(out=gg[:, 0:3 * NGC, :], in_=sig_in,
                                 func=AF.Sigmoid)
            nc.scalar.activation(out=gg[:, 3 * NGC:, :], in_=tanh_in,
                                 func=AF.Tanh)
            i_g, f_g = gg[:, 0:NGC, :], gg[:, NGC:2 * NGC, :]
            o_g, g_g = gg[:, 2 * NGC:3 * NGC, :], gg[:, 3 * NGC:, :]
            if first:
                nc.vector.tensor_tensor(out=c_t[d][:], in0=i_g, in1=g_g,
                                        op=OP.mult)
            else:
                nc.vector.tensor_tensor(out=ig_t[d][:], in0=i_g, in1=g_g,
                                        op=OP.mult)
                nc.vector.tensor_tensor(out=c_t[d][:], in0=f_g, in1=c_t[d][:],
                                        op=OP.mult)
                nc.vector.tensor_tensor(out=c_t[d][:], in0=c_t[d][:],
                                        in1=ig_t[d][:], op=OP.add)
            nc.scalar.activation(out=tc_t[d][:], in_=c_t[d][:], func=AF.Tanh)
            if intra:
                nc.vector.tensor_tensor(out=hbufs[d][:, l, :], in0=o_g,
                                        in1=tc_t[d][:], op=OP.mult)
            else:
                nc.vector.tensor_tensor(out=hb2[:, :, l, :], in0=o_g,
                                        in1=tc_t[d][:], op=OP.mult)

        # --- ConvT + bias + residual, l-chunked, interleaved with steps ---
        xu_t = rec.tile([128, 2, NT], F32)
        nc.sync.dma_start(out=xu_t[:], in_=x_u[:])
        CL = 2 if intra else 30

        def convt_chunk(l0):
            ln_ = min(CL, L - l0)
            nn_ = ln_ * NB
            for mo in range(2):
                ps2 = psumB.tile([128, 512], F32, tag="pct")
                nch = 0
                for d in range(ND):
                    for k in range(KC):
                        hsl = (hbufs[d][:, l0:l0 + ln_, :] if intra
                               else hb2[:, k, l0:l0 + ln_, :])
                        nc.tensor.matmul(
                            ps2[:, :nn_], ct_tl[:, d, mo, k, :],
                            hsl.rearrange("p l t -> p (l t)"),
                            start=(nch == 0), stop=(nch == ND * KC - 1))
                        nch += 1
                nc.vector.scalar_tensor_tensor(
                    out=ou[:, mo, l0:l0 + ln_, :].rearrange(
                        "p l t -> p (l t)"),
                    in0=ps2[:, :nn_], scalar=ctb_t[:, mo:mo + 1],
                    in1=xu_t[:, mo, l0 * NB:l0 * NB + nn_],
                    op0=OP.add, op1=OP.add)

        pending = list(range(0, L, CL))
        for i in range(L):
            step(0, i, i == 0)
            if intra:
                step(1, L - 1 - i, i == 0)
            for l0 in list(pending):
                ln_ = min(CL, L - l0)
                ready = i >= l0 + ln_ - 1
                if intra:
                    ready = ready and i >= L - 1 - l0
                if ready:
                    convt_chunk(l0)
                    pending.remove(l0)
        for l0 in pending:
            convt_chunk(l0)
        nc.sync.dma_start(out=outu[:],
                          in_=ou[:].rearrange("p a l t -> p a (l t)"))
    nc.compile()
    return nc


# ---------------- Launch 3a: QKV conv + PReLU + LN ----------------

def build_l3a():
    nc = new_nc()
    icm = nc.dram_tensor("icm", [64, TSH, Qp], BF16, kind="ExternalInput")
    wall = nc.dram_tensor("wall", [64, 96], BF16, kind="ExternalInput")
    bs = nc.dram_tensor("bs", [96, 4], F32, kind="ExternalInput")
    # bs cols: bias, alpha, cnt_inv, gscale (per row)
    gmat = nc.dram_tensor("gmat", [96, 96], BF16, kind="ExternalInput")
    qkvo = nc.dram_tensor("qkvo", [96, TSH, Qp], BF16, kind="ExternalOutput")
    NTF = TSH * Qp  # 17000
    ctx = contextlib.ExitStack()
    with tile.TileContext(nc) as tc, ctx:
        const = ctx.enter_context(tc.tile_pool(name="const", bufs=1))
        big = ctx.enter_context(tc.tile_pool(name="big", bufs=1))
        work = ctx.enter_context(tc.tile_pool(name="work", bufs=2))
        psum = ctx.enter_context(tc.tile_pool(name="psum", bufs=2, space="PSUM"))
        eps_t = const.tile([96, 1], F32)
        nc.vector.memset(eps_t[:], EPS)
        ict = big.tile([64, NTF], BF16, tag="ict")
        nc.sync.dma_start(out=ict[:], in_=icm.rearrange("c t f -> c (t f)"))
        wt = const.tile([64, 96], BF16)
        nc.sync.dma_start(out=wt[:], in_=wall[:])
        bst = const.tile([96, 4], F32)
        nc.sync.dma_start(out=bst[:], in_=bs[:])
        gm = const.tile([96, 96], BF16)
        nc.sync.dma_start(out=gm[:], in_=gmat[:])

        qr = big.tile([96, NTF], F32, tag="qr")
        for n0 in range(0, NTF, 512):
            nn_ = min(512, NTF - n0)
            ps = psum.tile([96, 512], F32, tag="pc")
            nc.tensor.matmul(ps[:, :nn_], wt[:], ict[:, n0:n0 + nn_],
                             start=True, stop=True)
            nc.scalar.activation(out=qr[:, n0:n0 + nn_], in_=ps[:, :nn_],
                                 func=AF.Prelu, bias=bst[:, 0:1],
                                 alpha=bst[:, 1:2])
        # stats over (e,f) groups: reduce f, then group-collapse via gmat
        s1 = work.tile([96, TSH], F32, tag="s1")
        nc.vector.tensor_reduce(out=s1[:], in_=qr[:].rearrange(
            "p (t f) -> p t f", f=Qp), axis=AX.X, op=OP.add)
        sq = big.tile([96, NTF], BF16, tag="sq")
        nc.scalar.activation(out=sq[:], in_=qr[:], func=AF.Square)
        s2 = work.tile([96, TSH], F32, tag="s2")
        nc.vector.tensor_reduce(out=s2[:], in_=sq[:].rearrange(
            "p (t f) -> p t f", f=Qp), axis=AX.X, op=OP.add)
        s1b = work.tile([96, TSH], BF16, tag="s1b")
        nc.vector.tensor_copy(out=s1b[:], in_=s1[:])
        s2b = work.tile([96, TSH], BF16, tag="s2b")
        nc.vector.tensor_copy(out=s2b[:], in_=s2[:])
        mu = work.tile([96, TSH], F32, tag="mu")
        ps1 = psum.tile([96, TSH], F32, tag="pg1")
        nc.tensor.matmul(ps1[:], gm[:], s1b[:], start=True, stop=True)
        nc.vector.tensor_scalar_mul(out=mu[:], in0=ps1[:], scalar1=bst[:, 2:3])
        var = work.tile([96, TSH], F32, tag="var")
        ps2g = psum.tile([96, TSH], F32, tag="pg2")
        nc.tensor.matmul(ps2g[:], gm[:], s2b[:], start=True, stop=True)
        nc.vector.tensor_scalar_mul(out=var[:], in0=ps2g[:], scalar1=bst[:, 2:3])
        mu2 = work.tile([96, TSH], F32, tag="mu2")
        nc.vector.tensor_tensor(out=mu2[:], in0=mu[:], in1=mu[:], op=OP.mult)
        nc.vector.tensor_tensor(out=var[:], in0=var[:], in1=mu2[:],
                                op=OP.subtract)
        rs = work.tile([96, TSH], F32, tag="rs")
        nc.scalar.activation(out=rs[:], in_=var[:], func=AF.Sqrt, bias=eps_t[:])
        nc.vector.reciprocal(out=rs[:], in_=rs[:])
        nc.vector.tensor_scalar_mul(out=rs[:], in0=rs[:], scalar1=bst[:, 3:4])
        zh = big.tile([96, TSH, Qp], BF16, tag="zh")
        qr3 = qr[:].rearrange("p (t f) -> p t f", f=Qp)
        nc.vector.tensor_tensor(out=zh[:], in0=qr3, in1=bap(mu, [Qp]),
                                op=OP.subtract)
        nc.vector.tensor_tensor(out=zh[:], in0=zh[:], in1=bap(rs, [Qp]),
                                op=OP.mult)
        nc.vector.memset(zh[:, :, Q:Qp], 0.0)
        nc.sync.dma_start(out=qkvo[:], in_=zh[:])
    nc.compile()
    return nc


# ---------------- Launch 3b: attention per (h,b) ----------------

def build_l3b():
    nc = new_nc()
    qT = nc.dram_tensor("qT", [128, 3, T], BF16, kind="ExternalInput")
    kT = nc.dram_tensor("kT", [128, 3, T], BF16, kind="ExternalInput")
    vm = nc.dram_tensor("vm", [128, 8, Dv * Qp], BF16,
                        kind="ExternalInput")
    msk = nc.dram_tensor("msk", [128, 128], F32, kind="ExternalInput")
    avo = nc.dram_tensor("avo", [128, 8, Dv * Qp], BF16,
                         kind="ExternalOutput")
    DFv = Dv * Qp
    ctx = contextlib.ExitStack()
    with tile.TileContext(nc) as tc, ctx:
        const = ctx.enter_context(tc.tile_pool(name="const", bufs=1))
        big = ctx.enter_context(tc.tile_pool(name="big", bufs=1))
        work = ctx.enter_context(tc.tile_pool(name="work", bufs=3))
        psum = ctx.enter_context(tc.tile_pool(name="psum", bufs=2, space="PSUM"))
        psumB = ctx.enter_context(tc.tile_pool(name="psumB", bufs=1,
                                               space="PSUM"))
        ident = const.tile([128, 128], F32)
        make_identity(nc, ident[:])
        qt_t = big.tile([128, 3, T], BF16, tag="qt")
        nc.sync.dma_start(out=qt_t[:], in_=qT[:])
        kt_t = big.tile([128, 3, T], BF16, tag="kt")
        nc.sync.dma_start(out=kt_t[:], in_=kT[:])
        vm_t = big.tile([128, 8, DFv], BF16, tag="vm")
        nc.sync.dma_start(out=vm_t[:], in_=vm[:])
        msk_t = const.tile([128, 128], F32)
        nc.sync.dma_start(out=msk_t[:], in_=msk[:])

        for tcn in range(8):
            ns = min((tcn + 1) * 128, T)
            tch = min(128, T - tcn * 128)
            sc = big.tile([128, 1024], F32, tag="sc")
            for s0 in range(0, ns, 512):
                nn_ = min(512, ns - s0)
                ps = psum.tile([128, 512], F32, tag="psc")
                for kc in range(3):
                    nc.tensor.matmul(
                        ps[:tch, :nn_],
                        qt_t[:, kc, tcn * 128:tcn * 128 + tch],
                        kt_t[:, kc, s0:s0 + nn_],
                        start=(kc == 0), stop=(kc == 2))
                nc.vector.tensor_copy(out=sc[:tch, s0:s0 + nn_],
                                      in_=ps[:tch, :nn_])
            dw = ns - tcn * 128
            nc.vector.tensor_tensor(out=sc[:tch, tcn * 128:ns],
                                    in0=sc[:tch, tcn * 128:ns],
                                    in1=msk_t[:tch, :dw], op=OP.add)
            mx = work.tile([128, 1], F32, tag="mx")
            nc.vector.tensor_reduce(out=mx[:tch], in_=sc[:tch, :ns], axis=AX.X,
                                    op=OP.max)
            nc.vector.tensor_scalar_mul(out=mx[:tch], in0=mx[:tch],
                                        scalar1=-1.0)
            sme = work.tile([128, 1], F32, tag="sme")
            nc.scalar.activation(out=sc[:tch, :ns], in_=sc[:tch, :ns],
                                 func=AF.Exp, bias=mx[:tch],
                                 accum_out=sme[:tch])
            nc.vector.reciprocal(out=sme[:tch], in_=sme[:tch])
            av = psumB.tile([128, 3, 512], F32, tag="pav")
            for sb0 in range(0, tcn + 1, 4):
                sbn = min(4, tcn + 1 - sb0)
                pT = psum.tile([128, 4, 128], F32, tag="ptr")
                for j in range(sbn):
                    sb = sb0 + j
                    scb = min(128, ns - sb * 128)
                    nc.tensor.transpose(pT[:scb, j, :tch],
                                        sc[:tch, sb * 128:sb * 128 + scb],
                                        ident[:tch, :tch])
                aT = work.tile([128, 4, 128], BF16, tag="aT")
                nc.scalar.copy(out=aT[:, 0:sbn, :].rearrange("p a b -> p (a b)"),
                               in_=pT[:, 0:sbn, :].rearrange("p a b -> p (a b)"))
                for j in range(sbn):
                    sb = sb0 + j
                    scb = min(128, ns - sb * 128)
                    for n3 in range(3):
                        nn_ = min(512, DFv - n3 * 512)
                        nc.tensor.matmul(
                            av[:tch, n3, :nn_], aT[:scb, j, :tch],
                            vm_t[:scb, sb, n3 * 512:n3 * 512 + nn_],
                            start=(sb == 0), stop=(sb == tcn))
            avs = big.tile([128, DFv], BF16, tag="avs")
            av2 = bass.AP(tensor=av.tensor, offset=av.offset,
                          ap=[av.ap[0], [1, DFv]])
            nc.vector.tensor_scalar_mul(out=avs[:tch], in0=av2[:tch],
                                        scalar1=sme[:tch])
            nc.sync.dma_start(out=avo[:, tcn, :], in_=avs[:])
    nc.compile()
    return nc


# ---------------- Launch 3c: proj + out-LN + residual ----------------

def build_l3c():
    nc = new_nc()
    avf = nc.dram_tensor("avf", [64, TSH, Qp], BF16, kind="ExternalInput")
    icm = nc.dram_tensor("icm", [64, TSH, Qp], F32, kind="ExternalInput")
    pw = nc.dram_tensor("pw", [64, 64], BF16, kind="ExternalInput")
    pb = nc.dram_tensor("pb", [64, 3], F32, kind="ExternalInput")
    outo = nc.dram_tensor("outo", [64, TSH, Q], F32, kind="ExternalOutput")
    NTF = TSH * Qp
    ctx = contextlib.ExitStack()
    with tile.TileContext(nc) as tc, ctx:
        const = ctx.enter_context(tc.tile_pool(name="const", bufs=1))
        big = ctx.enter_context(tc.tile_pool(name="big", bufs=1))
        work = ctx.enter_context(tc.tile_pool(name="work", bufs=1))
        psum = ctx.enter_context(tc.tile_pool(name="psum", bufs=2, space="PSUM"))
        eps_t = const.tile([128, 1], F32)
        nc.vector.memset(eps_t[:], EPS)
        ones_t = const.tile([64, 128], BF16)
        nc.vector.memset(ones_t[:], 1.0)
        avt = big.tile([64, NTF], BF16, tag="avt")
        nc.sync.dma_start(out=avt[:], in_=avf.rearrange("c t f -> c (t f)"))
        pwt = const.tile([64, 64], BF16)
        nc.sync.dma_start(out=pwt[:], in_=pw[:])
        pbt = const.tile([64, 3], F32)
        nc.sync.dma_start(out=pbt[:], in_=pb[:])

        P = big.tile([64, NTF], F32, tag="P")
        for n0 in range(0, NTF, 512):
            nn_ = min(512, NTF - n0)
            ps = psum.tile([64, 512], F32, tag="pp")
            nc.tensor.matmul(ps[:, :nn_], pwt[:], avt[:, n0:n0 + nn_],
                             start=True, stop=True)
            nc.scalar.activation(out=P[:, n0:n0 + nn_], in_=ps[:, :nn_],
                                 func=AF.Prelu, bias=pbt[:, 0:1],
                                 alpha=pbt[:, 1:2])
        P3 = P[:].rearrange("p (t f) -> p t f", f=Qp)
        nc.vector.memset(P3[:, :, Q:Qp], 0.0)
        s1 = work.tile([64, TSH], F32, tag="s1")
        nc.vector.tensor_reduce(out=s1[:], in_=P3, axis=AX.X, op=OP.add)
        sq = big.tile([64, NTF], BF16, tag="avt")
        nc.scalar.activation(out=sq[:], in_=P[:], func=AF.Square)
        s2 = work.tile([64, TSH], F32, tag="s2")
        nc.vector.tensor_reduce(out=s2[:], in_=sq[:].rearrange(
            "p (t f) -> p t f", f=Qp), axis=AX.X, op=OP.add)
        s1b = work.tile([64, TSH], BF16, tag="s1b")
        nc.vector.tensor_copy(out=s1b[:], in_=s1[:])
        s2b = work.tile([64, TSH], BF16, tag="s2b")
        nc.vector.tensor_copy(out=s2b[:], in_=s2[:])
        NCF = 64 * Q  # 4160
        mu = work.tile([128, TSH], F32, tag="mu")
        psg = psum.tile([128, TSH], F32, tag="pg")
        nc.tensor.matmul(psg[:], ones_t[:], s1b[:], start=True, stop=True)
        nc.vector.tensor_scalar_mul(out=mu[:], in0=psg[:], scalar1=1.0 / NCF)
        var = work.tile([128, TSH], F32, tag="var")
        psg2 = psum.tile([128, TSH], F32, tag="pg2")
        nc.tensor.matmul(psg2[:], ones_t[:], s2b[:], start=True, stop=True)
        nc.vector.tensor_scalar_mul(out=var[:], in0=psg2[:], scalar1=1.0 / NCF)
        mu2 = work.tile([128, TSH], F32, tag="mu2")
        nc.vector.tensor_tensor(out=mu2[:], in0=mu[:], in1=mu[:], op=OP.mult)
        nc.vector.tensor_tensor(out=var[:], in0=var[:], in1=mu2[:],
                                op=OP.subtract)
        rs = work.tile([128, TSH], F32, tag="rs")
        nc.scalar.activation(out=rs[:], in_=var[:], func=AF.Sqrt, bias=eps_t[:])
        nc.vector.reciprocal(out=rs[:], in_=rs[:])
        # out = (P - mu)*rs + inter
        o1 = big.tile([64, TSH, Qp], F32, tag="o1")
        nc.vector.tensor_tensor(out=o1[:], in0=P3, in1=bap(mu[0:64, :], [Qp]),
                                op=OP.subtract)
        nc.vector.tensor_tensor(out=o1[:], in0=o1[:], in1=bap(rs[0:64, :], [Qp]),
                                op=OP.mult)
        ict = big.tile([64, NTF], F32, tag="P")
        nc.sync.dma_start(out=ict[:], in_=icm.rearrange("c t f -> c (t f)"))
        nc.vector.tensor_tensor(out=o1[:], in0=o1[:],
                                in1=ict[:].rearrange("p (t f) -> p t f", f=Qp),
                                op=OP.add)
        nc.sync.dma_start(out=outo[:], in_=o1[:, :, :Q])
    nc.compile()
    return nc


# ======================= host weight prep =======================

def _uniform(a):
    a = np.asarray(a)
    assert np.all(a == a.flat[0]), "nonuniform LN affine not supported"
    return float(a.flat[0])


def _prep_lstm_v2(wih, whh, bih, bhh, gamma, beta):
    """LN-folded, gate-reordered (i,f,o,g) weight arrays."""
    g = np.asarray(gamma, np.float64).reshape(-1)
    b = np.asarray(beta, np.float64).reshape(-1)
    NH4 = np.asarray(wih).shape[0]
    w4 = np.asarray(wih, np.float64).reshape(NH4, C, KS)
    wih_eff = w4 * g[None, :, None]
    bih_eff = (np.asarray(bih, np.float64) + np.asarray(bhh, np.float64)
               + (w4 * b[None, :, None]).sum((1, 2)))
    H = NH4 // 4
    perm = np.r_[0:H, H:2 * H, 3 * H:4 * H, 2 * H:3 * H]
    return wih_eff[perm], bih_eff[perm], np.asarray(whh, np.float64)[perm]


def _build_weight_arrays(ii):
    """All per-core weight arrays as float32 numpy (pre-cast layouts)."""
    w = {}
    # ---- L1 (intra BiLSTM) ----
    wts, whs = [], []
    for d in range(2):
        we, be, wp = _prep_lstm_v2(
            ii["intra_wih"][d], ii["intra_whh"][d], ii["intra_bih"][d],
            ii["intra_bhh"][d], ii["intra_gamma"], ii["intra_beta"])
        wt = np.zeros((65, 4, 4, 128), np.float32)
        wh = np.zeros((128, 4, 128), np.float32)
        for m in range(4):
            for k in range(4):
                wt[:64, m, k] = we[m * 128:(m + 1) * 128, :, k].T
            wt[64, m, 0] = be[m * 128:(m + 1) * 128]
            wh[:, m] = wp[m * 128:(m + 1) * 128].T
        wts.append(wt); whs.append(wh)
    ctw_i = np.asarray(ii["intra_ct_w"], np.float64)
    ct_d = np.zeros((2, 2, 128, 128), np.float32)
    for d in range(2):
        sub = ctw_i[d * 128:(d + 1) * 128]
        for mo in range(2):
            for kp in range(2):
                for cc in range(64):
                    ct_d[d, mo, :, kp * 64 + cc] = sub[:, cc, mo * 2 + kp]
    ctb1 = np.zeros((128, 2), np.float32)
    for mo in range(2):
        for kp in range(2):
            ctb1[kp * 64:(kp + 1) * 64, mo] = np.asarray(ii["intra_ct_b"])
    w["l1"] = [
        ("wih", np.stack(wts, axis=1), BF16),
        ("whh", np.stack(whs, axis=1), BF16),
        ("ctw", ct_d.reshape(2, 2, 1, 128, 128).transpose(3, 0, 1, 2, 4), BF16),
        ("ctb", ctb1, F32),
    ]
    # ---- L2 (inter LSTM) ----
    we2, be2, wp2 = _prep_lstm_v2(
        ii["inter_wih"], ii["inter_whh"], ii["inter_bih"], ii["inter_bhh"],
        ii["inter_gamma"], ii["inter_beta"])
    wih2 = np.zeros((128, 8, 2, 128), np.float32)
    whh2 = np.zeros((128, 16, 128), np.float32)
    bih2 = np.zeros((128, 8), np.float32)
    for m in range(8):
        rows = we2[m * 128:(m + 1) * 128]
        for kp in range(2):
            wih2[:64, m, kp] = rows[:, :, 2 * kp].T
            wih2[64:, m, kp] = rows[:, :, 2 * kp + 1].T
        for kc in range(2):
            whh2[:, m * 2 + kc] = wp2[m * 128:(m + 1) * 128,
                                      kc * 128:(kc + 1) * 128].T
        bih2[:, m] = be2[m * 128:(m + 1) * 128]
    ctw2 = np.asarray(ii["inter_ct_w"], np.float64)
    ct2 = np.zeros((2, 256, 128), np.float32)
    for mo in range(2):
        for kp in range(2):
            for cc in range(64):
                j = kp * 64 + cc
                ct2[mo, :, j] = ctw2[:, cc, mo * 2 + kp]
    ctb2 = np.zeros((128, 2), np.float32)
    for mo in range(2):
        for kp in range(2):
            ctb2[kp * 64:(kp + 1) * 64, mo] = np.asarray(ii["inter_ct_b"])
    w["l2"] = [
        ("wih", wih2, BF16),
        ("whh", whh2, BF16),
        ("bih", bih2, F32),
        ("ctw", ct2.reshape(2, 2, 128, 128).transpose(2, 0, 1, 3)
         .reshape(128, 1, 2, 2, 128), BF16),
        ("ctb", ctb2, F32),
    ]
    # ---- L3a ----
    qg = _uniform(ii["q_g"]); kg = _uniform(ii["k_g"]); vg = _uniform(ii["v_g"])
    assert _uniform(ii["q_bt"]) == 0 and _uniform(ii["k_bt"]) == 0
    assert _uniform(ii["v_bt"]) == 0
    wall = np.zeros((64, 96), np.float32)
    bias96 = np.zeros((96,), np.float32)
    alpha96 = np.zeros((96,), np.float32)
    cnt96 = np.zeros((96,), np.float32)
    gs96 = np.zeros((96,), np.float32)
    grp = np.zeros((96,), np.int32)
    for h in range(NH):
        wall[:, h * 4:h * 4 + 4] = np.asarray(ii["q_w"][h]).T
        wall[:, 16 + h * 4:16 + h * 4 + 4] = np.asarray(ii["k_w"][h]).T
        wall[:, 32 + h * 16:32 + h * 16 + 16] = np.asarray(ii["v_w"][h]).T
        bias96[h * 4:h * 4 + 4] = np.asarray(ii["q_b"][h])
        bias96[16 + h * 4:16 + h * 4 + 4] = np.asarray(ii["k_b"][h])
        alpha96[h * 4:h * 4 + 4] = float(ii["q_p"][h])
        alpha96[16 + h * 4:16 + h * 4 + 4] = float(ii["k_p"][h])
        alpha96[32 + h * 16:32 + h * 16 + 16] = float(ii["v_p"][h])
        cnt96[h * 4:h * 4 + 4] = 1.0 / (E * Q)
        cnt96[16 + h * 4:16 + h * 4 + 4] = 1.0 / (E * Q)
        cnt96[32 + h * 16:32 + h * 16 + 16] = 1.0 / (Dv * Q)
        gs96[h * 4:h * 4 + 4] = qg / np.sqrt(E * Q)
        gs96[16 + h * 4:16 + h * 4 + 4] = kg
        gs96[32 + h * 16:32 + h * 16 + 16] = vg
        grp[h * 4:h * 4 + 4] = h
        grp[16 + h * 4:16 + h * 4 + 4] = 4 + h
        grp[32 + h * 16:32 + h * 16 + 16] = 8 + h
    gmat = (grp[:, None] == grp[None, :]).astype(np.float32)
    bs96 = np.stack([bias96, alpha96, cnt96, gs96], axis=1)
    w["l3a"] = [("wall", wall, BF16), ("bs", bs96, F32), ("gmat", gmat, BF16)]
    # ---- L3b mask ----
    mask = np.triu(np.full((128, 128), -1e9, np.float32), 1)
    w["msk"] = mask
    # ---- L3c ----
    assert _uniform(ii["proj_g"]) == 1.0 and _uniform(ii["proj_bt"]) == 0.0
    pw = np.ascontiguousarray(np.asarray(ii["proj_w"], np.float32).T)
    pb3 = np.zeros((64, 3), np.float32)
    pb3[:, 0] = np.asarray(ii["proj_b"])
    pb3[:, 1] = float(ii["proj_p"])
    w["l3c"] = [("pw", pw, BF16), ("pb", pb3, F32)]
    return w


# ======================= glue (device jnp) =======================

BF = jnp.bfloat16


def _posmajor_j(v):
    """[8, NPOS, 64] (NPOS=17000) -> [1024, 133, 64] pos-major tiles."""
    v = jnp.concatenate([v, jnp.zeros((8, G1 * 128 - NP1, 64), v.dtype)], 1)
    return v.reshape(8, G1, 128, 64).transpose(0, 2, 1, 3).reshape(
        8 * 128, G1, 64)


def glue0_fn(xg):
    """xg [8, 64, 250, 68] f16 -> (x_pm bf16 [1024,133,64],
    x_u f32 [1024,2,4250])."""
    v32 = xg.astype(jnp.float32)
    pm = _posmajor_j(v32.transpose(0, 2, 3, 1).reshape(8, NP1, 64)
                     .astype(BF))
    xu = v32.reshape(8, 64, 250, 17, 2, 2).transpose(0, 4, 5, 1, 3, 2)
    xu = xu.reshape(8, 2, 128, NT1).transpose(0, 2, 1, 3).reshape(
        1024, 2, NT1)
    return pm, xu


GROUPS = [[0, 1, 2, 3], [4, 5, 6, 7]]


def _a2a(x):
    """all-to-all within the 4-core b-group along dim0 (size 4)."""
    return jax.lax.all_to_all(x, "core", 0, 0, tiled=True,
                              axis_index_groups=GROUPS)


def _posmajor_l(rows_pc):
    """local [NPOS=17000, 64] -> [128, 133, 64] pos-major tiles."""
    v = jnp.concatenate(
        [rows_pc, jnp.zeros((G1 * 128 - NP1, 64), rows_pc.dtype)], 0)
    return v.reshape(G1, 128, 64).transpose(1, 0, 2)


def glue1_local(o1l):
    """per-core l1 outu [128,2,4250] f32 -> l2 (x_pm, x_u) local blocks.

    Core (b,tc) holds intra t-chunk; l2 wants q-chunks: all-to-all
    within the 4-core b-group."""
    ou = o1l.reshape(2, 64, 2, 17, 250)          # [kp, c, mo, l, t]
    intra = ou.transpose(1, 4, 3, 2, 0).reshape(64, 250, 68)
    s4 = intra.reshape(64, 250, 4, 17).transpose(2, 0, 1, 3)
    rcv = _a2a(s4)                               # [tc', 64, 250, 17]
    rows = rcv.transpose(1, 3, 0, 2).reshape(64, 17, 1000)  # [c, r, t]
    pm2 = _posmajor_l(rows.transpose(1, 2, 0).reshape(NP1, 64).astype(BF))
    xu2 = rows.reshape(64, 17, 250, 2, 2).transpose(3, 4, 0, 2, 1)
    xu2 = xu2.reshape(2, 128, NT2).transpose(1, 0, 2)
    return pm2, xu2


def glue2_local(o2l):
    """per-core l2 outu [128,2,4250] f32 -> (icm bf16, icm f32) local."""
    ou = o2l.reshape(2, 64, 2, 250, 17)          # [kp, c, mo, l, r]
    rows = ou.transpose(1, 4, 3, 2, 0).reshape(64, 17, 1000)  # [c, r, t]
    s4 = rows.reshape(64, 17, 4, 250).transpose(2, 0, 1, 3)
    rcv = _a2a(s4)                               # [qc', 64, 17, 250]
    tloc = rcv.transpose(1, 3, 0, 2).reshape(64, 250, 68)     # [c, t', q]
    tloc = jnp.concatenate(
        [tloc[:, :, :Q], jnp.zeros((64, 250, Qp - Q), tloc.dtype)], 2)
    return tloc.astype(BF), tloc


def glue3_local(qkvl):
    """per-core l3a qkvo [96,250,68] bf16 -> (qT, kT, vm) local."""
    qs = qkvl[0:16].reshape(4, 4, 250, 68)
    ks_ = qkvl[16:32].reshape(4, 4, 250, 68)
    vs = qkvl[32:96].reshape(4, 16, 250, 68)
    s4 = jnp.concatenate([qs, ks_, vs], axis=1)  # [h, 24, 250, 68]
    rcv = _a2a(s4)                               # [tc', 24, 250, 68]
    qkvh = rcv.transpose(1, 0, 2, 3).reshape(24, 1000, 68)

    def fm(a):                                   # [4, 1000, 68] -> ef-major
        t = a.transpose(0, 2, 1).reshape(272, 1000)
        t = jnp.concatenate([t, jnp.zeros((112, 1000), a.dtype)], 0)
        return t.reshape(3, 128, 1000).transpose(1, 0, 2)

    qT = fm(qkvh[0:4])
    kT = fm(qkvh[4:8])
    vmm = qkvh[8:24].transpose(1, 0, 2).reshape(1000, Dv * Qp)
    vmm = jnp.concatenate(
        [vmm, jnp.zeros((24, Dv * Qp), vmm.dtype)], 0)
    vmm = vmm.reshape(8, 128, Dv * Qp).transpose(1, 0, 2)
    return qT, kT, vmm


def glue4_local(avol):
    """per-core l3b avo [128,8,1088] bf16 -> avf [64,250,68] local."""
    a = avol.transpose(1, 0, 2).reshape(1024, Dv, Qp)[:1000]
    s4 = a.reshape(4, 250, Dv, Qp)               # [tc, t', d, f]
    rcv = _a2a(s4)                               # [h', 250, 16, 68]
    return rcv.transpose(0, 2, 1, 3).reshape(64, 250, 68)


def glue5_fn(outo):
    return outo.astype(jnp.float16)


def zeros_fn():
    return (jnp.zeros((1024, 2, NT1), jnp.float32),
            jnp.zeros((1024, 2, NT2), jnp.float32),
            jnp.zeros((768, 250, 68), BF),
            jnp.zeros((1024, 8, Dv * Qp), BF),
            jnp.zeros((512, 250, Q), jnp.float32))


# ======================= stage runner =======================

class _StageRunner:
    """jit(shard_map(bass_exec)) built once per stage, reused every call."""

    def __init__(self, nc, mesh, n_cores=NCORES):
        install_neuronx_cc_hook()
        self.nc = nc
        partition_name = (nc.partition_id_tensor.name
                          if nc.partition_id_tensor else None)
        dbg_name = nc.dbg_addr.name if nc.dbg_addr is not None else None
        assert not nc.dbg_callbacks
        in_names, out_names, out_avals = [], [], []
        for alloc in nc.m.functions[0].allocations:
            if not isinstance(alloc, mybir.MemoryLocationSet):
                continue
            name = alloc.memorylocations[0].name
            if alloc.kind == "ExternalInput":
                if name != partition_name:
                    in_names.append(name)
            elif alloc.kind == "ExternalOutput":
                out_names.append(name)
                out_avals.append(jax.core.ShapedArray(
                    tuple(alloc.tensor_shape), mybir.dt.np(alloc.dtype)))
        assert dbg_name is None or dbg_name in in_names
        self.in_names, self.out_names = in_names, out_names
        n_params, n_outs = len(in_names), len(out_names)
        all_names = list(in_names) + list(out_names)
        if partition_name is not None:
            all_names.append(partition_name)

        def _body(*args):
            operands = list(args)
            if partition_name is not None:
                operands.append(partition_id_tensor())
            outs = _bass_exec_p.bind(
                *operands,
                out_avals=tuple(out_avals),
                in_names=tuple(all_names),
                out_names=tuple(out_names),
                lowering_input_output_aliases=(),
                sim_require_finite=True,
                sim_require_nnan=True,
                nc=nc,
            )
            return tuple(outs)

        in_specs = (PartitionSpec("core"),) * (n_params + n_outs)
        out_specs = (PartitionSpec("core"),) * n_outs
        donate = tuple(range(n_params, n_params + n_outs))
        self.fn = jax.jit(
            shard_map(_body, mesh=mesh, in_specs=in_specs,
                      out_specs=out_specs, check_rep=False),
            donate_argnums=donate, keep_unused=True)

    def __call__(self, arrs, zeros):
        return self.fn(*arrs, *zeros)


# ======================= host orchestration =======================

_C = {}


def _wfingerprint(ii):
    keys = ["intra_wih", "intra_whh", "inter_wih", "inter_whh", "q_w",
            "k_w", "v_w", "proj_w", "intra_ct_w", "inter_ct_w"]
    return tuple(float(np.asarray(ii[k]).sum()) for k in keys)


def _ensure(ii):
    if "mesh" not in _C:
        devs = jax.devices()[:NCORES]
        _C["mesh"] = Mesh(np.asarray(devs), ("core",))
        _C["shard"] = NamedSharding(_C["mesh"], PartitionSpec("core"))
    mesh, shard = _C["mesh"], _C["shard"]
    if "l1" not in _C:
        _C["l1"] = _StageRunner(build_lstm_launch("intra"), mesh)
        _C["l2"] = _StageRunner(build_lstm_launch("inter"), mesh)
        _C["l3a"] = _StageRunner(build_l3a(), mesh)
        _C["l3b"] = _StageRunner(build_l3b(), mesh)
        _C["l3c"] = _StageRunner(build_l3c(), mesh)
        jt = lambda f: jax.jit(f, out_shardings=shard)
        P = PartitionSpec("core")

        def sm(f, nin, nout):
            return jax.jit(shard_map(
                f, mesh=mesh, in_specs=(P,) * nin,
                out_specs=(P,) * nout if nout > 1 else P, check_rep=False))

        _C["g0"] = jt(glue0_fn)
        _C["g1"] = sm(glue1_local, 1, 2)
        _C["g2"] = sm(glue2_local, 1, 2)
        _C["g3"] = sm(glue3_local, 1, 3)
        _C["g4"] = sm(glue4_local, 1, 1)
        _C["g5"] = jt(glue5_fn)
        _C["zeros"] = jax.jit(zeros_fn, out_shardings=shard)
    fp = _wfingerprint(ii)
    if _C.get("wfp") != fp:
        w = _build_weight_arrays(ii)
        put = lambda a: jax.device_put(
            np.ascontiguousarray(np.tile(np.asarray(a, np.float32),
                                         (NCORES,) + (1,) * (a.ndim - 1))),
            _C["shard"])
        devw = {}
        for stage in ["l1", "l2", "l3a", "l3c"]:
            names, arrs, dts = zip(*w[stage])
            f32d = [put(a) for a in arrs]
            castfn = jax.jit(
                lambda *xs, dts=dts: tuple(
                    x.astype(jnp.bfloat16) if dt == BF16 else x
                    for x, dt in zip(xs, dts)),
                out_shardings=_C["shard"])
            casted = castfn(*f32d)
            devw[stage] = dict(zip(names, casted))
        devw["msk"] = jax.device_put(
            np.tile(w["msk"], (NCORES, 1)), _C["shard"])
        _C["w"] = devw
        _C["wfp"] = fp


def _stage_inputs(runner, data, weights):
    m = dict(data)
    m.update(weights)
    return [m[n] for n in runner.in_names]


def kernel(**inputs):
    ii = {k: np.asarray(v) for k, v in inputs.items()}
    _ensure(ii)
    w = _C["w"]

    x = ii["x"].astype(np.float32)
    xg = np.zeros((2, 4, 64, 250, 68), np.float16)
    xg[..., :Q] = x.reshape(2, 64, 4, 250, Q).transpose(0, 2, 1, 3, 4)
    xd = jax.device_put(xg.reshape(8, 64, 250, 68), _C["shard"])

    z1, z2, z3a, z3b, z3c = _C["zeros"]()
    pm1, xu1 = _C["g0"](xd)
    (o1,) = _C["l1"](_stage_inputs(_C["l1"], {"x_pm": pm1, "x_u": xu1},
                                   w["l1"]), [z1])
    pm2, xu2 = _C["g1"](o1)
    (o2,) = _C["l2"](_stage_inputs(_C["l2"], {"x_pm": pm2, "x_u": xu2},
                                   w["l2"]), [z2])
    icm_bf, icm_f = _C["g2"](o2)
    (qkv,) = _C["l3a"](_stage_inputs(_C["l3a"], {"icm": icm_bf}, w["l3a"]),
                       [z3a])
    qT, kT, vm = _C["g3"](qkv)
    (avo,) = _C["l3b"](_stage_inputs(
        _C["l3b"], {"qT": qT, "kT": kT, "vm": vm, "msk": w["msk"]}, {}),
        [z3b])
    avf = _C["g4"](avo)
    (outo,) = _C["l3c"](_stage_inputs(
        _C["l3c"], {"avf": avf, "icm": icm_f}, w["l3c"]), [z3c])
    of = _C["g5"](outo)
    oh = np.asarray(of)     # [512, 250, 65] f16

    out = oh.reshape(2, 4, 64, 250, Q).transpose(0, 2, 1, 3, 4)
    return np.ascontiguousarray(out.reshape(2, 64, 1000, Q)
                                .astype(np.float32))


# revision 8
# speedup vs baseline: 15.9906x; 1.0302x over previous
"""GridNetBlock (TF-GridNet) Trainium2 kernel: 8-core SPMD.

v2: fully device-resident pipeline. The five Bass stage kernels from v1
are unchanged, but all inter-stage glue (unfold/reshard/transpose) now
runs on-device as jitted jnp ops, weights are uploaded once and cached,
and host<->device traffic per call is only x (f16 in) + output (f16
out). The axon tunnel moves ~33MB/s, so this is what dominates wall.
"""
import sys, os, contextlib
for _p in ("/opt/trn_rl_repo", "/root/.axon_site/_ro/trn_rl_repo"):
    if os.path.isdir(_p) and _p not in sys.path:
        sys.path.insert(0, _p)
import numpy as np
import jax
import jax.numpy as jnp
from jax.sharding import Mesh, PartitionSpec, NamedSharding
from jax.experimental.shard_map import shard_map
import concourse.bass as bass
import concourse.bacc as bacc
import concourse.tile as tile
from concourse import mybir
from concourse.masks import make_identity
from concourse.bass2jax import (_bass_exec_p, install_neuronx_cc_hook,
                                partition_id_tensor)

F32 = mybir.dt.float32
BF16 = mybir.dt.bfloat16
AF = mybir.ActivationFunctionType
OP = mybir.AluOpType
AX = mybir.AxisListType

B, C, T, Q = 2, 64, 1000, 65
KS = 4
Qp, L1, Hh, HID, L2 = 68, 17, 128, 256, 250
NH, E, Dv = 4, 4, 16
EPS = 1e-5
NCORES = 8
TSH = T // 4
NP1 = TSH * Qp
G1 = (NP1 + 127) // 128   # 133
RW2 = (B * Qp) // NCORES  # 17
NT1 = L1 * TSH            # 4250
NT2 = L2 * RW2            # 4250


def bap(t, tail):
    ap = list(t.ap)
    for n in tail:
        ap.append([0, n])
    return bass.AP(tensor=t.tensor, offset=t.offset, ap=ap)


def new_nc():
    return bacc.Bacc("TRN2", target_bir_lowering=False, debug=False,
                     enable_asserts=True, num_devices=NCORES)


def ln_posmajor(nc, pool, work, xpm, G, nred, eps_t):
    s1 = work.tile([128, G], F32, tag="lns1")
    nc.vector.tensor_reduce(out=s1[:], in_=xpm[:], axis=AX.X, op=OP.add)
    xsq = pool.tile([128, G, nred], BF16, tag="xut")
    nc.scalar.activation(out=xsq[:], in_=xpm[:], func=AF.Square)
    s2 = work.tile([128, G], F32, tag="lns2")
    nc.vector.tensor_reduce(out=s2[:], in_=xsq[:], axis=AX.X, op=OP.add)
    mu = work.tile([128, G], F32, tag="lnmu")
    nc.vector.tensor_scalar_mul(out=mu[:], in0=s1[:], scalar1=1.0 / nred)
    var = work.tile([128, G], F32, tag="lnvar")
    nc.vector.tensor_tensor(out=var[:], in0=mu[:], in1=mu[:], op=OP.mult)
    nc.vector.scalar_tensor_tensor(out=var[:], in0=s2[:], scalar=1.0 / nred,
                                   in1=var[:], op0=OP.mult, op1=OP.subtract)
    rs = work.tile([128, G], F32, tag="lnrs")
    nc.scalar.activation(out=rs[:], in_=var[:], func=AF.Sqrt, bias=eps_t[:])
    nc.vector.reciprocal(out=rs[:], in_=rs[:])
    zpm = pool.tile([128, G, nred], BF16, tag="xut")
    nc.vector.tensor_tensor(out=zpm[:], in0=xpm[:], in1=bap(mu, [nred]),
                            op=OP.subtract)
    nc.vector.tensor_tensor(out=zpm[:], in0=zpm[:], in1=bap(rs, [nred]),
                            op=OP.mult)
    return zpm


def ap3(t, off, d1, n1, d2, n2):
    """Strided 2-free-dim AP view of tile t at element offset off."""
    return bass.AP(tensor=t.tensor, offset=t.offset + off,
                   ap=[t.ap[0], [d1, n1], [d2, n2]])


def build_lstm_launch(which):
    """which: 'intra' or 'inter'. Returns compiled nc."""
    intra = which == "intra"
    ND = 2 if intra else 1
    MC = 4 if intra else 8
    KC = 1 if intra else 2
    L = L1 if intra else L2
    NB = TSH if intra else RW2        # lstm batch per core
    NT = L * NB                       # 4250
    G = G1
    ZC = G * 128

    nc = new_nc()
    x_pm = nc.dram_tensor("x_pm", [128, G, C], BF16, kind="ExternalInput")
    x_u = nc.dram_tensor("x_u", [128, 2, NT], F32, kind="ExternalInput")
    if intra:
        wih = nc.dram_tensor("wih", [65, 2, 4, 4, 128], BF16,
                             kind="ExternalInput")
        whh = nc.dram_tensor("whh", [128, 2, 4, 128], BF16,
                             kind="ExternalInput")
    else:
        wih = nc.dram_tensor("wih", [128, 8, 2, 128], BF16,
                             kind="ExternalInput")
        whh = nc.dram_tensor("whh", [128, 16, 128], BF16,
                             kind="ExternalInput")
        bih = nc.dram_tensor("bih", [128, 8], F32, kind="ExternalInput")
    ctw = nc.dram_tensor("ctw", [128, ND, 2, KC, 128], BF16,
                         kind="ExternalInput")
    ctb = nc.dram_tensor("ctb", [128, 2], F32, kind="ExternalInput")
    outu = nc.dram_tensor("outu", [128, 2, NT], F32, kind="ExternalOutput")

    ctx = contextlib.ExitStack()
    with tile.TileContext(nc) as tc, ctx:
        const = ctx.enter_context(tc.tile_pool(name="const", bufs=1))
        persist = ctx.enter_context(tc.tile_pool(name="persist", bufs=1))
        psum = ctx.enter_context(tc.tile_pool(name="psum", bufs=2,
                                              space="PSUM"))
        psumB = ctx.enter_context(tc.tile_pool(name="psumB", bufs=2,
                                               space="PSUM"))
        psumS = ctx.enter_context(tc.tile_pool(
            name="psumS", bufs=1 if intra else 2, space="PSUM"))

        eps_t = const.tile([128, 1], F32)
        nc.vector.memset(eps_t[:], EPS)
        ident = const.tile([128, 128], BF16)
        make_identity(nc, ident[:])

        if intra:
            wih_t = const.tile([65, 2, 4, 4, 128], BF16)
            whh_t = const.tile([128, 2, 4, 128], BF16)
        else:
            wih_t = const.tile([128, 8, 2, 128], BF16)
            whh_t = const.tile([128, 16, 128], BF16)
            bih_t = const.tile([128, 8], F32)
            nc.sync.dma_start(out=bih_t[:], in_=bih[:])
        nc.sync.dma_start(out=wih_t[:], in_=wih[:])
        nc.sync.dma_start(out=whh_t[:], in_=whh[:])
        ct_tl = const.tile([128, ND, 2, KC, 128], BF16)
        nc.sync.dma_start(out=ct_tl[:], in_=ctw[:])
        ctb_t = const.tile([128, 2], F32)
        nc.sync.dma_start(out=ctb_t[:], in_=ctb[:])

        # --- persistent tiles ---
        if intra:
            # z channel-major [65, ZC]; row 64 = ones (bias row)
            z_cm = persist.tile([65, ZC], BF16)
            hbufs = [persist.tile([128, L, NB], BF16, name=f"hb{d}")
                     for d in range(ND * KC)]
        else:
            # z doubled rows: p<64: z[c, t'-3]; p>=64: z[c, t'-2]
            z2d = persist.tile([128, RW2, 1000], BF16)
            hb2 = persist.tile([128, 2, L, NB], BF16)
        ou = persist.tile([128, 2, L, NB], F32)

        # --- LN over C (pos-major) + transpose to channel-major ---
        with tc.tile_pool(name="tmpA", bufs=1) as tmpA:
            xpm = tmpA.tile([128, G, C], BF16)
            nc.sync.dma_start(out=xpm[:], in_=x_pm[:])
            work = tmpA
            zpm = ln_posmajor(nc, tmpA, work, xpm, G, C, eps_t)
            if intra:
                z_dst = z_cm
                nc.gpsimd.memset(z_cm[64:65, :], 1.0)
            else:
                z_tmp = tmpA.tile([64, ZC], BF16)
                z_dst = z_tmp
            for g0 in range(0, G, 4):
                gn = min(4, G - g0)
                pt = psum.tile([64, 4, 128], BF16, tag="tps")
                for gg in range(gn):
                    nc.tensor.transpose(pt[:, gg, :], zpm[:, g0 + gg, :],
                                        ident[:])
                if (g0 // 4) % 2 == 0:
                    nc.scalar.copy(
                        out=z_dst[0:64, g0 * 128:(g0 + gn) * 128],
                        in_=pt[:, 0:gn, :].rearrange("p a b -> p (a b)"))
                else:
                    nc.vector.tensor_copy(
                        out=z_dst[0:64, g0 * 128:(g0 + gn) * 128],
                        in_=pt[:, 0:gn, :].rearrange("p a b -> p (a b)"))
            if not intra:
                # z_tmp [64, row*1000+t] -> z2d two shifted copies
                nc.vector.memset(z2d[0:64, :, 0:3], 0.0)
                nc.vector.memset(z2d[64:128, :, 0:2], 0.0)
                nc.sync.dma_start(
                    out=z2d[0:64, :, 3:1000],
                    in_=ap3(z_tmp, 0, 1000, RW2, 1, 997))
                nc.sync.dma_start(
                    out=z2d[64:128, :, 2:1000],
                    in_=ap3(z_tmp, 0, 1000, RW2, 1, 998))

        # --- gate precompute (inter only): pre2 [128, L, 8, RW2] bf16 ---
        rec = ctx.enter_context(tc.tile_pool(name="rec", bufs=1))
        if not intra:
            pre2 = rec.tile([128, L, 8, RW2], BF16)
            LSUB = 30
            for l0 in range(0, L, LSUB):
                ln_ = min(LSUB, L - l0)
                for m in range(8):
                    pp = psumB.tile([128, 512], F32, tag="ppre")
                    for kp in range(2):
                        rhs = ap3(z2d, 4 * l0 + 2 * kp, 4, ln_, 1000, RW2)
                        nc.tensor.matmul(pp[:, :RW2 * ln_],
                                         wih_t[:, m, kp, :], rhs,
                                         start=(kp == 0), stop=(kp == 1))
                    dst = pre2[:, l0:l0 + ln_, m, :]
                    src = pp[:, :RW2 * ln_].rearrange(
                        "p (l r) -> p l r", r=RW2)
                    if m % 2 == 0:
                        nc.vector.tensor_scalar_add(out=dst, in0=src,
                                                    scalar1=bih_t[:, m:m + 1])
                    else:
                        nc.scalar.activation(out=dst, in_=src,
                                             func=AF.Identity,
                                             bias=bih_t[:, m:m + 1])

        # --- recurrence (interleaved directions) ---
        NGC = MC // 4                      # hidden chunks (1 or 2)
        gsb = [rec.tile([128, MC, NB], BF16, name=f"gs{d}")
               for d in range(ND)]
        c_t = [rec.tile([128, NGC, NB], F32, name=f"ct{d}")
               for d in range(ND)]
        ig_t = [rec.tile([128, NGC, NB], BF16, name=f"ig{d}")
               for d in range(ND)]
        tc_t = [rec.tile([128, NGC, NB], BF16, name=f"tc{d}")
               for d in range(ND)]
        slot = 64 if NB <= 64 else 256

        def step(d, l, first):
            lp = l + 1 if (intra and d == 1) else l - 1
            ps = psumS.tile([128, MC, slot if intra else RW2], F32,
                            tag=f"lps{d}")
            if intra:
                for m in range(4):
                    st = Qp
                    for k in range(4):
                        o0 = 4 * l + k
                        rhs = bass.AP(
                            tensor=z_cm.tensor, offset=z_cm.offset + o0,
                            ap=[z_cm.ap[0], [st, NB]])
                        nc.tensor.matmul(ps[:, m, :NB],
                                         wih_t[:, d, m, k, :], rhs,
                                         start=(k == 0),
                                         stop=(k == 3 and first))
                    if not first:
                        nc.tensor.matmul(ps[:, m, :NB], whh_t[:, d, m, :],
                                         hbufs[d][:, lp, :],
                                         start=False, stop=True)
            else:
                if not first:
                    nc.tensor.matmul(
                        ps[:].rearrange("p m n -> p (m n)"), ident[:],
                        pre2[:, l, :, :].rearrange("p m n -> p (m n)"),
                        start=True, stop=False, skip_group_check=True)
                    for m in range(8):
                        for kc in range(2):
                            nc.tensor.matmul(ps[:, m, :NB],
                                             whh_t[:, m * 2 + kc, :],
                                             hb2[:, kc, lp, :],
                                             start=False, stop=(kc == 1),
                                             skip_group_check=True)
            # gates: i (NGC), f (NGC), o (NGC), g (NGC)
            gg = gsb[d]
            if first and not intra:
                sig_in = pre2[:, l, 0:3 * NGC, :]
                tanh_in = pre2[:, l, 3 * NGC:, :]
            else:
                sig_in = ps[:, 0:3 * NGC, :NB]
                tanh_in = ps[:, 3 * NGC:, :NB]
            nc.scalar.activation(out=gg[:, 0:3 * NGC, :], in_=sig_in,
                                 func=AF.Sigmoid)
            nc.scalar.activation(out=gg[:, 3 * NGC:, :], in_=tanh_in,
                                 func=AF.Tanh)
            i_g, f_g = gg[:, 0:NGC, :], gg[:, NGC:2 * NGC, :]
            o_g, g_g = gg[:, 2 * NGC:3 * NGC, :], gg[:, 3 * NGC:, :]
            if first:
                nc.vector.tensor_tensor(out=c_t[d][:], in0=i_g, in1=g_g,
                                        op=OP.mult)
            else:
                nc.vector.tensor_tensor(out=ig_t[d][:], in0=i_g, in1=g_g,
                                        op=OP.mult)
                nc.vector.tensor_tensor(out=c_t[d][:], in0=f_g, in1=c_t[d][:],
                                        op=OP.mult)
                nc.vector.tensor_tensor(out=c_t[d][:], in0=c_t[d][:],
                                        in1=ig_t[d][:], op=OP.add)
            nc.scalar.activation(out=tc_t[d][:], in_=c_t[d][:], func=AF.Tanh)
            if intra:
                nc.vector.tensor_tensor(out=hbufs[d][:, l, :], in0=o_g,
                                        in1=tc_t[d][:], op=OP.mult)
            else:
                nc.vector.tensor_tensor(out=hb2[:, :, l, :], in0=o_g,
                                        in1=tc_t[d][:], op=OP.mult)

        # --- ConvT + bias + residual, l-chunked, interleaved with steps ---
        xu_t = rec.tile([128, 2, NT], F32)
        nc.sync.dma_start(out=xu_t[:], in_=x_u[:])
        CL = 2 if intra else 30

        def convt_chunk(l0):
            ln_ = min(CL, L - l0)
            nn_ = ln_ * NB
            for mo in range(2):
                ps2 = psumB.tile([128, 512], F32, tag="pct")
                nch = 0
                for d in range(ND):
                    for k in range(KC):
                        hsl = (hbufs[d][:, l0:l0 + ln_, :] if intra
                               else hb2[:, k, l0:l0 + ln_, :])
                        nc.tensor.matmul(
                            ps2[:, :nn_], ct_tl[:, d, mo, k, :],
                            hsl.rearrange("p l t -> p (l t)"),
                            start=(nch == 0), stop=(nch == ND * KC - 1))
                        nch += 1
                nc.vector.scalar_tensor_tensor(
                    out=ou[:, mo, l0:l0 + ln_, :].rearrange(
                        "p l t -> p (l t)"),
                    in0=ps2[:, :nn_], scalar=ctb_t[:, mo:mo + 1],
                    in1=xu_t[:, mo, l0 * NB:l0 * NB + nn_],
                    op0=OP.add, op1=OP.add)

        pending = list(range(0, L, CL))
        for i in range(L):
            step(0, i, i == 0)
            if intra:
                step(1, L - 1 - i, i == 0)
            for l0 in list(pending):
                ln_ = min(CL, L - l0)
                ready = i >= l0 + ln_ - 1
                if intra:
                    ready = ready and i >= L - 1 - l0
                if ready:
                    convt_chunk(l0)
                    pending.remove(l0)
        for l0 in pending:
            convt_chunk(l0)
        nc.sync.dma_start(out=outu[:],
                          in_=ou[:].rearrange("p a l t -> p a (l t)"))
    nc.compile()
    return nc


# ---------------- Launch 3a: QKV conv + PReLU + LN ----------------

def build_l3a():
    nc = new_nc()
    icm = nc.dram_tensor("icm", [64, TSH, Qp], BF16, kind="ExternalInput")
    wall = nc.dram_tensor("wall", [64, 96], BF16, kind="ExternalInput")
    bs = nc.dram_tensor("bs", [96, 4], F32, kind="ExternalInput")
    # bs cols: bias, alpha, cnt_inv, gscale (per row)
    gmat = nc.dram_tensor("gmat", [96, 96], BF16, kind="ExternalInput")
    qkvo = nc.dram_tensor("qkvo", [96, TSH, Qp], BF16, kind="ExternalOutput")
    NTF = TSH * Qp  # 17000
    ctx = contextlib.ExitStack()
    with tile.TileContext(nc) as tc, ctx:
        const = ctx.enter_context(tc.tile_pool(name="const", bufs=1))
        big = ctx.enter_context(tc.tile_pool(name="big", bufs=1))
        work = ctx.enter_context(tc.tile_pool(name="work", bufs=2))
        psum = ctx.enter_context(tc.tile_pool(name="psum", bufs=2, space="PSUM"))
        eps_t = const.tile([96, 1], F32)
        nc.vector.memset(eps_t[:], EPS)
        ict = big.tile([64, NTF], BF16, tag="ict")
        nc.sync.dma_start(out=ict[:], in_=icm.rearrange("c t f -> c (t f)"))
        wt = const.tile([64, 96], BF16)
        nc.sync.dma_start(out=wt[:], in_=wall[:])
        bst = const.tile([96, 4], F32)
        nc.sync.dma_start(out=bst[:], in_=bs[:])
        gm = const.tile([96, 96], BF16)
        nc.sync.dma_start(out=gm[:], in_=gmat[:])

        qr = big.tile([96, NTF], F32, tag="qr")
        for n0 in range(0, NTF, 512):
            nn_ = min(512, NTF - n0)
            ps = psum.tile([96, 512], F32, tag="pc")
            nc.tensor.matmul(ps[:, :nn_], wt[:], ict[:, n0:n0 + nn_],
                             start=True, stop=True)
            nc.scalar.activation(out=qr[:, n0:n0 + nn_], in_=ps[:, :nn_],
                                 func=AF.Prelu, bias=bst[:, 0:1],
                                 alpha=bst[:, 1:2])
        # stats over (e,f) groups: reduce f, then group-collapse via gmat
        s1 = work.tile([96, TSH], F32, tag="s1")
        nc.vector.tensor_reduce(out=s1[:], in_=qr[:].rearrange(
            "p (t f) -> p t f", f=Qp), axis=AX.X, op=OP.add)
        sq = big.tile([96, NTF], BF16, tag="sq")
        nc.scalar.activation(out=sq[:], in_=qr[:], func=AF.Square)
        s2 = work.tile([96, TSH], F32, tag="s2")
        nc.vector.tensor_reduce(out=s2[:], in_=sq[:].rearrange(
            "p (t f) -> p t f", f=Qp), axis=AX.X, op=OP.add)
        s1b = work.tile([96, TSH], BF16, tag="s1b")
        nc.vector.tensor_copy(out=s1b[:], in_=s1[:])
        s2b = work.tile([96, TSH], BF16, tag="s2b")
        nc.vector.tensor_copy(out=s2b[:], in_=s2[:])
        mu = work.tile([96, TSH], F32, tag="mu")
        ps1 = psum.tile([96, TSH], F32, tag="pg1")
        nc.tensor.matmul(ps1[:], gm[:], s1b[:], start=True, stop=True)
        nc.vector.tensor_scalar_mul(out=mu[:], in0=ps1[:], scalar1=bst[:, 2:3])
        var = work.tile([96, TSH], F32, tag="var")
        ps2g = psum.tile([96, TSH], F32, tag="pg2")
        nc.tensor.matmul(ps2g[:], gm[:], s2b[:], start=True, stop=True)
        nc.vector.tensor_scalar_mul(out=var[:], in0=ps2g[:], scalar1=bst[:, 2:3])
        mu2 = work.tile([96, TSH], F32, tag="mu2")
        nc.vector.tensor_tensor(out=mu2[:], in0=mu[:], in1=mu[:], op=OP.mult)
        nc.vector.tensor_tensor(out=var[:], in0=var[:], in1=mu2[:],
                                op=OP.subtract)
        rs = work.tile([96, TSH], F32, tag="rs")
        nc.scalar.activation(out=rs[:], in_=var[:], func=AF.Sqrt, bias=eps_t[:])
        nc.vector.reciprocal(out=rs[:], in_=rs[:])
        nc.vector.tensor_scalar_mul(out=rs[:], in0=rs[:], scalar1=bst[:, 3:4])
        zh = big.tile([96, TSH, Qp], BF16, tag="zh")
        qr3 = qr[:].rearrange("p (t f) -> p t f", f=Qp)
        nc.vector.tensor_tensor(out=zh[:], in0=qr3, in1=bap(mu, [Qp]),
                                op=OP.subtract)
        nc.vector.tensor_tensor(out=zh[:], in0=zh[:], in1=bap(rs, [Qp]),
                                op=OP.mult)
        nc.vector.memset(zh[:, :, Q:Qp], 0.0)
        nc.sync.dma_start(out=qkvo[:], in_=zh[:])
    nc.compile()
    return nc


# ---------------- Launch 3b: attention per (h,b) ----------------

def build_l3b():
    nc = new_nc()
    qT = nc.dram_tensor("qT", [128, 3, T], BF16, kind="ExternalInput")
    kT = nc.dram_tensor("kT", [128, 3, T], BF16, kind="ExternalInput")
    vm = nc.dram_tensor("vm", [128, 8, Dv * Qp], BF16,
                        kind="ExternalInput")
    msk = nc.dram_tensor("msk", [128, 128], F32, kind="ExternalInput")
    avo = nc.dram_tensor("avo", [128, 8, Dv * Qp], BF16,
                         kind="ExternalOutput")
    DFv = Dv * Qp
    ctx = contextlib.ExitStack()
    with tile.TileContext(nc) as tc, ctx:
        const = ctx.enter_context(tc.tile_pool(name="const", bufs=1))
        big = ctx.enter_context(tc.tile_pool(name="big", bufs=1))
        work = ctx.enter_context(tc.tile_pool(name="work", bufs=3))
        psum = ctx.enter_context(tc.tile_pool(name="psum", bufs=2, space="PSUM"))
        psumB = ctx.enter_context(tc.tile_pool(name="psumB", bufs=1,
                                               space="PSUM"))
        ident = const.tile([128, 128], F32)
        make_identity(nc, ident[:])
        qt_t = big.tile([128, 3, T], BF16, tag="qt")
        nc.sync.dma_start(out=qt_t[:], in_=qT[:])
        kt_t = big.tile([128, 3, T], BF16, tag="kt")
        nc.sync.dma_start(out=kt_t[:], in_=kT[:])
        vm_t = big.tile([128, 8, DFv], BF16, tag="vm")
        nc.sync.dma_start(out=vm_t[:], in_=vm[:])
        msk_t = const.tile([128, 128], F32)
        nc.sync.dma_start(out=msk_t[:], in_=msk[:])

        for tcn in range(8):
            ns = min((tcn + 1) * 128, T)
            tch = min(128, T - tcn * 128)
            sc = big.tile([128, 1024], F32, tag="sc")
            for s0 in range(0, ns, 512):
                nn_ = min(512, ns - s0)
                ps = psum.tile([128, 512], F32, tag="psc")
                for kc in range(3):
                    nc.tensor.matmul(
                        ps[:tch, :nn_],
                        qt_t[:, kc, tcn * 128:tcn * 128 + tch],
                        kt_t[:, kc, s0:s0 + nn_],
                        start=(kc == 0), stop=(kc == 2))
                nc.vector.tensor_copy(out=sc[:tch, s0:s0 + nn_],
                                      in_=ps[:tch, :nn_])
            dw = ns - tcn * 128
            nc.vector.tensor_tensor(out=sc[:tch, tcn * 128:ns],
                                    in0=sc[:tch, tcn * 128:ns],
                                    in1=msk_t[:tch, :dw], op=OP.add)
            mx = work.tile([128, 1], F32, tag="mx")
            nc.vector.tensor_reduce(out=mx[:tch], in_=sc[:tch, :ns], axis=AX.X,
                                    op=OP.max)
            nc.vector.tensor_scalar_mul(out=mx[:tch], in0=mx[:tch],
                                        scalar1=-1.0)
            sme = work.tile([128, 1], F32, tag="sme")
            nc.scalar.activation(out=sc[:tch, :ns], in_=sc[:tch, :ns],
                                 func=AF.Exp, bias=mx[:tch],
                                 accum_out=sme[:tch])
            nc.vector.reciprocal(out=sme[:tch], in_=sme[:tch])
            av = psumB.tile([128, 3, 512], F32, tag="pav")
            for sb0 in range(0, tcn + 1, 4):
                sbn = min(4, tcn + 1 - sb0)
                pT = psum.tile([128, 4, 128], F32, tag="ptr")
                for j in range(sbn):
                    sb = sb0 + j
                    scb = min(128, ns - sb * 128)
                    nc.tensor.transpose(pT[:scb, j, :tch],
                                        sc[:tch, sb * 128:sb * 128 + scb],
                                        ident[:tch, :tch])
                aT = work.tile([128, 4, 128], BF16, tag="aT")
                nc.scalar.copy(out=aT[:, 0:sbn, :].rearrange("p a b -> p (a b)"),
                               in_=pT[:, 0:sbn, :].rearrange("p a b -> p (a b)"))
                for j in range(sbn):
                    sb = sb0 + j
                    scb = min(128, ns - sb * 128)
                    for n3 in range(3):
                        nn_ = min(512, DFv - n3 * 512)
                        nc.tensor.matmul(
                            av[:tch, n3, :nn_], aT[:scb, j, :tch],
                            vm_t[:scb, sb, n3 * 512:n3 * 512 + nn_],
                            start=(sb == 0), stop=(sb == tcn))
            avs = big.tile([128, DFv], BF16, tag="avs")
            av2 = bass.AP(tensor=av.tensor, offset=av.offset,
                          ap=[av.ap[0], [1, DFv]])
            nc.vector.tensor_scalar_mul(out=avs[:tch], in0=av2[:tch],
                                        scalar1=sme[:tch])
            nc.sync.dma_start(out=avo[:, tcn, :], in_=avs[:])
    nc.compile()
    return nc


# ---------------- Launch 3c: proj + out-LN + residual ----------------

def build_l3c():
    nc = new_nc()
    avf = nc.dram_tensor("avf", [64, TSH, Qp], BF16, kind="ExternalInput")
    icm = nc.dram_tensor("icm", [64, TSH, Qp], F32, kind="ExternalInput")
    pw = nc.dram_tensor("pw", [64, 64], BF16, kind="ExternalInput")
    pb = nc.dram_tensor("pb", [64, 3], F32, kind="ExternalInput")
    outo = nc.dram_tensor("outo", [64, TSH, Q], F32, kind="ExternalOutput")
    NTF = TSH * Qp
    ctx = contextlib.ExitStack()
    with tile.TileContext(nc) as tc, ctx:
        const = ctx.enter_context(tc.tile_pool(name="const", bufs=1))
        big = ctx.enter_context(tc.tile_pool(name="big", bufs=1))
        work = ctx.enter_context(tc.tile_pool(name="work", bufs=1))
        psum = ctx.enter_context(tc.tile_pool(name="psum", bufs=2, space="PSUM"))
        eps_t = const.tile([128, 1], F32)
        nc.vector.memset(eps_t[:], EPS)
        ones_t = const.tile([64, 128], BF16)
        nc.vector.memset(ones_t[:], 1.0)
        avt = big.tile([64, NTF], BF16, tag="avt")
        nc.sync.dma_start(out=avt[:], in_=avf.rearrange("c t f -> c (t f)"))
        pwt = const.tile([64, 64], BF16)
        nc.sync.dma_start(out=pwt[:], in_=pw[:])
        pbt = const.tile([64, 3], F32)
        nc.sync.dma_start(out=pbt[:], in_=pb[:])

        P = big.tile([64, NTF], F32, tag="P")
        for n0 in range(0, NTF, 512):
            nn_ = min(512, NTF - n0)
            ps = psum.tile([64, 512], F32, tag="pp")
            nc.tensor.matmul(ps[:, :nn_], pwt[:], avt[:, n0:n0 + nn_],
                             start=True, stop=True)
            nc.scalar.activation(out=P[:, n0:n0 + nn_], in_=ps[:, :nn_],
                                 func=AF.Prelu, bias=pbt[:, 0:1],
                                 alpha=pbt[:, 1:2])
        P3 = P[:].rearrange("p (t f) -> p t f", f=Qp)
        nc.vector.memset(P3[:, :, Q:Qp], 0.0)
        s1 = work.tile([64, TSH], F32, tag="s1")
        nc.vector.tensor_reduce(out=s1[:], in_=P3, axis=AX.X, op=OP.add)
        sq = big.tile([64, NTF], BF16, tag="avt")
        nc.scalar.activation(out=sq[:], in_=P[:], func=AF.Square)
        s2 = work.tile([64, TSH], F32, tag="s2")
        nc.vector.tensor_reduce(out=s2[:], in_=sq[:].rearrange(
            "p (t f) -> p t f", f=Qp), axis=AX.X, op=OP.add)
        s1b = work.tile([64, TSH], BF16, tag="s1b")
        nc.vector.tensor_copy(out=s1b[:], in_=s1[:])
        s2b = work.tile([64, TSH], BF16, tag="s2b")
        nc.vector.tensor_copy(out=s2b[:], in_=s2[:])
        NCF = 64 * Q  # 4160
        mu = work.tile([128, TSH], F32, tag="mu")
        psg = psum.tile([128, TSH], F32, tag="pg")
        nc.tensor.matmul(psg[:], ones_t[:], s1b[:], start=True, stop=True)
        nc.vector.tensor_scalar_mul(out=mu[:], in0=psg[:], scalar1=1.0 / NCF)
        var = work.tile([128, TSH], F32, tag="var")
        psg2 = psum.tile([128, TSH], F32, tag="pg2")
        nc.tensor.matmul(psg2[:], ones_t[:], s2b[:], start=True, stop=True)
        nc.vector.tensor_scalar_mul(out=var[:], in0=psg2[:], scalar1=1.0 / NCF)
        mu2 = work.tile([128, TSH], F32, tag="mu2")
        nc.vector.tensor_tensor(out=mu2[:], in0=mu[:], in1=mu[:], op=OP.mult)
        nc.vector.tensor_tensor(out=var[:], in0=var[:], in1=mu2[:],
                                op=OP.subtract)
        rs = work.tile([128, TSH], F32, tag="rs")
        nc.scalar.activation(out=rs[:], in_=var[:], func=AF.Sqrt, bias=eps_t[:])
        nc.vector.reciprocal(out=rs[:], in_=rs[:])
        # out = (P - mu)*rs + inter
        o1 = big.tile([64, TSH, Qp], F32, tag="o1")
        nc.vector.tensor_tensor(out=o1[:], in0=P3, in1=bap(mu[0:64, :], [Qp]),
                                op=OP.subtract)
        nc.vector.tensor_tensor(out=o1[:], in0=o1[:], in1=bap(rs[0:64, :], [Qp]),
                                op=OP.mult)
        ict = big.tile([64, NTF], F32, tag="P")
        nc.sync.dma_start(out=ict[:], in_=icm.rearrange("c t f -> c (t f)"))
        nc.vector.tensor_tensor(out=o1[:], in0=o1[:],
                                in1=ict[:].rearrange("p (t f) -> p t f", f=Qp),
                                op=OP.add)
        nc.sync.dma_start(out=outo[:], in_=o1[:, :, :Q])
    nc.compile()
    return nc


# ======================= host weight prep =======================

def _uniform(a):
    a = np.asarray(a)
    assert np.all(a == a.flat[0]), "nonuniform LN affine not supported"
    return float(a.flat[0])


def _prep_lstm_v2(wih, whh, bih, bhh, gamma, beta):
    """LN-folded, gate-reordered (i,f,o,g) weight arrays."""
    g = np.asarray(gamma, np.float64).reshape(-1)
    b = np.asarray(beta, np.float64).reshape(-1)
    NH4 = np.asarray(wih).shape[0]
    w4 = np.asarray(wih, np.float64).reshape(NH4, C, KS)
    wih_eff = w4 * g[None, :, None]
    bih_eff = (np.asarray(bih, np.float64) + np.asarray(bhh, np.float64)
               + (w4 * b[None, :, None]).sum((1, 2)))
    H = NH4 // 4
    perm = np.r_[0:H, H:2 * H, 3 * H:4 * H, 2 * H:3 * H]
    return wih_eff[perm], bih_eff[perm], np.asarray(whh, np.float64)[perm]


def _build_weight_arrays(ii):
    """All per-core weight arrays as float32 numpy (pre-cast layouts)."""
    w = {}
    # ---- L1 (intra BiLSTM) ----
    wts, whs = [], []
    for d in range(2):
        we, be, wp = _prep_lstm_v2(
            ii["intra_wih"][d], ii["intra_whh"][d], ii["intra_bih"][d],
            ii["intra_bhh"][d], ii["intra_gamma"], ii["intra_beta"])
        wt = np.zeros((65, 4, 4, 128), np.float32)
        wh = np.zeros((128, 4, 128), np.float32)
        for m in range(4):
            for k in range(4):
                wt[:64, m, k] = we[m * 128:(m + 1) * 128, :, k].T
            wt[64, m, 0] = be[m * 128:(m + 1) * 128]
            wh[:, m] = wp[m * 128:(m + 1) * 128].T
        wts.append(wt); whs.append(wh)
    ctw_i = np.asarray(ii["intra_ct_w"], np.float64)
    ct_d = np.zeros((2, 2, 128, 128), np.float32)
    for d in range(2):
        sub = ctw_i[d * 128:(d + 1) * 128]
        for mo in range(2):
            for kp in range(2):
                for cc in range(64):
                    ct_d[d, mo, :, kp * 64 + cc] = sub[:, cc, mo * 2 + kp]
    ctb1 = np.zeros((128, 2), np.float32)
    for mo in range(2):
        for kp in range(2):
            ctb1[kp * 64:(kp + 1) * 64, mo] = np.asarray(ii["intra_ct_b"])
    w["l1"] = [
        ("wih", np.stack(wts, axis=1), BF16),
        ("whh", np.stack(whs, axis=1), BF16),
        ("ctw", ct_d.reshape(2, 2, 1, 128, 128).transpose(3, 0, 1, 2, 4), BF16),
        ("ctb", ctb1, F32),
    ]
    # ---- L2 (inter LSTM) ----
    we2, be2, wp2 = _prep_lstm_v2(
        ii["inter_wih"], ii["inter_whh"], ii["inter_bih"], ii["inter_bhh"],
        ii["inter_gamma"], ii["inter_beta"])
    wih2 = np.zeros((128, 8, 2, 128), np.float32)
    whh2 = np.zeros((128, 16, 128), np.float32)
    bih2 = np.zeros((128, 8), np.float32)
    for m in range(8):
        rows = we2[m * 128:(m + 1) * 128]
        for kp in range(2):
            wih2[:64, m, kp] = rows[:, :, 2 * kp].T
            wih2[64:, m, kp] = rows[:, :, 2 * kp + 1].T
        for kc in range(2):
            whh2[:, m * 2 + kc] = wp2[m * 128:(m + 1) * 128,
                                      kc * 128:(kc + 1) * 128].T
        bih2[:, m] = be2[m * 128:(m + 1) * 128]
    ctw2 = np.asarray(ii["inter_ct_w"], np.float64)
    ct2 = np.zeros((2, 256, 128), np.float32)
    for mo in range(2):
        for kp in range(2):
            for cc in range(64):
                j = kp * 64 + cc
                ct2[mo, :, j] = ctw2[:, cc, mo * 2 + kp]
    ctb2 = np.zeros((128, 2), np.float32)
    for mo in range(2):
        for kp in range(2):
            ctb2[kp * 64:(kp + 1) * 64, mo] = np.asarray(ii["inter_ct_b"])
    w["l2"] = [
        ("wih", wih2, BF16),
        ("whh", whh2, BF16),
        ("bih", bih2, F32),
        ("ctw", ct2.reshape(2, 2, 128, 128).transpose(2, 0, 1, 3)
         .reshape(128, 1, 2, 2, 128), BF16),
        ("ctb", ctb2, F32),
    ]
    # ---- L3a ----
    qg = _uniform(ii["q_g"]); kg = _uniform(ii["k_g"]); vg = _uniform(ii["v_g"])
    assert _uniform(ii["q_bt"]) == 0 and _uniform(ii["k_bt"]) == 0
    assert _uniform(ii["v_bt"]) == 0
    wall = np.zeros((64, 96), np.float32)
    bias96 = np.zeros((96,), np.float32)
    alpha96 = np.zeros((96,), np.float32)
    cnt96 = np.zeros((96,), np.float32)
    gs96 = np.zeros((96,), np.float32)
    grp = np.zeros((96,), np.int32)
    for h in range(NH):
        wall[:, h * 4:h * 4 + 4] = np.asarray(ii["q_w"][h]).T
        wall[:, 16 + h * 4:16 + h * 4 + 4] = np.asarray(ii["k_w"][h]).T
        wall[:, 32 + h * 16:32 + h * 16 + 16] = np.asarray(ii["v_w"][h]).T
        bias96[h * 4:h * 4 + 4] = np.asarray(ii["q_b"][h])
        bias96[16 + h * 4:16 + h * 4 + 4] = np.asarray(ii["k_b"][h])
        alpha96[h * 4:h * 4 + 4] = float(ii["q_p"][h])
        alpha96[16 + h * 4:16 + h * 4 + 4] = float(ii["k_p"][h])
        alpha96[32 + h * 16:32 + h * 16 + 16] = float(ii["v_p"][h])
        cnt96[h * 4:h * 4 + 4] = 1.0 / (E * Q)
        cnt96[16 + h * 4:16 + h * 4 + 4] = 1.0 / (E * Q)
        cnt96[32 + h * 16:32 + h * 16 + 16] = 1.0 / (Dv * Q)
        gs96[h * 4:h * 4 + 4] = qg / np.sqrt(E * Q)
        gs96[16 + h * 4:16 + h * 4 + 4] = kg
        gs96[32 + h * 16:32 + h * 16 + 16] = vg
        grp[h * 4:h * 4 + 4] = h
        grp[16 + h * 4:16 + h * 4 + 4] = 4 + h
        grp[32 + h * 16:32 + h * 16 + 16] = 8 + h
    gmat = (grp[:, None] == grp[None, :]).astype(np.float32)
    bs96 = np.stack([bias96, alpha96, cnt96, gs96], axis=1)
    w["l3a"] = [("wall", wall, BF16), ("bs", bs96, F32), ("gmat", gmat, BF16)]
    # ---- L3b mask ----
    mask = np.triu(np.full((128, 128), -1e9, np.float32), 1)
    w["msk"] = mask
    # ---- L3c ----
    assert _uniform(ii["proj_g"]) == 1.0 and _uniform(ii["proj_bt"]) == 0.0
    pw = np.ascontiguousarray(np.asarray(ii["proj_w"], np.float32).T)
    pb3 = np.zeros((64, 3), np.float32)
    pb3[:, 0] = np.asarray(ii["proj_b"])
    pb3[:, 1] = float(ii["proj_p"])
    w["l3c"] = [("pw", pw, BF16), ("pb", pb3, F32)]
    return w


# ======================= glue (device jnp) =======================

BF = jnp.bfloat16


def _posmajor_j(v):
    """[8, NPOS, 64] (NPOS=17000) -> [1024, 133, 64] pos-major tiles."""
    v = jnp.concatenate([v, jnp.zeros((8, G1 * 128 - NP1, 64), v.dtype)], 1)
    return v.reshape(8, G1, 128, 64).transpose(0, 2, 1, 3).reshape(
        8 * 128, G1, 64)


def glue0_fn(xg):
    """xg [8, 64, 250, 68] f16 -> (x_pm bf16 [1024,133,64],
    x_u f32 [1024,2,4250])."""
    v32 = xg.astype(jnp.float32)
    pm = _posmajor_j(v32.transpose(0, 2, 3, 1).reshape(8, NP1, 64)
                     .astype(BF))
    xu = v32.reshape(8, 64, 250, 17, 2, 2).transpose(0, 4, 5, 1, 3, 2)
    xu = xu.reshape(8, 2, 128, NT1).transpose(0, 2, 1, 3).reshape(
        1024, 2, NT1)
    return pm, xu


GROUPS = [[0, 1, 2, 3], [4, 5, 6, 7]]


def _a2a(x):
    """all-to-all within the 4-core b-group along dim0 (size 4)."""
    return jax.lax.all_to_all(x, "core", 0, 0, tiled=True,
                              axis_index_groups=GROUPS)


def _posmajor_l(rows_pc):
    """local [NPOS=17000, 64] -> [128, 133, 64] pos-major tiles."""
    v = jnp.concatenate(
        [rows_pc, jnp.zeros((G1 * 128 - NP1, 64), rows_pc.dtype)], 0)
    return v.reshape(G1, 128, 64).transpose(1, 0, 2)


def glue1_local(o1l):
    """per-core l1 outu [128,2,4250] f32 -> l2 (x_pm, x_u) local blocks.

    Core (b,tc) holds intra t-chunk; l2 wants q-chunks: all-to-all
    within the 4-core b-group."""
    ou = o1l.reshape(2, 64, 2, 17, 250)          # [kp, c, mo, l, t]
    intra = ou.transpose(1, 4, 3, 2, 0).reshape(64, 250, 68)
    s4 = intra.reshape(64, 250, 4, 17).transpose(2, 0, 1, 3)
    rcv = _a2a(s4)                               # [tc', 64, 250, 17]
    rows = rcv.transpose(1, 3, 0, 2).reshape(64, 17, 1000)  # [c, r, t]
    pm2 = _posmajor_l(rows.transpose(1, 2, 0).reshape(NP1, 64).astype(BF))
    xu2 = rows.reshape(64, 17, 250, 2, 2).transpose(3, 4, 0, 2, 1)
    xu2 = xu2.reshape(2, 128, NT2).transpose(1, 0, 2)
    return pm2, xu2


def glue2_local(o2l):
    """per-core l2 outu [128,2,4250] f32 -> (icm bf16, icm f32) local."""
    ou = o2l.reshape(2, 64, 2, 250, 17)          # [kp, c, mo, l, r]
    rows = ou.transpose(1, 4, 3, 2, 0).reshape(64, 17, 1000)  # [c, r, t]
    s4 = rows.reshape(64, 17, 4, 250).transpose(2, 0, 1, 3)
    rcv = _a2a(s4)                               # [qc', 64, 17, 250]
    tloc = rcv.transpose(1, 3, 0, 2).reshape(64, 250, 68)     # [c, t', q]
    tloc = jnp.concatenate(
        [tloc[:, :, :Q], jnp.zeros((64, 250, Qp - Q), tloc.dtype)], 2)
    return tloc.astype(BF), tloc


def glue3_local(qkvl):
    """per-core l3a qkvo [96,250,68] bf16 -> (qT, kT, vm) local."""
    qs = qkvl[0:16].reshape(4, 4, 250, 68)
    ks_ = qkvl[16:32].reshape(4, 4, 250, 68)
    vs = qkvl[32:96].reshape(4, 16, 250, 68)
    s4 = jnp.concatenate([qs, ks_, vs], axis=1)  # [h, 24, 250, 68]
    rcv = _a2a(s4)                               # [tc', 24, 250, 68]
    qkvh = rcv.transpose(1, 0, 2, 3).reshape(24, 1000, 68)

    def fm(a):                                   # [4, 1000, 68] -> ef-major
        t = a.transpose(0, 2, 1).reshape(272, 1000)
        t = jnp.concatenate([t, jnp.zeros((112, 1000), a.dtype)], 0)
        return t.reshape(3, 128, 1000).transpose(1, 0, 2)

    qT = fm(qkvh[0:4])
    kT = fm(qkvh[4:8])
    vmm = qkvh[8:24].transpose(1, 0, 2).reshape(1000, Dv * Qp)
    vmm = jnp.concatenate(
        [vmm, jnp.zeros((24, Dv * Qp), vmm.dtype)], 0)
    vmm = vmm.reshape(8, 128, Dv * Qp).transpose(1, 0, 2)
    return qT, kT, vmm


def glue4_local(avol):
    """per-core l3b avo [128,8,1088] bf16 -> avf [64,250,68] local."""
    a = avol.transpose(1, 0, 2).reshape(1024, Dv, Qp)[:1000]
    s4 = a.reshape(4, 250, Dv, Qp)               # [tc, t', d, f]
    rcv = _a2a(s4)                               # [h', 250, 16, 68]
    return rcv.transpose(0, 2, 1, 3).reshape(64, 250, 68)


def glue5_fn(outo):
    return outo.astype(jnp.float16)


# ---- two-pass (per-b) pipelined variants ----
PAIRS = [[0, 4], [1, 5], [2, 6], [3, 7]]


def glue0p_local(xl):
    """pass-mode input glue. xl local [32, 250, 68] f16: core r holds
    channel-half r//4 of t-chunk r%4 of this pass's b. Pair all-gather
    rebuilds the full [64, 250, 68] chunk on both group members, then
    the usual pos-major + unfold transforms."""
    full = jax.lax.all_gather(xl, "core", axis_index_groups=PAIRS)
    v32 = full.reshape(64, 250, 68).astype(jnp.float32)
    pm = _posmajor_l(v32.transpose(1, 2, 0).reshape(NP1, 64).astype(BF))
    xu = v32.reshape(64, 250, 17, 2, 2).transpose(3, 4, 0, 2, 1)
    xu = xu.reshape(2, 128, NT1).transpose(1, 0, 2)
    return pm, xu


def glue5p_local(outol):
    """pass-mode output glue. outol local [64, 250, 65] f32; core r
    returns channel-half r//4 so the global D2H is half-sized."""
    idx = jax.lax.axis_index("core")
    m = (idx >= 4).astype(jnp.float16)
    lo = outol[0:32].astype(jnp.float16)
    hi = outol[32:64].astype(jnp.float16)
    return lo * (1 - m) + hi * m


def zeros_fn():
    return (jnp.zeros((1024, 2, NT1), jnp.float32),
            jnp.zeros((1024, 2, NT2), jnp.float32),
            jnp.zeros((768, 250, 68), BF),
            jnp.zeros((1024, 8, Dv * Qp), BF),
            jnp.zeros((512, 250, Q), jnp.float32))


# ======================= stage runner =======================

class _StageRunner:
    """jit(shard_map(bass_exec)) built once per stage, reused every call."""

    def __init__(self, nc, mesh, n_cores=NCORES):
        install_neuronx_cc_hook()
        self.nc = nc
        partition_name = (nc.partition_id_tensor.name
                          if nc.partition_id_tensor else None)
        dbg_name = nc.dbg_addr.name if nc.dbg_addr is not None else None
        assert not nc.dbg_callbacks
        in_names, out_names, out_avals = [], [], []
        for alloc in nc.m.functions[0].allocations:
            if not isinstance(alloc, mybir.MemoryLocationSet):
                continue
            name = alloc.memorylocations[0].name
            if alloc.kind == "ExternalInput":
                if name != partition_name:
                    in_names.append(name)
            elif alloc.kind == "ExternalOutput":
                out_names.append(name)
                out_avals.append(jax.core.ShapedArray(
                    tuple(alloc.tensor_shape), mybir.dt.np(alloc.dtype)))
        assert dbg_name is None or dbg_name in in_names
        self.in_names, self.out_names = in_names, out_names
        n_params, n_outs = len(in_names), len(out_names)
        all_names = list(in_names) + list(out_names)
        if partition_name is not None:
            all_names.append(partition_name)

        def _body(*args):
            operands = list(args)
            if partition_name is not None:
                operands.append(partition_id_tensor())
            outs = _bass_exec_p.bind(
                *operands,
                out_avals=tuple(out_avals),
                in_names=tuple(all_names),
                out_names=tuple(out_names),
                lowering_input_output_aliases=(),
                sim_require_finite=True,
                sim_require_nnan=True,
                nc=nc,
            )
            return tuple(outs)

        in_specs = (PartitionSpec("core"),) * (n_params + n_outs)
        out_specs = (PartitionSpec("core"),) * n_outs
        donate = tuple(range(n_params, n_params + n_outs))
        self.fn = jax.jit(
            shard_map(_body, mesh=mesh, in_specs=in_specs,
                      out_specs=out_specs, check_rep=False),
            donate_argnums=donate, keep_unused=True)

    def __call__(self, arrs, zeros):
        return self.fn(*arrs, *zeros)


# ======================= host orchestration =======================

_C = {}


def _wfingerprint(ii):
    keys = ["intra_wih", "intra_whh", "inter_wih", "inter_whh", "q_w",
            "k_w", "v_w", "proj_w", "intra_ct_w", "inter_ct_w"]
    return tuple(float(np.asarray(ii[k]).sum()) for k in keys)


def _ensure(ii):
    if "mesh" not in _C:
        devs = jax.devices()[:NCORES]
        _C["mesh"] = Mesh(np.asarray(devs), ("core",))
        _C["shard"] = NamedSharding(_C["mesh"], PartitionSpec("core"))
    mesh, shard = _C["mesh"], _C["shard"]
    if "l1" not in _C:
        _C["l1"] = _StageRunner(build_lstm_launch("intra"), mesh)
        _C["l2"] = _StageRunner(build_lstm_launch("inter"), mesh)
        _C["l3a"] = _StageRunner(build_l3a(), mesh)
        _C["l3b"] = _StageRunner(build_l3b(), mesh)
        _C["l3c"] = _StageRunner(build_l3c(), mesh)
        jt = lambda f: jax.jit(f, out_shardings=shard)
        P = PartitionSpec("core")

        def sm(f, nin, nout):
            return jax.jit(shard_map(
                f, mesh=mesh, in_specs=(P,) * nin,
                out_specs=(P,) * nout if nout > 1 else P, check_rep=False))

        _C["g0"] = jt(glue0_fn)
        _C["g0p"] = sm(glue0p_local, 1, 2)
        _C["g1"] = sm(glue1_local, 1, 2)
        _C["g2"] = sm(glue2_local, 1, 2)
        _C["g3"] = sm(glue3_local, 1, 3)
        _C["g4"] = sm(glue4_local, 1, 1)
        _C["g5"] = jt(glue5_fn)
        _C["g5p"] = sm(glue5p_local, 1, 1)
        _C["zeros"] = jax.jit(zeros_fn, out_shardings=shard)
    fp = _wfingerprint(ii)
    if _C.get("wfp") != fp:
        w = _build_weight_arrays(ii)
        put = lambda a: jax.device_put(
            np.ascontiguousarray(np.tile(np.asarray(a, np.float32),
                                         (NCORES,) + (1,) * (a.ndim - 1))),
            _C["shard"])
        devw = {}
        for stage in ["l1", "l2", "l3a", "l3c"]:
            names, arrs, dts = zip(*w[stage])
            f32d = [put(a) for a in arrs]
            castfn = jax.jit(
                lambda *xs, dts=dts: tuple(
                    x.astype(jnp.bfloat16) if dt == BF16 else x
                    for x, dt in zip(xs, dts)),
                out_shardings=_C["shard"])
            casted = castfn(*f32d)
            devw[stage] = dict(zip(names, casted))
        devw["msk"] = jax.device_put(
            np.tile(w["msk"], (NCORES, 1)), _C["shard"])
        _C["w"] = devw
        _C["wfp"] = fp


def _stage_inputs(runner, data, weights):
    m = dict(data)
    m.update(weights)
    return [m[n] for n in runner.in_names]


TWO_PASS = os.environ.get("K_TWO_PASS", "1") == "1"


def _run_chain(first_glue, xd, last_glue):
    w = _C["w"]
    z1, z2, z3a, z3b, z3c = _C["zeros"]()
    pm1, xu1 = first_glue(xd)
    (o1,) = _C["l1"](_stage_inputs(_C["l1"], {"x_pm": pm1, "x_u": xu1},
                                   w["l1"]), [z1])
    pm2, xu2 = _C["g1"](o1)
    (o2,) = _C["l2"](_stage_inputs(_C["l2"], {"x_pm": pm2, "x_u": xu2},
                                   w["l2"]), [z2])
    icm_bf, icm_f = _C["g2"](o2)
    (qkv,) = _C["l3a"](_stage_inputs(_C["l3a"], {"icm": icm_bf}, w["l3a"]),
                       [z3a])
    qT, kT, vm = _C["g3"](qkv)
    (avo,) = _C["l3b"](_stage_inputs(
        _C["l3b"], {"qT": qT, "kT": kT, "vm": vm, "msk": w["msk"]}, {}),
        [z3b])
    avf = _C["g4"](avo)
    (outo,) = _C["l3c"](_stage_inputs(
        _C["l3c"], {"avf": avf, "icm": icm_f}, w["l3c"]), [z3c])
    return last_glue(outo)


def kernel(**inputs):
    ii = {k: np.asarray(v) for k, v in inputs.items()}
    _ensure(ii)
    x = np.asarray(ii["x"], np.float32)

    if not TWO_PASS:
        xg = np.zeros((2, 4, 64, 250, 68), np.float16)
        xg[..., :Q] = x.reshape(2, 64, 4, 250, Q).transpose(0, 2, 1, 3, 4)
        xd = jax.device_put(xg.reshape(8, 64, 250, 68), _C["shard"])
        of = _run_chain(_C["g0"], xd, _C["g5"])
        oh = np.asarray(of)     # [512, 250, 65] f16
        out = oh.reshape(2, 4, 64, 250, Q).transpose(0, 2, 1, 3, 4)
        return np.ascontiguousarray(out.reshape(2, 64, 1000, Q)
                                    .astype(np.float32))

    # two-pass pipelined: pass b uploads half the bytes (channel-halves
    # across core pairs), both 4-core groups compute that b, pass 0's
    # download overlaps pass 1's upload (the tunnel is full duplex).
    import threading
    out = np.empty((2, 64, 1000, Q), np.float32)

    def pack(b):
        g = np.zeros((2, 4, 32, 250, 68), np.float16)
        g[..., :Q] = x[b].reshape(2, 32, 4, 250, Q).transpose(0, 2, 1, 3, 4)
        return g.reshape(8, 32, 250, 68)

    def drain(b, of):
        oh = np.asarray(of)     # [256, 250, 65] f16, rows (half, chunk)
        o = oh.reshape(2, 4, 32, 250, Q).transpose(0, 2, 1, 3, 4)
        out[b] = o.reshape(64, 1000, Q).astype(np.float32)

    xg0 = pack(0)
    xg1 = pack(1)
    xd0 = jax.device_put(xg0, _C["shard"])
    of0 = _run_chain(_C["g0p"], xd0, _C["g5p"])
    th = threading.Thread(target=drain, args=(0, of0))
    th.start()
    xd1 = jax.device_put(xg1, _C["shard"])
    of1 = _run_chain(_C["g0p"], xd1, _C["g5p"])
    drain(1, of1)
    th.join()
    return out


# revision 12
# speedup vs baseline: 16.6809x; 1.0432x over previous
"""GridNetBlock (TF-GridNet) Trainium2 kernel: 8-core SPMD.

v2: fully device-resident pipeline. The five Bass stage kernels from v1
are unchanged, but all inter-stage glue (unfold/reshard/transpose) now
runs on-device as jitted jnp ops, weights are uploaded once and cached,
and host<->device traffic per call is only x (f16 in) + output (f16
out). The axon tunnel moves ~33MB/s, so this is what dominates wall.
"""
import sys, os, contextlib
for _p in ("/opt/trn_rl_repo", "/root/.axon_site/_ro/trn_rl_repo"):
    if os.path.isdir(_p) and _p not in sys.path:
        sys.path.insert(0, _p)
import numpy as np
import jax
import jax.numpy as jnp
from jax.sharding import Mesh, PartitionSpec, NamedSharding
from jax.experimental.shard_map import shard_map
import concourse.bass as bass
import concourse.bacc as bacc
import concourse.tile as tile
from concourse import mybir
from concourse.masks import make_identity
from concourse.bass2jax import (_bass_exec_p, install_neuronx_cc_hook,
                                partition_id_tensor)

F32 = mybir.dt.float32
BF16 = mybir.dt.bfloat16
AF = mybir.ActivationFunctionType
OP = mybir.AluOpType
AX = mybir.AxisListType

B, C, T, Q = 2, 64, 1000, 65
KS = 4
Qp, L1, Hh, HID, L2 = 68, 17, 128, 256, 250
NH, E, Dv = 4, 4, 16
EPS = 1e-5
NCORES = 8
TSH = T // 4
NP1 = TSH * Qp
G1 = (NP1 + 127) // 128   # 133
RW2 = (B * Qp) // NCORES  # 17
NT1 = L1 * TSH            # 4250
NT2 = L2 * RW2            # 4250


def bap(t, tail):
    ap = list(t.ap)
    for n in tail:
        ap.append([0, n])
    return bass.AP(tensor=t.tensor, offset=t.offset, ap=ap)


def new_nc():
    return bacc.Bacc("TRN2", target_bir_lowering=False, debug=False,
                     enable_asserts=True, num_devices=NCORES)


def ln_posmajor(nc, pool, work, xpm, G, nred, eps_t):
    s1 = work.tile([128, G], F32, tag="lns1")
    nc.vector.tensor_reduce(out=s1[:], in_=xpm[:], axis=AX.X, op=OP.add)
    xsq = pool.tile([128, G, nred], BF16, tag="xut")
    nc.scalar.activation(out=xsq[:], in_=xpm[:], func=AF.Square)
    s2 = work.tile([128, G], F32, tag="lns2")
    nc.vector.tensor_reduce(out=s2[:], in_=xsq[:], axis=AX.X, op=OP.add)
    mu = work.tile([128, G], F32, tag="lnmu")
    nc.vector.tensor_scalar_mul(out=mu[:], in0=s1[:], scalar1=1.0 / nred)
    var = work.tile([128, G], F32, tag="lnvar")
    nc.vector.tensor_tensor(out=var[:], in0=mu[:], in1=mu[:], op=OP.mult)
    nc.vector.scalar_tensor_tensor(out=var[:], in0=s2[:], scalar=1.0 / nred,
                                   in1=var[:], op0=OP.mult, op1=OP.subtract)
    rs = work.tile([128, G], F32, tag="lnrs")
    nc.scalar.activation(out=rs[:], in_=var[:], func=AF.Sqrt, bias=eps_t[:])
    nc.vector.reciprocal(out=rs[:], in_=rs[:])
    zpm = pool.tile([128, G, nred], BF16, tag="xut")
    nc.vector.tensor_tensor(out=zpm[:], in0=xpm[:], in1=bap(mu, [nred]),
                            op=OP.subtract)
    nc.vector.tensor_tensor(out=zpm[:], in0=zpm[:], in1=bap(rs, [nred]),
                            op=OP.mult)
    return zpm


def ap3(t, off, d1, n1, d2, n2):
    """Strided 2-free-dim AP view of tile t at element offset off."""
    return bass.AP(tensor=t.tensor, offset=t.offset + off,
                   ap=[t.ap[0], [d1, n1], [d2, n2]])


def build_lstm_launch(which):
    """which: 'intra' or 'inter'. Returns compiled nc."""
    intra = which == "intra"
    ND = 2 if intra else 1
    MC = 4 if intra else 8
    KC = 1 if intra else 2
    L = L1 if intra else L2
    NB = TSH if intra else RW2        # lstm batch per core
    NT = L * NB                       # 4250
    G = G1
    ZC = G * 128

    nc = new_nc()
    x_pm = nc.dram_tensor("x_pm", [128, G, C], BF16, kind="ExternalInput")
    x_u = nc.dram_tensor("x_u", [128, 2, NT], F32, kind="ExternalInput")
    if intra:
        wih = nc.dram_tensor("wih", [65, 2, 4, 4, 128], BF16,
                             kind="ExternalInput")
        whh = nc.dram_tensor("whh", [128, 2, 4, 128], BF16,
                             kind="ExternalInput")
    else:
        wih = nc.dram_tensor("wih", [128, 8, 2, 128], BF16,
                             kind="ExternalInput")
        whh = nc.dram_tensor("whh", [128, 16, 128], BF16,
                             kind="ExternalInput")
        bih = nc.dram_tensor("bih", [128, 8], F32, kind="ExternalInput")
    ctw = nc.dram_tensor("ctw", [128, ND, 2, KC, 128], BF16,
                         kind="ExternalInput")
    ctb = nc.dram_tensor("ctb", [128, 2], F32, kind="ExternalInput")
    outu = nc.dram_tensor("outu", [128, 2, NT], F32, kind="ExternalOutput")

    ctx = contextlib.ExitStack()
    with tile.TileContext(nc) as tc, ctx:
        const = ctx.enter_context(tc.tile_pool(name="const", bufs=1))
        persist = ctx.enter_context(tc.tile_pool(name="persist", bufs=1))
        psum = ctx.enter_context(tc.tile_pool(name="psum", bufs=2,
                                              space="PSUM"))
        psumB = ctx.enter_context(tc.tile_pool(name="psumB", bufs=2,
                                               space="PSUM"))
        psumS = ctx.enter_context(tc.tile_pool(
            name="psumS", bufs=1 if intra else 2, space="PSUM"))

        eps_t = const.tile([128, 1], F32)
        nc.vector.memset(eps_t[:], EPS)
        ident = const.tile([128, 128], BF16)
        make_identity(nc, ident[:])

        if intra:
            wih_t = const.tile([65, 2, 4, 4, 128], BF16)
            whh_t = const.tile([128, 2, 4, 128], BF16)
        else:
            wih_t = const.tile([128, 8, 2, 128], BF16)
            whh_t = const.tile([128, 16, 128], BF16)
            bih_t = const.tile([128, 8], F32)
            nc.sync.dma_start(out=bih_t[:], in_=bih[:])
        nc.sync.dma_start(out=wih_t[:], in_=wih[:])
        nc.sync.dma_start(out=whh_t[:], in_=whh[:])
        ct_tl = const.tile([128, ND, 2, KC, 128], BF16)
        nc.sync.dma_start(out=ct_tl[:], in_=ctw[:])
        ctb_t = const.tile([128, 2], F32)
        nc.sync.dma_start(out=ctb_t[:], in_=ctb[:])

        # --- persistent tiles ---
        if intra:
            # z channel-major [65, ZC]; row 64 = ones (bias row)
            z_cm = persist.tile([65, ZC], BF16)
            hbufs = [persist.tile([128, L, NB], BF16, name=f"hb{d}")
                     for d in range(ND * KC)]
        else:
            # z doubled rows: p<64: z[c, t'-3]; p>=64: z[c, t'-2]
            z2d = persist.tile([128, RW2, 1000], BF16)
            hb2 = persist.tile([128, 2, L, NB], BF16)
        ou = persist.tile([128, 2, L, NB], F32)

        # --- LN over C (pos-major) + transpose to channel-major ---
        with tc.tile_pool(name="tmpA", bufs=1) as tmpA:
            xpm = tmpA.tile([128, G, C], BF16)
            nc.sync.dma_start(out=xpm[:], in_=x_pm[:])
            work = tmpA
            zpm = ln_posmajor(nc, tmpA, work, xpm, G, C, eps_t)
            if intra:
                z_dst = z_cm
                nc.gpsimd.memset(z_cm[64:65, :], 1.0)
            else:
                z_tmp = tmpA.tile([64, ZC], BF16)
                z_dst = z_tmp
            for g0 in range(0, G, 4):
                gn = min(4, G - g0)
                pt = psum.tile([64, 4, 128], BF16, tag="tps")
                for gg in range(gn):
                    nc.tensor.transpose(pt[:, gg, :], zpm[:, g0 + gg, :],
                                        ident[:])
                if (g0 // 4) % 2 == 0:
                    nc.scalar.copy(
                        out=z_dst[0:64, g0 * 128:(g0 + gn) * 128],
                        in_=pt[:, 0:gn, :].rearrange("p a b -> p (a b)"))
                else:
                    nc.vector.tensor_copy(
                        out=z_dst[0:64, g0 * 128:(g0 + gn) * 128],
                        in_=pt[:, 0:gn, :].rearrange("p a b -> p (a b)"))
            if not intra:
                # z_tmp [64, row*1000+t] -> z2d two shifted copies
                nc.vector.memset(z2d[0:64, :, 0:3], 0.0)
                nc.vector.memset(z2d[64:128, :, 0:2], 0.0)
                nc.sync.dma_start(
                    out=z2d[0:64, :, 3:1000],
                    in_=ap3(z_tmp, 0, 1000, RW2, 1, 997))
                nc.sync.dma_start(
                    out=z2d[64:128, :, 2:1000],
                    in_=ap3(z_tmp, 0, 1000, RW2, 1, 998))

        # --- gate precompute (inter only): pre2 [128, L, 8, RW2] bf16 ---
        rec = ctx.enter_context(tc.tile_pool(name="rec", bufs=1))
        if not intra:
            pre2 = rec.tile([128, L, 8, RW2], BF16)
            LSUB = 30
            for l0 in range(0, L, LSUB):
                ln_ = min(LSUB, L - l0)
                for m in range(8):
                    pp = psumB.tile([128, 512], F32, tag="ppre")
                    for kp in range(2):
                        rhs = ap3(z2d, 4 * l0 + 2 * kp, 4, ln_, 1000, RW2)
                        nc.tensor.matmul(pp[:, :RW2 * ln_],
                                         wih_t[:, m, kp, :], rhs,
                                         start=(kp == 0), stop=(kp == 1))
                    dst = pre2[:, l0:l0 + ln_, m, :]
                    src = pp[:, :RW2 * ln_].rearrange(
                        "p (l r) -> p l r", r=RW2)
                    if m % 2 == 0:
                        nc.vector.tensor_scalar_add(out=dst, in0=src,
                                                    scalar1=bih_t[:, m:m + 1])
                    else:
                        nc.scalar.activation(out=dst, in_=src,
                                             func=AF.Identity,
                                             bias=bih_t[:, m:m + 1])

        # --- recurrence (interleaved directions) ---
        NGC = MC // 4                      # hidden chunks (1 or 2)
        gsb = [rec.tile([128, MC, NB], BF16, name=f"gs{d}")
               for d in range(ND)]
        c_t = [rec.tile([128, NGC, NB], F32, name=f"ct{d}")
               for d in range(ND)]
        ig_t = [rec.tile([128, NGC, NB], BF16, name=f"ig{d}")
               for d in range(ND)]
        tc_t = [rec.tile([128, NGC, NB], BF16, name=f"tc{d}")
               for d in range(ND)]
        slot = 64 if NB <= 64 else 256

        def step(d, l, first):
            lp = l + 1 if (intra and d == 1) else l - 1
            ps = psumS.tile([128, MC, slot if intra else RW2], F32,
                            tag=f"lps{d}")
            if intra:
                for m in range(4):
                    st = Qp
                    for k in range(4):
                        o0 = 4 * l + k
                        rhs = bass.AP(
                            tensor=z_cm.tensor, offset=z_cm.offset + o0,
                            ap=[z_cm.ap[0], [st, NB]])
                        nc.tensor.matmul(ps[:, m, :NB],
                                         wih_t[:, d, m, k, :], rhs,
                                         start=(k == 0),
                                         stop=(k == 3 and first))
                    if not first:
                        nc.tensor.matmul(ps[:, m, :NB], whh_t[:, d, m, :],
                                         hbufs[d][:, lp, :],
                                         start=False, stop=True)
            else:
                if not first:
                    nc.tensor.matmul(
                        ps[:].rearrange("p m n -> p (m n)"), ident[:],
                        pre2[:, l, :, :].rearrange("p m n -> p (m n)"),
                        start=True, stop=False, skip_group_check=True)
                    for m in range(8):
                        for kc in range(2):
                            nc.tensor.matmul(ps[:, m, :NB],
                                             whh_t[:, m * 2 + kc, :],
                                             hb2[:, kc, lp, :],
                                             start=False, stop=(kc == 1),
                                             skip_group_check=True)
            # gates: i (NGC), f (NGC), o (NGC), g (NGC)
            gg = gsb[d]
            if first and not intra:
                sig_in = pre2[:, l, 0:3 * NGC, :]
                tanh_in = pre2[:, l, 3 * NGC:, :]
            else:
                sig_in = ps[:, 0:3 * NGC, :NB]
                tanh_in = ps[:, 3 * NGC:, :NB]
            nc.scalar.activation(out=gg[:, 0:3 * NGC, :], in_=sig_in,
                                 func=AF.Sigmoid)
            nc.scalar.activation(out=gg[:, 3 * NGC:, :], in_=tanh_in,
                                 func=AF.Tanh)
            i_g, f_g = gg[:, 0:NGC, :], gg[:, NGC:2 * NGC, :]
            o_g, g_g = gg[:, 2 * NGC:3 * NGC, :], gg[:, 3 * NGC:, :]
            if first:
                nc.vector.tensor_tensor(out=c_t[d][:], in0=i_g, in1=g_g,
                                        op=OP.mult)
            else:
                nc.vector.tensor_tensor(out=ig_t[d][:], in0=i_g, in1=g_g,
                                        op=OP.mult)
                nc.vector.tensor_tensor(out=c_t[d][:], in0=f_g, in1=c_t[d][:],
                                        op=OP.mult)
                nc.vector.tensor_tensor(out=c_t[d][:], in0=c_t[d][:],
                                        in1=ig_t[d][:], op=OP.add)
            nc.scalar.activation(out=tc_t[d][:], in_=c_t[d][:], func=AF.Tanh)
            if intra:
                nc.vector.tensor_tensor(out=hbufs[d][:, l, :], in0=o_g,
                                        in1=tc_t[d][:], op=OP.mult)
            else:
                nc.vector.tensor_tensor(out=hb2[:, :, l, :], in0=o_g,
                                        in1=tc_t[d][:], op=OP.mult)

        # --- ConvT + bias + residual, l-chunked, interleaved with steps ---
        xu_t = rec.tile([128, 2, NT], F32)
        nc.sync.dma_start(out=xu_t[:], in_=x_u[:])
        CL = 2 if intra else 30

        def convt_chunk(l0):
            ln_ = min(CL, L - l0)
            nn_ = ln_ * NB
            for mo in range(2):
                ps2 = psumB.tile([128, 512], F32, tag="pct")
                nch = 0
                for d in range(ND):
                    for k in range(KC):
                        hsl = (hbufs[d][:, l0:l0 + ln_, :] if intra
                               else hb2[:, k, l0:l0 + ln_, :])
                        nc.tensor.matmul(
                            ps2[:, :nn_], ct_tl[:, d, mo, k, :],
                            hsl.rearrange("p l t -> p (l t)"),
                            start=(nch == 0), stop=(nch == ND * KC - 1))
                        nch += 1
                nc.vector.scalar_tensor_tensor(
                    out=ou[:, mo, l0:l0 + ln_, :].rearrange(
                        "p l t -> p (l t)"),
                    in0=ps2[:, :nn_], scalar=ctb_t[:, mo:mo + 1],
                    in1=xu_t[:, mo, l0 * NB:l0 * NB + nn_],
                    op0=OP.add, op1=OP.add)

        pending = list(range(0, L, CL))
        for i in range(L):
            step(0, i, i == 0)
            if intra:
                step(1, L - 1 - i, i == 0)
            for l0 in list(pending):
                ln_ = min(CL, L - l0)
                ready = i >= l0 + ln_ - 1
                if intra:
                    ready = ready and i >= L - 1 - l0
                if ready:
                    convt_chunk(l0)
                    pending.remove(l0)
        for l0 in pending:
            convt_chunk(l0)
        nc.sync.dma_start(out=outu[:],
                          in_=ou[:].rearrange("p a l t -> p a (l t)"))
    nc.compile()
    return nc


# ---------------- Launch 3a: QKV conv + PReLU + LN ----------------

def build_l3a():
    nc = new_nc()
    icm = nc.dram_tensor("icm", [64, TSH, Qp], BF16, kind="ExternalInput")
    wall = nc.dram_tensor("wall", [64, 96], BF16, kind="ExternalInput")
    bs = nc.dram_tensor("bs", [96, 4], F32, kind="ExternalInput")
    # bs cols: bias, alpha, cnt_inv, gscale (per row)
    gmat = nc.dram_tensor("gmat", [96, 96], BF16, kind="ExternalInput")
    qkvo = nc.dram_tensor("qkvo", [96, TSH, Qp], BF16, kind="ExternalOutput")
    NTF = TSH * Qp  # 17000
    ctx = contextlib.ExitStack()
    with tile.TileContext(nc) as tc, ctx:
        const = ctx.enter_context(tc.tile_pool(name="const", bufs=1))
        big = ctx.enter_context(tc.tile_pool(name="big", bufs=1))
        work = ctx.enter_context(tc.tile_pool(name="work", bufs=2))
        psum = ctx.enter_context(tc.tile_pool(name="psum", bufs=2, space="PSUM"))
        eps_t = const.tile([96, 1], F32)
        nc.vector.memset(eps_t[:], EPS)
        ict = big.tile([64, NTF], BF16, tag="ict")
        nc.sync.dma_start(out=ict[:], in_=icm.rearrange("c t f -> c (t f)"))
        wt = const.tile([64, 96], BF16)
        nc.sync.dma_start(out=wt[:], in_=wall[:])
        bst = const.tile([96, 4], F32)
        nc.sync.dma_start(out=bst[:], in_=bs[:])
        gm = const.tile([96, 96], BF16)
        nc.sync.dma_start(out=gm[:], in_=gmat[:])

        qr = big.tile([96, NTF], F32, tag="qr")
        for n0 in range(0, NTF, 512):
            nn_ = min(512, NTF - n0)
            ps = psum.tile([96, 512], F32, tag="pc")
            nc.tensor.matmul(ps[:, :nn_], wt[:], ict[:, n0:n0 + nn_],
                             start=True, stop=True)
            nc.scalar.activation(out=qr[:, n0:n0 + nn_], in_=ps[:, :nn_],
                                 func=AF.Prelu, bias=bst[:, 0:1],
                                 alpha=bst[:, 1:2])
        # stats over (e,f) groups: reduce f, then group-collapse via gmat
        s1 = work.tile([96, TSH], F32, tag="s1")
        nc.vector.tensor_reduce(out=s1[:], in_=qr[:].rearrange(
            "p (t f) -> p t f", f=Qp), axis=AX.X, op=OP.add)
        sq = big.tile([96, NTF], BF16, tag="sq")
        nc.scalar.activation(out=sq[:], in_=qr[:], func=AF.Square)
        s2 = work.tile([96, TSH], F32, tag="s2")
        nc.vector.tensor_reduce(out=s2[:], in_=sq[:].rearrange(
            "p (t f) -> p t f", f=Qp), axis=AX.X, op=OP.add)
        s1b = work.tile([96, TSH], BF16, tag="s1b")
        nc.vector.tensor_copy(out=s1b[:], in_=s1[:])
        s2b = work.tile([96, TSH], BF16, tag="s2b")
        nc.vector.tensor_copy(out=s2b[:], in_=s2[:])
        mu = work.tile([96, TSH], F32, tag="mu")
        ps1 = psum.tile([96, TSH], F32, tag="pg1")
        nc.tensor.matmul(ps1[:], gm[:], s1b[:], start=True, stop=True)
        nc.vector.tensor_scalar_mul(out=mu[:], in0=ps1[:], scalar1=bst[:, 2:3])
        var = work.tile([96, TSH], F32, tag="var")
        ps2g = psum.tile([96, TSH], F32, tag="pg2")
        nc.tensor.matmul(ps2g[:], gm[:], s2b[:], start=True, stop=True)
        nc.vector.tensor_scalar_mul(out=var[:], in0=ps2g[:], scalar1=bst[:, 2:3])
        mu2 = work.tile([96, TSH], F32, tag="mu2")
        nc.vector.tensor_tensor(out=mu2[:], in0=mu[:], in1=mu[:], op=OP.mult)
        nc.vector.tensor_tensor(out=var[:], in0=var[:], in1=mu2[:],
                                op=OP.subtract)
        rs = work.tile([96, TSH], F32, tag="rs")
        nc.scalar.activation(out=rs[:], in_=var[:], func=AF.Sqrt, bias=eps_t[:])
        nc.vector.reciprocal(out=rs[:], in_=rs[:])
        nc.vector.tensor_scalar_mul(out=rs[:], in0=rs[:], scalar1=bst[:, 3:4])
        zh = big.tile([96, TSH, Qp], BF16, tag="zh")
        qr3 = qr[:].rearrange("p (t f) -> p t f", f=Qp)
        nc.vector.tensor_tensor(out=zh[:], in0=qr3, in1=bap(mu, [Qp]),
                                op=OP.subtract)
        nc.vector.tensor_tensor(out=zh[:], in0=zh[:], in1=bap(rs, [Qp]),
                                op=OP.mult)
        nc.vector.memset(zh[:, :, Q:Qp], 0.0)
        nc.sync.dma_start(out=qkvo[:], in_=zh[:])
    nc.compile()
    return nc


# ---------------- Launch 3b: attention per (h,b) ----------------

def build_l3b():
    nc = new_nc()
    qT = nc.dram_tensor("qT", [128, 3, T], BF16, kind="ExternalInput")
    kT = nc.dram_tensor("kT", [128, 3, T], BF16, kind="ExternalInput")
    vm = nc.dram_tensor("vm", [128, 8, Dv * Qp], BF16,
                        kind="ExternalInput")
    msk = nc.dram_tensor("msk", [128, 128], F32, kind="ExternalInput")
    avo = nc.dram_tensor("avo", [128, 8, Dv * Qp], BF16,
                         kind="ExternalOutput")
    DFv = Dv * Qp
    ctx = contextlib.ExitStack()
    with tile.TileContext(nc) as tc, ctx:
        const = ctx.enter_context(tc.tile_pool(name="const", bufs=1))
        big = ctx.enter_context(tc.tile_pool(name="big", bufs=1))
        work = ctx.enter_context(tc.tile_pool(name="work", bufs=3))
        psum = ctx.enter_context(tc.tile_pool(name="psum", bufs=2, space="PSUM"))
        psumB = ctx.enter_context(tc.tile_pool(name="psumB", bufs=1,
                                               space="PSUM"))
        ident = const.tile([128, 128], F32)
        make_identity(nc, ident[:])
        qt_t = big.tile([128, 3, T], BF16, tag="qt")
        nc.sync.dma_start(out=qt_t[:], in_=qT[:])
        kt_t = big.tile([128, 3, T], BF16, tag="kt")
        nc.sync.dma_start(out=kt_t[:], in_=kT[:])
        vm_t = big.tile([128, 8, DFv], BF16, tag="vm")
        nc.sync.dma_start(out=vm_t[:], in_=vm[:])
        msk_t = const.tile([128, 128], F32)
        nc.sync.dma_start(out=msk_t[:], in_=msk[:])

        for tcn in range(8):
            ns = min((tcn + 1) * 128, T)
            tch = min(128, T - tcn * 128)
            sc = big.tile([128, 1024], F32, tag="sc")
            for s0 in range(0, ns, 512):
                nn_ = min(512, ns - s0)
                ps = psum.tile([128, 512], F32, tag="psc")
                for kc in range(3):
                    nc.tensor.matmul(
                        ps[:tch, :nn_],
                        qt_t[:, kc, tcn * 128:tcn * 128 + tch],
                        kt_t[:, kc, s0:s0 + nn_],
                        start=(kc == 0), stop=(kc == 2))
                nc.vector.tensor_copy(out=sc[:tch, s0:s0 + nn_],
                                      in_=ps[:tch, :nn_])
            dw = ns - tcn * 128
            nc.vector.tensor_tensor(out=sc[:tch, tcn * 128:ns],
                                    in0=sc[:tch, tcn * 128:ns],
                                    in1=msk_t[:tch, :dw], op=OP.add)
            mx = work.tile([128, 1], F32, tag="mx")
            nc.vector.tensor_reduce(out=mx[:tch], in_=sc[:tch, :ns], axis=AX.X,
                                    op=OP.max)
            nc.vector.tensor_scalar_mul(out=mx[:tch], in0=mx[:tch],
                                        scalar1=-1.0)
            sme = work.tile([128, 1], F32, tag="sme")
            nc.scalar.activation(out=sc[:tch, :ns], in_=sc[:tch, :ns],
                                 func=AF.Exp, bias=mx[:tch],
                                 accum_out=sme[:tch])
            nc.vector.reciprocal(out=sme[:tch], in_=sme[:tch])
            av = psumB.tile([128, 3, 512], F32, tag="pav")
            for sb0 in range(0, tcn + 1, 4):
                sbn = min(4, tcn + 1 - sb0)
                pT = psum.tile([128, 4, 128], F32, tag="ptr")
                for j in range(sbn):
                    sb = sb0 + j
                    scb = min(128, ns - sb * 128)
                    nc.tensor.transpose(pT[:scb, j, :tch],
                                        sc[:tch, sb * 128:sb * 128 + scb],
                                        ident[:tch, :tch])
                aT = work.tile([128, 4, 128], BF16, tag="aT")
                nc.scalar.copy(out=aT[:, 0:sbn, :].rearrange("p a b -> p (a b)"),
                               in_=pT[:, 0:sbn, :].rearrange("p a b -> p (a b)"))
                for j in range(sbn):
                    sb = sb0 + j
                    scb = min(128, ns - sb * 128)
                    for n3 in range(3):
                        nn_ = min(512, DFv - n3 * 512)
                        nc.tensor.matmul(
                            av[:tch, n3, :nn_], aT[:scb, j, :tch],
                            vm_t[:scb, sb, n3 * 512:n3 * 512 + nn_],
                            start=(sb == 0), stop=(sb == tcn))
            avs = big.tile([128, DFv], BF16, tag="avs")
            av2 = bass.AP(tensor=av.tensor, offset=av.offset,
                          ap=[av.ap[0], [1, DFv]])
            nc.vector.tensor_scalar_mul(out=avs[:tch], in0=av2[:tch],
                                        scalar1=sme[:tch])
            nc.sync.dma_start(out=avo[:, tcn, :], in_=avs[:])
    nc.compile()
    return nc


# ---------------- Launch 3c: proj + out-LN + residual ----------------

def build_l3c():
    nc = new_nc()
    avf = nc.dram_tensor("avf", [64, TSH, Qp], BF16, kind="ExternalInput")
    icm = nc.dram_tensor("icm", [64, TSH, Qp], F32, kind="ExternalInput")
    pw = nc.dram_tensor("pw", [64, 64], BF16, kind="ExternalInput")
    pb = nc.dram_tensor("pb", [64, 3], F32, kind="ExternalInput")
    outo = nc.dram_tensor("outo", [64, TSH, Q], F32, kind="ExternalOutput")
    NTF = TSH * Qp
    ctx = contextlib.ExitStack()
    with tile.TileContext(nc) as tc, ctx:
        const = ctx.enter_context(tc.tile_pool(name="const", bufs=1))
        big = ctx.enter_context(tc.tile_pool(name="big", bufs=1))
        work = ctx.enter_context(tc.tile_pool(name="work", bufs=1))
        psum = ctx.enter_context(tc.tile_pool(name="psum", bufs=2, space="PSUM"))
        eps_t = const.tile([128, 1], F32)
        nc.vector.memset(eps_t[:], EPS)
        ones_t = const.tile([64, 128], BF16)
        nc.vector.memset(ones_t[:], 1.0)
        avt = big.tile([64, NTF], BF16, tag="avt")
        nc.sync.dma_start(out=avt[:], in_=avf.rearrange("c t f -> c (t f)"))
        pwt = const.tile([64, 64], BF16)
        nc.sync.dma_start(out=pwt[:], in_=pw[:])
        pbt = const.tile([64, 3], F32)
        nc.sync.dma_start(out=pbt[:], in_=pb[:])

        P = big.tile([64, NTF], F32, tag="P")
        for n0 in range(0, NTF, 512):
            nn_ = min(512, NTF - n0)
            ps = psum.tile([64, 512], F32, tag="pp")
            nc.tensor.matmul(ps[:, :nn_], pwt[:], avt[:, n0:n0 + nn_],
                             start=True, stop=True)
            nc.scalar.activation(out=P[:, n0:n0 + nn_], in_=ps[:, :nn_],
                                 func=AF.Prelu, bias=pbt[:, 0:1],
                                 alpha=pbt[:, 1:2])
        P3 = P[:].rearrange("p (t f) -> p t f", f=Qp)
        nc.vector.memset(P3[:, :, Q:Qp], 0.0)
        s1 = work.tile([64, TSH], F32, tag="s1")
        nc.vector.tensor_reduce(out=s1[:], in_=P3, axis=AX.X, op=OP.add)
        sq = big.tile([64, NTF], BF16, tag="avt")
        nc.scalar.activation(out=sq[:], in_=P[:], func=AF.Square)
        s2 = work.tile([64, TSH], F32, tag="s2")
        nc.vector.tensor_reduce(out=s2[:], in_=sq[:].rearrange(
            "p (t f) -> p t f", f=Qp), axis=AX.X, op=OP.add)
        s1b = work.tile([64, TSH], BF16, tag="s1b")
        nc.vector.tensor_copy(out=s1b[:], in_=s1[:])
        s2b = work.tile([64, TSH], BF16, tag="s2b")
        nc.vector.tensor_copy(out=s2b[:], in_=s2[:])
        NCF = 64 * Q  # 4160
        mu = work.tile([128, TSH], F32, tag="mu")
        psg = psum.tile([128, TSH], F32, tag="pg")
        nc.tensor.matmul(psg[:], ones_t[:], s1b[:], start=True, stop=True)
        nc.vector.tensor_scalar_mul(out=mu[:], in0=psg[:], scalar1=1.0 / NCF)
        var = work.tile([128, TSH], F32, tag="var")
        psg2 = psum.tile([128, TSH], F32, tag="pg2")
        nc.tensor.matmul(psg2[:], ones_t[:], s2b[:], start=True, stop=True)
        nc.vector.tensor_scalar_mul(out=var[:], in0=psg2[:], scalar1=1.0 / NCF)
        mu2 = work.tile([128, TSH], F32, tag="mu2")
        nc.vector.tensor_tensor(out=mu2[:], in0=mu[:], in1=mu[:], op=OP.mult)
        nc.vector.tensor_tensor(out=var[:], in0=var[:], in1=mu2[:],
                                op=OP.subtract)
        rs = work.tile([128, TSH], F32, tag="rs")
        nc.scalar.activation(out=rs[:], in_=var[:], func=AF.Sqrt, bias=eps_t[:])
        nc.vector.reciprocal(out=rs[:], in_=rs[:])
        # out = (P - mu)*rs + inter
        o1 = big.tile([64, TSH, Qp], F32, tag="o1")
        nc.vector.tensor_tensor(out=o1[:], in0=P3, in1=bap(mu[0:64, :], [Qp]),
                                op=OP.subtract)
        nc.vector.tensor_tensor(out=o1[:], in0=o1[:], in1=bap(rs[0:64, :], [Qp]),
                                op=OP.mult)
        ict = big.tile([64, NTF], F32, tag="P")
        nc.sync.dma_start(out=ict[:], in_=icm.rearrange("c t f -> c (t f)"))
        nc.vector.tensor_tensor(out=o1[:], in0=o1[:],
                                in1=ict[:].rearrange("p (t f) -> p t f", f=Qp),
                                op=OP.add)
        nc.sync.dma_start(out=outo[:], in_=o1[:, :, :Q])
    nc.compile()
    return nc


# ======================= host weight prep =======================

def _uniform(a):
    a = np.asarray(a)
    assert np.all(a == a.flat[0]), "nonuniform LN affine not supported"
    return float(a.flat[0])


def _prep_lstm_v2(wih, whh, bih, bhh, gamma, beta):
    """LN-folded, gate-reordered (i,f,o,g) weight arrays."""
    g = np.asarray(gamma, np.float64).reshape(-1)
    b = np.asarray(beta, np.float64).reshape(-1)
    NH4 = np.asarray(wih).shape[0]
    w4 = np.asarray(wih, np.float64).reshape(NH4, C, KS)
    wih_eff = w4 * g[None, :, None]
    bih_eff = (np.asarray(bih, np.float64) + np.asarray(bhh, np.float64)
               + (w4 * b[None, :, None]).sum((1, 2)))
    H = NH4 // 4
    perm = np.r_[0:H, H:2 * H, 3 * H:4 * H, 2 * H:3 * H]
    return wih_eff[perm], bih_eff[perm], np.asarray(whh, np.float64)[perm]


def _build_weight_arrays(ii):
    """All per-core weight arrays as float32 numpy (pre-cast layouts)."""
    w = {}
    # ---- L1 (intra BiLSTM) ----
    wts, whs = [], []
    for d in range(2):
        we, be, wp = _prep_lstm_v2(
            ii["intra_wih"][d], ii["intra_whh"][d], ii["intra_bih"][d],
            ii["intra_bhh"][d], ii["intra_gamma"], ii["intra_beta"])
        wt = np.zeros((65, 4, 4, 128), np.float32)
        wh = np.zeros((128, 4, 128), np.float32)
        for m in range(4):
            for k in range(4):
                wt[:64, m, k] = we[m * 128:(m + 1) * 128, :, k].T
            wt[64, m, 0] = be[m * 128:(m + 1) * 128]
            wh[:, m] = wp[m * 128:(m + 1) * 128].T
        wts.append(wt); whs.append(wh)
    ctw_i = np.asarray(ii["intra_ct_w"], np.float64)
    ct_d = np.zeros((2, 2, 128, 128), np.float32)
    for d in range(2):
        sub = ctw_i[d * 128:(d + 1) * 128]
        for mo in range(2):
            for kp in range(2):
                for cc in range(64):
                    ct_d[d, mo, :, kp * 64 + cc] = sub[:, cc, mo * 2 + kp]
    ctb1 = np.zeros((128, 2), np.float32)
    for mo in range(2):
        for kp in range(2):
            ctb1[kp * 64:(kp + 1) * 64, mo] = np.asarray(ii["intra_ct_b"])
    w["l1"] = [
        ("wih", np.stack(wts, axis=1), BF16),
        ("whh", np.stack(whs, axis=1), BF16),
        ("ctw", ct_d.reshape(2, 2, 1, 128, 128).transpose(3, 0, 1, 2, 4), BF16),
        ("ctb", ctb1, F32),
    ]
    # ---- L2 (inter LSTM) ----
    we2, be2, wp2 = _prep_lstm_v2(
        ii["inter_wih"], ii["inter_whh"], ii["inter_bih"], ii["inter_bhh"],
        ii["inter_gamma"], ii["inter_beta"])
    wih2 = np.zeros((128, 8, 2, 128), np.float32)
    whh2 = np.zeros((128, 16, 128), np.float32)
    bih2 = np.zeros((128, 8), np.float32)
    for m in range(8):
        rows = we2[m * 128:(m + 1) * 128]
        for kp in range(2):
            wih2[:64, m, kp] = rows[:, :, 2 * kp].T
            wih2[64:, m, kp] = rows[:, :, 2 * kp + 1].T
        for kc in range(2):
            whh2[:, m * 2 + kc] = wp2[m * 128:(m + 1) * 128,
                                      kc * 128:(kc + 1) * 128].T
        bih2[:, m] = be2[m * 128:(m + 1) * 128]
    ctw2 = np.asarray(ii["inter_ct_w"], np.float64)
    ct2 = np.zeros((2, 256, 128), np.float32)
    for mo in range(2):
        for kp in range(2):
            for cc in range(64):
                j = kp * 64 + cc
                ct2[mo, :, j] = ctw2[:, cc, mo * 2 + kp]
    ctb2 = np.zeros((128, 2), np.float32)
    for mo in range(2):
        for kp in range(2):
            ctb2[kp * 64:(kp + 1) * 64, mo] = np.asarray(ii["inter_ct_b"])
    w["l2"] = [
        ("wih", wih2, BF16),
        ("whh", whh2, BF16),
        ("bih", bih2, F32),
        ("ctw", ct2.reshape(2, 2, 128, 128).transpose(2, 0, 1, 3)
         .reshape(128, 1, 2, 2, 128), BF16),
        ("ctb", ctb2, F32),
    ]
    # ---- L3a ----
    qg = _uniform(ii["q_g"]); kg = _uniform(ii["k_g"]); vg = _uniform(ii["v_g"])
    assert _uniform(ii["q_bt"]) == 0 and _uniform(ii["k_bt"]) == 0
    assert _uniform(ii["v_bt"]) == 0
    wall = np.zeros((64, 96), np.float32)
    bias96 = np.zeros((96,), np.float32)
    alpha96 = np.zeros((96,), np.float32)
    cnt96 = np.zeros((96,), np.float32)
    gs96 = np.zeros((96,), np.float32)
    grp = np.zeros((96,), np.int32)
    for h in range(NH):
        wall[:, h * 4:h * 4 + 4] = np.asarray(ii["q_w"][h]).T
        wall[:, 16 + h * 4:16 + h * 4 + 4] = np.asarray(ii["k_w"][h]).T
        wall[:, 32 + h * 16:32 + h * 16 + 16] = np.asarray(ii["v_w"][h]).T
        bias96[h * 4:h * 4 + 4] = np.asarray(ii["q_b"][h])
        bias96[16 + h * 4:16 + h * 4 + 4] = np.asarray(ii["k_b"][h])
        alpha96[h * 4:h * 4 + 4] = float(ii["q_p"][h])
        alpha96[16 + h * 4:16 + h * 4 + 4] = float(ii["k_p"][h])
        alpha96[32 + h * 16:32 + h * 16 + 16] = float(ii["v_p"][h])
        cnt96[h * 4:h * 4 + 4] = 1.0 / (E * Q)
        cnt96[16 + h * 4:16 + h * 4 + 4] = 1.0 / (E * Q)
        cnt96[32 + h * 16:32 + h * 16 + 16] = 1.0 / (Dv * Q)
        gs96[h * 4:h * 4 + 4] = qg / np.sqrt(E * Q)
        gs96[16 + h * 4:16 + h * 4 + 4] = kg
        gs96[32 + h * 16:32 + h * 16 + 16] = vg
        grp[h * 4:h * 4 + 4] = h
        grp[16 + h * 4:16 + h * 4 + 4] = 4 + h
        grp[32 + h * 16:32 + h * 16 + 16] = 8 + h
    gmat = (grp[:, None] == grp[None, :]).astype(np.float32)
    bs96 = np.stack([bias96, alpha96, cnt96, gs96], axis=1)
    w["l3a"] = [("wall", wall, BF16), ("bs", bs96, F32), ("gmat", gmat, BF16)]
    # ---- L3b mask ----
    mask = np.triu(np.full((128, 128), -1e9, np.float32), 1)
    w["msk"] = mask
    # ---- L3c ----
    assert _uniform(ii["proj_g"]) == 1.0 and _uniform(ii["proj_bt"]) == 0.0
    pw = np.ascontiguousarray(np.asarray(ii["proj_w"], np.float32).T)
    pb3 = np.zeros((64, 3), np.float32)
    pb3[:, 0] = np.asarray(ii["proj_b"])
    pb3[:, 1] = float(ii["proj_p"])
    w["l3c"] = [("pw", pw, BF16), ("pb", pb3, F32)]
    return w


# ======================= glue (device jnp) =======================

BF = jnp.bfloat16


def _posmajor_j(v):
    """[8, NPOS, 64] (NPOS=17000) -> [1024, 133, 64] pos-major tiles."""
    v = jnp.concatenate([v, jnp.zeros((8, G1 * 128 - NP1, 64), v.dtype)], 1)
    return v.reshape(8, G1, 128, 64).transpose(0, 2, 1, 3).reshape(
        8 * 128, G1, 64)


def glue0_fn(xg):
    """xg [8, 64, 250, 68] f16 -> (x_pm bf16 [1024,133,64],
    x_u f32 [1024,2,4250])."""
    v32 = xg.astype(jnp.float32)
    pm = _posmajor_j(v32.transpose(0, 2, 3, 1).reshape(8, NP1, 64)
                     .astype(BF))
    xu = v32.reshape(8, 64, 250, 17, 2, 2).transpose(0, 4, 5, 1, 3, 2)
    xu = xu.reshape(8, 2, 128, NT1).transpose(0, 2, 1, 3).reshape(
        1024, 2, NT1)
    return pm, xu


GROUPS = [[0, 1, 2, 3], [4, 5, 6, 7]]


def _a2a(x):
    """all-to-all within the 4-core b-group along dim0 (size 4)."""
    return jax.lax.all_to_all(x, "core", 0, 0, tiled=True,
                              axis_index_groups=GROUPS)


def _posmajor_l(rows_pc):
    """local [NPOS=17000, 64] -> [128, 133, 64] pos-major tiles."""
    v = jnp.concatenate(
        [rows_pc, jnp.zeros((G1 * 128 - NP1, 64), rows_pc.dtype)], 0)
    return v.reshape(G1, 128, 64).transpose(1, 0, 2)


def glue1_local(o1l):
    """per-core l1 outu [128,2,4250] f32 -> l2 (x_pm, x_u) local blocks.

    Core (b,tc) holds intra t-chunk; l2 wants q-chunks: all-to-all
    within the 4-core b-group."""
    ou = o1l.reshape(2, 64, 2, 17, 250)          # [kp, c, mo, l, t]
    intra = ou.transpose(1, 4, 3, 2, 0).reshape(64, 250, 68)
    s4 = intra.reshape(64, 250, 4, 17).transpose(2, 0, 1, 3)
    rcv = _a2a(s4)                               # [tc', 64, 250, 17]
    rows = rcv.transpose(1, 3, 0, 2).reshape(64, 17, 1000)  # [c, r, t]
    pm2 = _posmajor_l(rows.transpose(1, 2, 0).reshape(NP1, 64).astype(BF))
    xu2 = rows.reshape(64, 17, 250, 2, 2).transpose(3, 4, 0, 2, 1)
    xu2 = xu2.reshape(2, 128, NT2).transpose(1, 0, 2)
    return pm2, xu2


def glue2_local(o2l):
    """per-core l2 outu [128,2,4250] f32 -> (icm bf16, icm f32) local."""
    ou = o2l.reshape(2, 64, 2, 250, 17)          # [kp, c, mo, l, r]
    rows = ou.transpose(1, 4, 3, 2, 0).reshape(64, 17, 1000)  # [c, r, t]
    s4 = rows.reshape(64, 17, 4, 250).transpose(2, 0, 1, 3)
    rcv = _a2a(s4)                               # [qc', 64, 17, 250]
    tloc = rcv.transpose(1, 3, 0, 2).reshape(64, 250, 68)     # [c, t', q]
    tloc = jnp.concatenate(
        [tloc[:, :, :Q], jnp.zeros((64, 250, Qp - Q), tloc.dtype)], 2)
    return tloc.astype(BF), tloc


def glue3_local(qkvl):
    """per-core l3a qkvo [96,250,68] bf16 -> (qT, kT, vm) local."""
    qs = qkvl[0:16].reshape(4, 4, 250, 68)
    ks_ = qkvl[16:32].reshape(4, 4, 250, 68)
    vs = qkvl[32:96].reshape(4, 16, 250, 68)
    s4 = jnp.concatenate([qs, ks_, vs], axis=1)  # [h, 24, 250, 68]
    rcv = _a2a(s4)                               # [tc', 24, 250, 68]
    qkvh = rcv.transpose(1, 0, 2, 3).reshape(24, 1000, 68)

    def fm(a):                                   # [4, 1000, 68] -> ef-major
        t = a.transpose(0, 2, 1).reshape(272, 1000)
        t = jnp.concatenate([t, jnp.zeros((112, 1000), a.dtype)], 0)
        return t.reshape(3, 128, 1000).transpose(1, 0, 2)

    qT = fm(qkvh[0:4])
    kT = fm(qkvh[4:8])
    vmm = qkvh[8:24].transpose(1, 0, 2).reshape(1000, Dv * Qp)
    vmm = jnp.concatenate(
        [vmm, jnp.zeros((24, Dv * Qp), vmm.dtype)], 0)
    vmm = vmm.reshape(8, 128, Dv * Qp).transpose(1, 0, 2)
    return qT, kT, vmm


def glue4_local(avol):
    """per-core l3b avo [128,8,1088] bf16 -> avf [64,250,68] local."""
    a = avol.transpose(1, 0, 2).reshape(1024, Dv, Qp)[:1000]
    s4 = a.reshape(4, 250, Dv, Qp)               # [tc, t', d, f]
    rcv = _a2a(s4)                               # [h', 250, 16, 68]
    return rcv.transpose(0, 2, 1, 3).reshape(64, 250, 68)


def glue5_fn(outo):
    return outo.astype(jnp.float16)


# ---- two-pass (per-b) pipelined variants ----
PAIRS = [[0, 4], [1, 5], [2, 6], [3, 7]]


def glue0p_local(xl):
    """pass-mode input glue. xl local [32, 250, 65] f16: core r holds
    channel-half r//4 of t-chunk r%4 of this pass's b. Pair all-gather
    rebuilds the full [64, 250, 65] chunk on both group members, then
    the usual pos-major + unfold transforms."""
    full = jax.lax.all_gather(xl, "core", axis_index_groups=PAIRS)
    v32 = full.reshape(64, 250, Q).astype(jnp.float32)
    v32 = jnp.concatenate(
        [v32, jnp.zeros((64, 250, Qp - Q), jnp.float32)], 2)
    pm = _posmajor_l(v32.transpose(1, 2, 0).reshape(NP1, 64).astype(BF))
    xu = v32.reshape(64, 250, 17, 2, 2).transpose(3, 4, 0, 2, 1)
    xu = xu.reshape(2, 128, NT1).transpose(1, 0, 2)
    return pm, xu


def glue5p_local(outol):
    """pass-mode output glue. outol local [64, 250, 65] f32; core r
    returns channel-half r//4 so the global D2H is half-sized."""
    idx = jax.lax.axis_index("core")
    m = (idx >= 4).astype(jnp.float16)
    lo = outol[0:32].astype(jnp.float16)
    hi = outol[32:64].astype(jnp.float16)
    return lo * (1 - m) + hi * m


def zeros_fn():
    return (jnp.zeros((1024, 2, NT1), jnp.float32),
            jnp.zeros((1024, 2, NT2), jnp.float32),
            jnp.zeros((768, 250, 68), BF),
            jnp.zeros((1024, 8, Dv * Qp), BF),
            jnp.zeros((512, 250, Q), jnp.float32))


# ======================= stage runner =======================

class _StageRunner:
    """jit(shard_map(bass_exec)) built once per stage, reused every call."""

    def __init__(self, nc, mesh, n_cores=NCORES):
        install_neuronx_cc_hook()
        self.nc = nc
        partition_name = (nc.partition_id_tensor.name
                          if nc.partition_id_tensor else None)
        dbg_name = nc.dbg_addr.name if nc.dbg_addr is not None else None
        assert not nc.dbg_callbacks
        in_names, out_names, out_avals = [], [], []
        for alloc in nc.m.functions[0].allocations:
            if not isinstance(alloc, mybir.MemoryLocationSet):
                continue
            name = alloc.memorylocations[0].name
            if alloc.kind == "ExternalInput":
                if name != partition_name:
                    in_names.append(name)
            elif alloc.kind == "ExternalOutput":
                out_names.append(name)
                out_avals.append(jax.core.ShapedArray(
                    tuple(alloc.tensor_shape), mybir.dt.np(alloc.dtype)))
        assert dbg_name is None or dbg_name in in_names
        self.in_names, self.out_names = in_names, out_names
        n_params, n_outs = len(in_names), len(out_names)
        all_names = list(in_names) + list(out_names)
        if partition_name is not None:
            all_names.append(partition_name)

        def _body(*args):
            operands = list(args)
            if partition_name is not None:
                operands.append(partition_id_tensor())
            outs = _bass_exec_p.bind(
                *operands,
                out_avals=tuple(out_avals),
                in_names=tuple(all_names),
                out_names=tuple(out_names),
                lowering_input_output_aliases=(),
                sim_require_finite=True,
                sim_require_nnan=True,
                nc=nc,
            )
            return tuple(outs)

        in_specs = (PartitionSpec("core"),) * (n_params + n_outs)
        out_specs = (PartitionSpec("core"),) * n_outs
        donate = tuple(range(n_params, n_params + n_outs))
        self.fn = jax.jit(
            shard_map(_body, mesh=mesh, in_specs=in_specs,
                      out_specs=out_specs, check_rep=False),
            donate_argnums=donate, keep_unused=True)

    def __call__(self, arrs, zeros):
        return self.fn(*arrs, *zeros)


# ======================= host orchestration =======================

_C = {}


def _wfingerprint(ii):
    keys = ["intra_wih", "intra_whh", "inter_wih", "inter_whh", "q_w",
            "k_w", "v_w", "proj_w", "intra_ct_w", "inter_ct_w"]
    return tuple(float(np.asarray(ii[k]).sum()) for k in keys)


def _ensure(ii):
    if "mesh" not in _C:
        devs = jax.devices()[:NCORES]
        _C["mesh"] = Mesh(np.asarray(devs), ("core",))
        _C["shard"] = NamedSharding(_C["mesh"], PartitionSpec("core"))
    mesh, shard = _C["mesh"], _C["shard"]
    if "l1" not in _C:
        _C["l1"] = _StageRunner(build_lstm_launch("intra"), mesh)
        _C["l2"] = _StageRunner(build_lstm_launch("inter"), mesh)
        _C["l3a"] = _StageRunner(build_l3a(), mesh)
        _C["l3b"] = _StageRunner(build_l3b(), mesh)
        _C["l3c"] = _StageRunner(build_l3c(), mesh)
        jt = lambda f: jax.jit(f, out_shardings=shard)
        P = PartitionSpec("core")

        def sm(f, nin, nout):
            return jax.jit(shard_map(
                f, mesh=mesh, in_specs=(P,) * nin,
                out_specs=(P,) * nout if nout > 1 else P, check_rep=False))

        _C["g0"] = jt(glue0_fn)
        _g0p_sm = shard_map(glue0p_local, mesh=mesh, in_specs=(P,),
                            out_specs=(P, P), check_rep=False)

        def _g0pz(xl):
            pm, xu = _g0p_sm(xl)
            return (pm, xu) + zeros_fn()

        _C["g0pz"] = jax.jit(_g0pz, out_shardings=shard)
        _C["g1"] = sm(glue1_local, 1, 2)
        _C["g2"] = sm(glue2_local, 1, 2)
        _C["g3"] = sm(glue3_local, 1, 3)
        _C["g4"] = sm(glue4_local, 1, 1)
        _C["g5"] = jt(glue5_fn)
        _C["g5p"] = sm(glue5p_local, 1, 1)
        _C["zeros"] = jax.jit(zeros_fn, out_shardings=shard)
    fp = _wfingerprint(ii)
    if _C.get("wfp") != fp:
        w = _build_weight_arrays(ii)
        put = lambda a: jax.device_put(
            np.ascontiguousarray(np.tile(np.asarray(a, np.float32),
                                         (NCORES,) + (1,) * (a.ndim - 1))),
            _C["shard"])
        devw = {}
        for stage in ["l1", "l2", "l3a", "l3c"]:
            names, arrs, dts = zip(*w[stage])
            f32d = [put(a) for a in arrs]
            castfn = jax.jit(
                lambda *xs, dts=dts: tuple(
                    x.astype(jnp.bfloat16) if dt == BF16 else x
                    for x, dt in zip(xs, dts)),
                out_shardings=_C["shard"])
            casted = castfn(*f32d)
            devw[stage] = dict(zip(names, casted))
        devw["msk"] = jax.device_put(
            np.tile(w["msk"], (NCORES, 1)), _C["shard"])
        _C["w"] = devw
        _C["wfp"] = fp


def _stage_inputs(runner, data, weights):
    m = dict(data)
    m.update(weights)
    return [m[n] for n in runner.in_names]


TWO_PASS = os.environ.get("K_TWO_PASS", "1") == "1"


def _run_stages(pm1, xu1, zs):
    w = _C["w"]
    z1, z2, z3a, z3b, z3c = zs
    (o1,) = _C["l1"](_stage_inputs(_C["l1"], {"x_pm": pm1, "x_u": xu1},
                                   w["l1"]), [z1])
    pm2, xu2 = _C["g1"](o1)
    (o2,) = _C["l2"](_stage_inputs(_C["l2"], {"x_pm": pm2, "x_u": xu2},
                                   w["l2"]), [z2])
    icm_bf, icm_f = _C["g2"](o2)
    (qkv,) = _C["l3a"](_stage_inputs(_C["l3a"], {"icm": icm_bf}, w["l3a"]),
                       [z3a])
    qT, kT, vm = _C["g3"](qkv)
    (avo,) = _C["l3b"](_stage_inputs(
        _C["l3b"], {"qT": qT, "kT": kT, "vm": vm, "msk": w["msk"]}, {}),
        [z3b])
    avf = _C["g4"](avo)
    (outo,) = _C["l3c"](_stage_inputs(
        _C["l3c"], {"avf": avf, "icm": icm_f}, w["l3c"]), [z3c])
    return outo


def kernel(**inputs):
    ii = {k: np.asarray(v) for k, v in inputs.items()}
    _ensure(ii)
    x = np.asarray(ii["x"], np.float32)

    if not TWO_PASS:
        xg = np.zeros((2, 4, 64, 250, 68), np.float16)
        xg[..., :Q] = x.reshape(2, 64, 4, 250, Q).transpose(0, 2, 1, 3, 4)
        xd = jax.device_put(xg.reshape(8, 64, 250, 68), _C["shard"])
        zs = _C["zeros"]()
        pm1, xu1 = _C["g0"](xd)
        of = _C["g5"](_run_stages(pm1, xu1, zs))
        oh = np.asarray(of)     # [512, 250, 65] f16
        out = oh.reshape(2, 4, 64, 250, Q).transpose(0, 2, 1, 3, 4)
        return np.ascontiguousarray(out.reshape(2, 64, 1000, Q)
                                    .astype(np.float32))

    # two-pass pipelined: pass b uploads half the bytes (channel-halves
    # across core pairs), both 4-core groups compute that b, pass 0's
    # download overlaps pass 1's upload (the tunnel is full duplex).
    import threading
    out = np.empty((2, 64, 1000, Q), np.float32)

    def pack(b):
        g = x[b].reshape(2, 32, 4, 250, Q).transpose(0, 2, 1, 3, 4)
        return np.ascontiguousarray(g, dtype=np.float16).reshape(
            8, 32, 250, Q)

    def run_pass(xg):
        xd = jax.device_put(xg, _C["shard"])
        pm1, xu1, *zs = _C["g0pz"](xd)
        return _C["g5p"](_run_stages(pm1, xu1, zs))

    def drain(b, of):
        oh = np.asarray(of)     # [256, 250, 65] f16, rows (half, chunk)
        o = oh.reshape(2, 4, 32, 250, Q).transpose(0, 2, 1, 3, 4)
        out[b] = o.reshape(64, 1000, Q).astype(np.float32)

    xg0 = pack(0)
    xg1 = pack(1)
    of0 = run_pass(xg0)
    th = threading.Thread(target=drain, args=(0, of0))
    th.start()
    of1 = run_pass(xg1)
    drain(1, of1)
    th.join()
    return out


# revision 13
# speedup vs baseline: 17.3157x; 1.0381x over previous
"""GridNetBlock (TF-GridNet) Trainium2 kernel: 8-core SPMD.

v2: fully device-resident pipeline. The five Bass stage kernels from v1
are unchanged, but all inter-stage glue (unfold/reshard/transpose) now
runs on-device as jitted jnp ops, weights are uploaded once and cached,
and host<->device traffic per call is only x (f16 in) + output (f16
out). The axon tunnel moves ~33MB/s, so this is what dominates wall.
"""
import sys, os, contextlib
for _p in ("/opt/trn_rl_repo", "/root/.axon_site/_ro/trn_rl_repo"):
    if os.path.isdir(_p) and _p not in sys.path:
        sys.path.insert(0, _p)
import numpy as np
import jax
import jax.numpy as jnp
from jax.sharding import Mesh, PartitionSpec, NamedSharding
from jax.experimental.shard_map import shard_map
import concourse.bass as bass
import concourse.bacc as bacc
import concourse.tile as tile
from concourse import mybir
from concourse.masks import make_identity
from concourse.bass2jax import (_bass_exec_p, install_neuronx_cc_hook,
                                partition_id_tensor)

F32 = mybir.dt.float32
BF16 = mybir.dt.bfloat16
AF = mybir.ActivationFunctionType
OP = mybir.AluOpType
AX = mybir.AxisListType

B, C, T, Q = 2, 64, 1000, 65
KS = 4
Qp, L1, Hh, HID, L2 = 68, 17, 128, 256, 250
NH, E, Dv = 4, 4, 16
EPS = 1e-5
NCORES = 8
TSH = T // 4
NP1 = TSH * Qp
G1 = (NP1 + 127) // 128   # 133
RW2 = (B * Qp) // NCORES  # 17
NT1 = L1 * TSH            # 4250
NT2 = L2 * RW2            # 4250


def bap(t, tail):
    ap = list(t.ap)
    for n in tail:
        ap.append([0, n])
    return bass.AP(tensor=t.tensor, offset=t.offset, ap=ap)


def new_nc():
    return bacc.Bacc("TRN2", target_bir_lowering=False, debug=False,
                     enable_asserts=True, num_devices=NCORES)


def ln_posmajor(nc, pool, work, xpm, G, nred, eps_t):
    s1 = work.tile([128, G], F32, tag="lns1")
    nc.vector.tensor_reduce(out=s1[:], in_=xpm[:], axis=AX.X, op=OP.add)
    xsq = pool.tile([128, G, nred], BF16, tag="xut")
    nc.scalar.activation(out=xsq[:], in_=xpm[:], func=AF.Square)
    s2 = work.tile([128, G], F32, tag="lns2")
    nc.vector.tensor_reduce(out=s2[:], in_=xsq[:], axis=AX.X, op=OP.add)
    mu = work.tile([128, G], F32, tag="lnmu")
    nc.vector.tensor_scalar_mul(out=mu[:], in0=s1[:], scalar1=1.0 / nred)
    var = work.tile([128, G], F32, tag="lnvar")
    nc.vector.tensor_tensor(out=var[:], in0=mu[:], in1=mu[:], op=OP.mult)
    nc.vector.scalar_tensor_tensor(out=var[:], in0=s2[:], scalar=1.0 / nred,
                                   in1=var[:], op0=OP.mult, op1=OP.subtract)
    rs = work.tile([128, G], F32, tag="lnrs")
    nc.scalar.activation(out=rs[:], in_=var[:], func=AF.Sqrt, bias=eps_t[:])
    nc.vector.reciprocal(out=rs[:], in_=rs[:])
    zpm = pool.tile([128, G, nred], BF16, tag="xut")
    nc.vector.tensor_tensor(out=zpm[:], in0=xpm[:], in1=bap(mu, [nred]),
                            op=OP.subtract)
    nc.vector.tensor_tensor(out=zpm[:], in0=zpm[:], in1=bap(rs, [nred]),
                            op=OP.mult)
    return zpm


def ap3(t, off, d1, n1, d2, n2):
    """Strided 2-free-dim AP view of tile t at element offset off."""
    return bass.AP(tensor=t.tensor, offset=t.offset + off,
                   ap=[t.ap[0], [d1, n1], [d2, n2]])


def build_lstm_launch(which):
    """which: 'intra' or 'inter'. Returns compiled nc."""
    intra = which == "intra"
    ND = 2 if intra else 1
    MC = 4 if intra else 8
    KC = 1 if intra else 2
    L = L1 if intra else L2
    NB = TSH if intra else RW2        # lstm batch per core
    NT = L * NB                       # 4250
    G = G1
    ZC = G * 128

    nc = new_nc()
    x_pm = nc.dram_tensor("x_pm", [128, G, C], BF16, kind="ExternalInput")
    x_u = nc.dram_tensor("x_u", [128, 2, NT], F32, kind="ExternalInput")
    if intra:
        wih = nc.dram_tensor("wih", [65, 2, 4, 4, 128], BF16,
                             kind="ExternalInput")
        whh = nc.dram_tensor("whh", [128, 2, 4, 128], BF16,
                             kind="ExternalInput")
    else:
        wih = nc.dram_tensor("wih", [128, 8, 2, 128], BF16,
                             kind="ExternalInput")
        whh = nc.dram_tensor("whh", [128, 16, 128], BF16,
                             kind="ExternalInput")
        bih = nc.dram_tensor("bih", [128, 8], F32, kind="ExternalInput")
    ctw = nc.dram_tensor("ctw", [128, ND, 2, KC, 128], BF16,
                         kind="ExternalInput")
    ctb = nc.dram_tensor("ctb", [128, 2], F32, kind="ExternalInput")
    outu = nc.dram_tensor("outu", [128, 2, NT], F32, kind="ExternalOutput")

    ctx = contextlib.ExitStack()
    with tile.TileContext(nc) as tc, ctx:
        const = ctx.enter_context(tc.tile_pool(name="const", bufs=1))
        persist = ctx.enter_context(tc.tile_pool(name="persist", bufs=1))
        psum = ctx.enter_context(tc.tile_pool(name="psum", bufs=2,
                                              space="PSUM"))
        psumB = ctx.enter_context(tc.tile_pool(name="psumB", bufs=2,
                                               space="PSUM"))
        psumS = ctx.enter_context(tc.tile_pool(
            name="psumS", bufs=1 if intra else 2, space="PSUM"))

        eps_t = const.tile([128, 1], F32)
        nc.vector.memset(eps_t[:], EPS)
        ident = const.tile([128, 128], BF16)
        make_identity(nc, ident[:])

        if intra:
            wih_t = const.tile([65, 2, 4, 4, 128], BF16)
            whh_t = const.tile([128, 2, 4, 128], BF16)
        else:
            wih_t = const.tile([128, 8, 2, 128], BF16)
            whh_t = const.tile([128, 16, 128], BF16)
            bih_t = const.tile([128, 8], F32)
            nc.sync.dma_start(out=bih_t[:], in_=bih[:])
        nc.sync.dma_start(out=wih_t[:], in_=wih[:])
        nc.sync.dma_start(out=whh_t[:], in_=whh[:])
        ct_tl = const.tile([128, ND, 2, KC, 128], BF16)
        nc.sync.dma_start(out=ct_tl[:], in_=ctw[:])
        ctb_t = const.tile([128, 2], F32)
        nc.sync.dma_start(out=ctb_t[:], in_=ctb[:])

        # --- persistent tiles ---
        if intra:
            # z channel-major [65, ZC]; row 64 = ones (bias row)
            z_cm = persist.tile([65, ZC], BF16)
            hbufs = [persist.tile([128, L, NB], BF16, name=f"hb{d}")
                     for d in range(ND * KC)]
        else:
            # z doubled rows: p<64: z[c, t'-3]; p>=64: z[c, t'-2]
            z2d = persist.tile([128, RW2, 1000], BF16)
            hb2 = persist.tile([128, 2, L, NB], BF16)
        ou = persist.tile([128, 2, L, NB], F32)

        # --- LN over C (pos-major) + transpose to channel-major ---
        with tc.tile_pool(name="tmpA", bufs=1) as tmpA:
            xpm = tmpA.tile([128, G, C], BF16)
            nc.sync.dma_start(out=xpm[:], in_=x_pm[:])
            work = tmpA
            zpm = ln_posmajor(nc, tmpA, work, xpm, G, C, eps_t)
            if intra:
                z_dst = z_cm
                nc.gpsimd.memset(z_cm[64:65, :], 1.0)
            else:
                z_tmp = tmpA.tile([64, ZC], BF16)
                z_dst = z_tmp
            for g0 in range(0, G, 4):
                gn = min(4, G - g0)
                pt = psum.tile([64, 4, 128], BF16, tag="tps")
                for gg in range(gn):
                    nc.tensor.transpose(pt[:, gg, :], zpm[:, g0 + gg, :],
                                        ident[:])
                if (g0 // 4) % 2 == 0:
                    nc.scalar.copy(
                        out=z_dst[0:64, g0 * 128:(g0 + gn) * 128],
                        in_=pt[:, 0:gn, :].rearrange("p a b -> p (a b)"))
                else:
                    nc.vector.tensor_copy(
                        out=z_dst[0:64, g0 * 128:(g0 + gn) * 128],
                        in_=pt[:, 0:gn, :].rearrange("p a b -> p (a b)"))
            if not intra:
                # z_tmp [64, row*1000+t] -> z2d two shifted copies
                nc.vector.memset(z2d[0:64, :, 0:3], 0.0)
                nc.vector.memset(z2d[64:128, :, 0:2], 0.0)
                nc.sync.dma_start(
                    out=z2d[0:64, :, 3:1000],
                    in_=ap3(z_tmp, 0, 1000, RW2, 1, 997))
                nc.sync.dma_start(
                    out=z2d[64:128, :, 2:1000],
                    in_=ap3(z_tmp, 0, 1000, RW2, 1, 998))

        # --- gate precompute (inter only): pre2 [128, L, 8, RW2] bf16 ---
        rec = ctx.enter_context(tc.tile_pool(name="rec", bufs=1))
        if not intra:
            pre2 = rec.tile([128, L, 8, RW2], BF16)
            LSUB = 30
            for l0 in range(0, L, LSUB):
                ln_ = min(LSUB, L - l0)
                for m in range(8):
                    pp = psumB.tile([128, 512], F32, tag="ppre")
                    for kp in range(2):
                        rhs = ap3(z2d, 4 * l0 + 2 * kp, 4, ln_, 1000, RW2)
                        nc.tensor.matmul(pp[:, :RW2 * ln_],
                                         wih_t[:, m, kp, :], rhs,
                                         start=(kp == 0), stop=(kp == 1))
                    dst = pre2[:, l0:l0 + ln_, m, :]
                    src = pp[:, :RW2 * ln_].rearrange(
                        "p (l r) -> p l r", r=RW2)
                    if m % 2 == 0:
                        nc.vector.tensor_scalar_add(out=dst, in0=src,
                                                    scalar1=bih_t[:, m:m + 1])
                    else:
                        nc.scalar.activation(out=dst, in_=src,
                                             func=AF.Identity,
                                             bias=bih_t[:, m:m + 1])

        # --- recurrence (interleaved directions) ---
        NGC = MC // 4                      # hidden chunks (1 or 2)
        gsb = [rec.tile([128, MC, NB], BF16, name=f"gs{d}")
               for d in range(ND)]
        c_t = [rec.tile([128, NGC, NB], F32, name=f"ct{d}")
               for d in range(ND)]
        ig_t = [rec.tile([128, NGC, NB], BF16, name=f"ig{d}")
               for d in range(ND)]
        tc_t = [rec.tile([128, NGC, NB], BF16, name=f"tc{d}")
               for d in range(ND)]
        slot = 64 if NB <= 64 else 256

        def step(d, l, first):
            lp = l + 1 if (intra and d == 1) else l - 1
            ps = psumS.tile([128, MC, slot if intra else RW2], F32,
                            tag=f"lps{d}")
            if intra:
                for m in range(4):
                    st = Qp
                    for k in range(4):
                        o0 = 4 * l + k
                        rhs = bass.AP(
                            tensor=z_cm.tensor, offset=z_cm.offset + o0,
                            ap=[z_cm.ap[0], [st, NB]])
                        nc.tensor.matmul(ps[:, m, :NB],
                                         wih_t[:, d, m, k, :], rhs,
                                         start=(k == 0),
                                         stop=(k == 3 and first))
                    if not first:
                        nc.tensor.matmul(ps[:, m, :NB], whh_t[:, d, m, :],
                                         hbufs[d][:, lp, :],
                                         start=False, stop=True)
            else:
                if not first:
                    nc.tensor.matmul(
                        ps[:].rearrange("p m n -> p (m n)"), ident[:],
                        pre2[:, l, :, :].rearrange("p m n -> p (m n)"),
                        start=True, stop=False, skip_group_check=True)
                    for m in range(8):
                        for kc in range(2):
                            nc.tensor.matmul(ps[:, m, :NB],
                                             whh_t[:, m * 2 + kc, :],
                                             hb2[:, kc, lp, :],
                                             start=False, stop=(kc == 1),
                                             skip_group_check=True)
            # gates: i (NGC), f (NGC), o (NGC), g (NGC)
            gg = gsb[d]
            if first and not intra:
                sig_in = pre2[:, l, 0:3 * NGC, :]
                tanh_in = pre2[:, l, 3 * NGC:, :]
            else:
                sig_in = ps[:, 0:3 * NGC, :NB]
                tanh_in = ps[:, 3 * NGC:, :NB]
            nc.scalar.activation(out=gg[:, 0:3 * NGC, :], in_=sig_in,
                                 func=AF.Sigmoid)
            nc.scalar.activation(out=gg[:, 3 * NGC:, :], in_=tanh_in,
                                 func=AF.Tanh)
            i_g, f_g = gg[:, 0:NGC, :], gg[:, NGC:2 * NGC, :]
            o_g, g_g = gg[:, 2 * NGC:3 * NGC, :], gg[:, 3 * NGC:, :]
            if first:
                nc.vector.tensor_tensor(out=c_t[d][:], in0=i_g, in1=g_g,
                                        op=OP.mult)
            else:
                nc.vector.tensor_tensor(out=ig_t[d][:], in0=i_g, in1=g_g,
                                        op=OP.mult)
                nc.vector.tensor_tensor(out=c_t[d][:], in0=f_g, in1=c_t[d][:],
                                        op=OP.mult)
                nc.vector.tensor_tensor(out=c_t[d][:], in0=c_t[d][:],
                                        in1=ig_t[d][:], op=OP.add)
            nc.scalar.activation(out=tc_t[d][:], in_=c_t[d][:], func=AF.Tanh)
            if intra:
                nc.vector.tensor_tensor(out=hbufs[d][:, l, :], in0=o_g,
                                        in1=tc_t[d][:], op=OP.mult)
            else:
                nc.vector.tensor_tensor(out=hb2[:, :, l, :], in0=o_g,
                                        in1=tc_t[d][:], op=OP.mult)

        # --- ConvT + bias + residual, l-chunked, interleaved with steps ---
        xu_t = rec.tile([128, 2, NT], F32)
        nc.sync.dma_start(out=xu_t[:], in_=x_u[:])
        CL = 2 if intra else 30

        def convt_chunk(l0):
            ln_ = min(CL, L - l0)
            nn_ = ln_ * NB
            for mo in range(2):
                ps2 = psumB.tile([128, 512], F32, tag="pct")
                nch = 0
                for d in range(ND):
                    for k in range(KC):
                        hsl = (hbufs[d][:, l0:l0 + ln_, :] if intra
                               else hb2[:, k, l0:l0 + ln_, :])
                        nc.tensor.matmul(
                            ps2[:, :nn_], ct_tl[:, d, mo, k, :],
                            hsl.rearrange("p l t -> p (l t)"),
                            start=(nch == 0), stop=(nch == ND * KC - 1))
                        nch += 1
                nc.vector.scalar_tensor_tensor(
                    out=ou[:, mo, l0:l0 + ln_, :].rearrange(
                        "p l t -> p (l t)"),
                    in0=ps2[:, :nn_], scalar=ctb_t[:, mo:mo + 1],
                    in1=xu_t[:, mo, l0 * NB:l0 * NB + nn_],
                    op0=OP.add, op1=OP.add)

        pending = list(range(0, L, CL))
        for i in range(L):
            step(0, i, i == 0)
            if intra:
                step(1, L - 1 - i, i == 0)
            for l0 in list(pending):
                ln_ = min(CL, L - l0)
                ready = i >= l0 + ln_ - 1
                if intra:
                    ready = ready and i >= L - 1 - l0
                if ready:
                    convt_chunk(l0)
                    pending.remove(l0)
        for l0 in pending:
            convt_chunk(l0)
        nc.sync.dma_start(out=outu[:],
                          in_=ou[:].rearrange("p a l t -> p a (l t)"))
    nc.compile()
    return nc


# ---------------- Launch 3a: QKV conv + PReLU + LN ----------------

def build_l3a():
    nc = new_nc()
    icm = nc.dram_tensor("icm", [64, TSH, Qp], BF16, kind="ExternalInput")
    wall = nc.dram_tensor("wall", [64, 96], BF16, kind="ExternalInput")
    bs = nc.dram_tensor("bs", [96, 4], F32, kind="ExternalInput")
    # bs cols: bias, alpha, cnt_inv, gscale (per row)
    gmat = nc.dram_tensor("gmat", [96, 96], BF16, kind="ExternalInput")
    qkvo = nc.dram_tensor("qkvo", [96, TSH, Qp], BF16, kind="ExternalOutput")
    NTF = TSH * Qp  # 17000
    ctx = contextlib.ExitStack()
    with tile.TileContext(nc) as tc, ctx:
        const = ctx.enter_context(tc.tile_pool(name="const", bufs=1))
        big = ctx.enter_context(tc.tile_pool(name="big", bufs=1))
        work = ctx.enter_context(tc.tile_pool(name="work", bufs=2))
        psum = ctx.enter_context(tc.tile_pool(name="psum", bufs=2, space="PSUM"))
        eps_t = const.tile([96, 1], F32)
        nc.vector.memset(eps_t[:], EPS)
        ict = big.tile([64, NTF], BF16, tag="ict")
        nc.sync.dma_start(out=ict[:], in_=icm.rearrange("c t f -> c (t f)"))
        wt = const.tile([64, 96], BF16)
        nc.sync.dma_start(out=wt[:], in_=wall[:])
        bst = const.tile([96, 4], F32)
        nc.sync.dma_start(out=bst[:], in_=bs[:])
        gm = const.tile([96, 96], BF16)
        nc.sync.dma_start(out=gm[:], in_=gmat[:])

        qr = big.tile([96, NTF], F32, tag="qr")
        for n0 in range(0, NTF, 512):
            nn_ = min(512, NTF - n0)
            ps = psum.tile([96, 512], F32, tag="pc")
            nc.tensor.matmul(ps[:, :nn_], wt[:], ict[:, n0:n0 + nn_],
                             start=True, stop=True)
            nc.scalar.activation(out=qr[:, n0:n0 + nn_], in_=ps[:, :nn_],
                                 func=AF.Prelu, bias=bst[:, 0:1],
                                 alpha=bst[:, 1:2])
        # stats over (e,f) groups: reduce f, then group-collapse via gmat
        s1 = work.tile([96, TSH], F32, tag="s1")
        nc.vector.tensor_reduce(out=s1[:], in_=qr[:].rearrange(
            "p (t f) -> p t f", f=Qp), axis=AX.X, op=OP.add)
        sq = big.tile([96, NTF], BF16, tag="sq")
        nc.scalar.activation(out=sq[:], in_=qr[:], func=AF.Square)
        s2 = work.tile([96, TSH], F32, tag="s2")
        nc.vector.tensor_reduce(out=s2[:], in_=sq[:].rearrange(
            "p (t f) -> p t f", f=Qp), axis=AX.X, op=OP.add)
        s1b = work.tile([96, TSH], BF16, tag="s1b")
        nc.vector.tensor_copy(out=s1b[:], in_=s1[:])
        s2b = work.tile([96, TSH], BF16, tag="s2b")
        nc.vector.tensor_copy(out=s2b[:], in_=s2[:])
        mu = work.tile([96, TSH], F32, tag="mu")
        ps1 = psum.tile([96, TSH], F32, tag="pg1")
        nc.tensor.matmul(ps1[:], gm[:], s1b[:], start=True, stop=True)
        nc.vector.tensor_scalar_mul(out=mu[:], in0=ps1[:], scalar1=bst[:, 2:3])
        var = work.tile([96, TSH], F32, tag="var")
        ps2g = psum.tile([96, TSH], F32, tag="pg2")
        nc.tensor.matmul(ps2g[:], gm[:], s2b[:], start=True, stop=True)
        nc.vector.tensor_scalar_mul(out=var[:], in0=ps2g[:], scalar1=bst[:, 2:3])
        mu2 = work.tile([96, TSH], F32, tag="mu2")
        nc.vector.tensor_tensor(out=mu2[:], in0=mu[:], in1=mu[:], op=OP.mult)
        nc.vector.tensor_tensor(out=var[:], in0=var[:], in1=mu2[:],
                                op=OP.subtract)
        rs = work.tile([96, TSH], F32, tag="rs")
        nc.scalar.activation(out=rs[:], in_=var[:], func=AF.Sqrt, bias=eps_t[:])
        nc.vector.reciprocal(out=rs[:], in_=rs[:])
        nc.vector.tensor_scalar_mul(out=rs[:], in0=rs[:], scalar1=bst[:, 3:4])
        zh = big.tile([96, TSH, Qp], BF16, tag="zh")
        qr3 = qr[:].rearrange("p (t f) -> p t f", f=Qp)
        nc.vector.tensor_tensor(out=zh[:], in0=qr3, in1=bap(mu, [Qp]),
                                op=OP.subtract)
        nc.vector.tensor_tensor(out=zh[:], in0=zh[:], in1=bap(rs, [Qp]),
                                op=OP.mult)
        nc.vector.memset(zh[:, :, Q:Qp], 0.0)
        nc.sync.dma_start(out=qkvo[:], in_=zh[:])
    nc.compile()
    return nc


# ---------------- Launch 3b: attention per (h,b) ----------------

def build_l3b():
    nc = new_nc()
    qT = nc.dram_tensor("qT", [128, 3, T], BF16, kind="ExternalInput")
    kT = nc.dram_tensor("kT", [128, 3, T], BF16, kind="ExternalInput")
    vm = nc.dram_tensor("vm", [128, 8, Dv * Qp], BF16,
                        kind="ExternalInput")
    msk = nc.dram_tensor("msk", [128, 128], F32, kind="ExternalInput")
    avo = nc.dram_tensor("avo", [128, 8, Dv * Qp], BF16,
                         kind="ExternalOutput")
    DFv = Dv * Qp
    ctx = contextlib.ExitStack()
    with tile.TileContext(nc) as tc, ctx:
        const = ctx.enter_context(tc.tile_pool(name="const", bufs=1))
        big = ctx.enter_context(tc.tile_pool(name="big", bufs=1))
        work = ctx.enter_context(tc.tile_pool(name="work", bufs=3))
        psum = ctx.enter_context(tc.tile_pool(name="psum", bufs=2, space="PSUM"))
        psumB = ctx.enter_context(tc.tile_pool(name="psumB", bufs=1,
                                               space="PSUM"))
        ident = const.tile([128, 128], F32)
        make_identity(nc, ident[:])
        qt_t = big.tile([128, 3, T], BF16, tag="qt")
        nc.sync.dma_start(out=qt_t[:], in_=qT[:])
        kt_t = big.tile([128, 3, T], BF16, tag="kt")
        nc.sync.dma_start(out=kt_t[:], in_=kT[:])
        vm_t = big.tile([128, 8, DFv], BF16, tag="vm")
        nc.sync.dma_start(out=vm_t[:], in_=vm[:])
        msk_t = const.tile([128, 128], F32)
        nc.sync.dma_start(out=msk_t[:], in_=msk[:])

        for tcn in range(8):
            ns = min((tcn + 1) * 128, T)
            tch = min(128, T - tcn * 128)
            sc = big.tile([128, 1024], F32, tag="sc")
            for s0 in range(0, ns, 512):
                nn_ = min(512, ns - s0)
                ps = psum.tile([128, 512], F32, tag="psc")
                for kc in range(3):
                    nc.tensor.matmul(
                        ps[:tch, :nn_],
                        qt_t[:, kc, tcn * 128:tcn * 128 + tch],
                        kt_t[:, kc, s0:s0 + nn_],
                        start=(kc == 0), stop=(kc == 2))
                nc.vector.tensor_copy(out=sc[:tch, s0:s0 + nn_],
                                      in_=ps[:tch, :nn_])
            dw = ns - tcn * 128
            nc.vector.tensor_tensor(out=sc[:tch, tcn * 128:ns],
                                    in0=sc[:tch, tcn * 128:ns],
                                    in1=msk_t[:tch, :dw], op=OP.add)
            mx = work.tile([128, 1], F32, tag="mx")
            nc.vector.tensor_reduce(out=mx[:tch], in_=sc[:tch, :ns], axis=AX.X,
                                    op=OP.max)
            nc.vector.tensor_scalar_mul(out=mx[:tch], in0=mx[:tch],
                                        scalar1=-1.0)
            sme = work.tile([128, 1], F32, tag="sme")
            nc.scalar.activation(out=sc[:tch, :ns], in_=sc[:tch, :ns],
                                 func=AF.Exp, bias=mx[:tch],
                                 accum_out=sme[:tch])
            nc.vector.reciprocal(out=sme[:tch], in_=sme[:tch])
            av = psumB.tile([128, 3, 512], F32, tag="pav")
            for sb0 in range(0, tcn + 1, 4):
                sbn = min(4, tcn + 1 - sb0)
                pT = psum.tile([128, 4, 128], F32, tag="ptr")
                for j in range(sbn):
                    sb = sb0 + j
                    scb = min(128, ns - sb * 128)
                    nc.tensor.transpose(pT[:scb, j, :tch],
                                        sc[:tch, sb * 128:sb * 128 + scb],
                                        ident[:tch, :tch])
                aT = work.tile([128, 4, 128], BF16, tag="aT")
                nc.scalar.copy(out=aT[:, 0:sbn, :].rearrange("p a b -> p (a b)"),
                               in_=pT[:, 0:sbn, :].rearrange("p a b -> p (a b)"))
                for j in range(sbn):
                    sb = sb0 + j
                    scb = min(128, ns - sb * 128)
                    for n3 in range(3):
                        nn_ = min(512, DFv - n3 * 512)
                        nc.tensor.matmul(
                            av[:tch, n3, :nn_], aT[:scb, j, :tch],
                            vm_t[:scb, sb, n3 * 512:n3 * 512 + nn_],
                            start=(sb == 0), stop=(sb == tcn))
            avs = big.tile([128, DFv], BF16, tag="avs")
            av2 = bass.AP(tensor=av.tensor, offset=av.offset,
                          ap=[av.ap[0], [1, DFv]])
            nc.vector.tensor_scalar_mul(out=avs[:tch], in0=av2[:tch],
                                        scalar1=sme[:tch])
            nc.sync.dma_start(out=avo[:, tcn, :], in_=avs[:])
    nc.compile()
    return nc


# ---------------- Launch 3c: proj + out-LN + residual ----------------

def build_l3c():
    nc = new_nc()
    avf = nc.dram_tensor("avf", [64, TSH, Qp], BF16, kind="ExternalInput")
    icm = nc.dram_tensor("icm", [64, TSH, Qp], F32, kind="ExternalInput")
    pw = nc.dram_tensor("pw", [64, 64], BF16, kind="ExternalInput")
    pb = nc.dram_tensor("pb", [64, 3], F32, kind="ExternalInput")
    outo = nc.dram_tensor("outo", [64, TSH, Q], F32, kind="ExternalOutput")
    NTF = TSH * Qp
    ctx = contextlib.ExitStack()
    with tile.TileContext(nc) as tc, ctx:
        const = ctx.enter_context(tc.tile_pool(name="const", bufs=1))
        big = ctx.enter_context(tc.tile_pool(name="big", bufs=1))
        work = ctx.enter_context(tc.tile_pool(name="work", bufs=1))
        psum = ctx.enter_context(tc.tile_pool(name="psum", bufs=2, space="PSUM"))
        eps_t = const.tile([128, 1], F32)
        nc.vector.memset(eps_t[:], EPS)
        ones_t = const.tile([64, 128], BF16)
        nc.vector.memset(ones_t[:], 1.0)
        avt = big.tile([64, NTF], BF16, tag="avt")
        nc.sync.dma_start(out=avt[:], in_=avf.rearrange("c t f -> c (t f)"))
        pwt = const.tile([64, 64], BF16)
        nc.sync.dma_start(out=pwt[:], in_=pw[:])
        pbt = const.tile([64, 3], F32)
        nc.sync.dma_start(out=pbt[:], in_=pb[:])

        P = big.tile([64, NTF], F32, tag="P")
        for n0 in range(0, NTF, 512):
            nn_ = min(512, NTF - n0)
            ps = psum.tile([64, 512], F32, tag="pp")
            nc.tensor.matmul(ps[:, :nn_], pwt[:], avt[:, n0:n0 + nn_],
                             start=True, stop=True)
            nc.scalar.activation(out=P[:, n0:n0 + nn_], in_=ps[:, :nn_],
                                 func=AF.Prelu, bias=pbt[:, 0:1],
                                 alpha=pbt[:, 1:2])
        P3 = P[:].rearrange("p (t f) -> p t f", f=Qp)
        nc.vector.memset(P3[:, :, Q:Qp], 0.0)
        s1 = work.tile([64, TSH], F32, tag="s1")
        nc.vector.tensor_reduce(out=s1[:], in_=P3, axis=AX.X, op=OP.add)
        sq = big.tile([64, NTF], BF16, tag="avt")
        nc.scalar.activation(out=sq[:], in_=P[:], func=AF.Square)
        s2 = work.tile([64, TSH], F32, tag="s2")
        nc.vector.tensor_reduce(out=s2[:], in_=sq[:].rearrange(
            "p (t f) -> p t f", f=Qp), axis=AX.X, op=OP.add)
        s1b = work.tile([64, TSH], BF16, tag="s1b")
        nc.vector.tensor_copy(out=s1b[:], in_=s1[:])
        s2b = work.tile([64, TSH], BF16, tag="s2b")
        nc.vector.tensor_copy(out=s2b[:], in_=s2[:])
        NCF = 64 * Q  # 4160
        mu = work.tile([128, TSH], F32, tag="mu")
        psg = psum.tile([128, TSH], F32, tag="pg")
        nc.tensor.matmul(psg[:], ones_t[:], s1b[:], start=True, stop=True)
        nc.vector.tensor_scalar_mul(out=mu[:], in0=psg[:], scalar1=1.0 / NCF)
        var = work.tile([128, TSH], F32, tag="var")
        psg2 = psum.tile([128, TSH], F32, tag="pg2")
        nc.tensor.matmul(psg2[:], ones_t[:], s2b[:], start=True, stop=True)
        nc.vector.tensor_scalar_mul(out=var[:], in0=psg2[:], scalar1=1.0 / NCF)
        mu2 = work.tile([128, TSH], F32, tag="mu2")
        nc.vector.tensor_tensor(out=mu2[:], in0=mu[:], in1=mu[:], op=OP.mult)
        nc.vector.tensor_tensor(out=var[:], in0=var[:], in1=mu2[:],
                                op=OP.subtract)
        rs = work.tile([128, TSH], F32, tag="rs")
        nc.scalar.activation(out=rs[:], in_=var[:], func=AF.Sqrt, bias=eps_t[:])
        nc.vector.reciprocal(out=rs[:], in_=rs[:])
        # out = (P - mu)*rs + inter
        o1 = big.tile([64, TSH, Qp], F32, tag="o1")
        nc.vector.tensor_tensor(out=o1[:], in0=P3, in1=bap(mu[0:64, :], [Qp]),
                                op=OP.subtract)
        nc.vector.tensor_tensor(out=o1[:], in0=o1[:], in1=bap(rs[0:64, :], [Qp]),
                                op=OP.mult)
        ict = big.tile([64, NTF], F32, tag="P")
        nc.sync.dma_start(out=ict[:], in_=icm.rearrange("c t f -> c (t f)"))
        nc.vector.tensor_tensor(out=o1[:], in0=o1[:],
                                in1=ict[:].rearrange("p (t f) -> p t f", f=Qp),
                                op=OP.add)
        nc.sync.dma_start(out=outo[:], in_=o1[:, :, :Q])
    nc.compile()
    return nc


# ======================= host weight prep =======================

def _uniform(a):
    a = np.asarray(a)
    assert np.all(a == a.flat[0]), "nonuniform LN affine not supported"
    return float(a.flat[0])


def _prep_lstm_v2(wih, whh, bih, bhh, gamma, beta):
    """LN-folded, gate-reordered (i,f,o,g) weight arrays."""
    g = np.asarray(gamma, np.float64).reshape(-1)
    b = np.asarray(beta, np.float64).reshape(-1)
    NH4 = np.asarray(wih).shape[0]
    w4 = np.asarray(wih, np.float64).reshape(NH4, C, KS)
    wih_eff = w4 * g[None, :, None]
    bih_eff = (np.asarray(bih, np.float64) + np.asarray(bhh, np.float64)
               + (w4 * b[None, :, None]).sum((1, 2)))
    H = NH4 // 4
    perm = np.r_[0:H, H:2 * H, 3 * H:4 * H, 2 * H:3 * H]
    return wih_eff[perm], bih_eff[perm], np.asarray(whh, np.float64)[perm]


def _build_weight_arrays(ii):
    """All per-core weight arrays as float32 numpy (pre-cast layouts)."""
    w = {}
    # ---- L1 (intra BiLSTM) ----
    wts, whs = [], []
    for d in range(2):
        we, be, wp = _prep_lstm_v2(
            ii["intra_wih"][d], ii["intra_whh"][d], ii["intra_bih"][d],
            ii["intra_bhh"][d], ii["intra_gamma"], ii["intra_beta"])
        wt = np.zeros((65, 4, 4, 128), np.float32)
        wh = np.zeros((128, 4, 128), np.float32)
        for m in range(4):
            for k in range(4):
                wt[:64, m, k] = we[m * 128:(m + 1) * 128, :, k].T
            wt[64, m, 0] = be[m * 128:(m + 1) * 128]
            wh[:, m] = wp[m * 128:(m + 1) * 128].T
        wts.append(wt); whs.append(wh)
    ctw_i = np.asarray(ii["intra_ct_w"], np.float64)
    ct_d = np.zeros((2, 2, 128, 128), np.float32)
    for d in range(2):
        sub = ctw_i[d * 128:(d + 1) * 128]
        for mo in range(2):
            for kp in range(2):
                for cc in range(64):
                    ct_d[d, mo, :, kp * 64 + cc] = sub[:, cc, mo * 2 + kp]
    ctb1 = np.zeros((128, 2), np.float32)
    for mo in range(2):
        for kp in range(2):
            ctb1[kp * 64:(kp + 1) * 64, mo] = np.asarray(ii["intra_ct_b"])
    w["l1"] = [
        ("wih", np.stack(wts, axis=1), BF16),
        ("whh", np.stack(whs, axis=1), BF16),
        ("ctw", ct_d.reshape(2, 2, 1, 128, 128).transpose(3, 0, 1, 2, 4), BF16),
        ("ctb", ctb1, F32),
    ]
    # ---- L2 (inter LSTM) ----
    we2, be2, wp2 = _prep_lstm_v2(
        ii["inter_wih"], ii["inter_whh"], ii["inter_bih"], ii["inter_bhh"],
        ii["inter_gamma"], ii["inter_beta"])
    wih2 = np.zeros((128, 8, 2, 128), np.float32)
    whh2 = np.zeros((128, 16, 128), np.float32)
    bih2 = np.zeros((128, 8), np.float32)
    for m in range(8):
        rows = we2[m * 128:(m + 1) * 128]
        for kp in range(2):
            wih2[:64, m, kp] = rows[:, :, 2 * kp].T
            wih2[64:, m, kp] = rows[:, :, 2 * kp + 1].T
        for kc in range(2):
            whh2[:, m * 2 + kc] = wp2[m * 128:(m + 1) * 128,
                                      kc * 128:(kc + 1) * 128].T
        bih2[:, m] = be2[m * 128:(m + 1) * 128]
    ctw2 = np.asarray(ii["inter_ct_w"], np.float64)
    ct2 = np.zeros((2, 256, 128), np.float32)
    for mo in range(2):
        for kp in range(2):
            for cc in range(64):
                j = kp * 64 + cc
                ct2[mo, :, j] = ctw2[:, cc, mo * 2 + kp]
    ctb2 = np.zeros((128, 2), np.float32)
    for mo in range(2):
        for kp in range(2):
            ctb2[kp * 64:(kp + 1) * 64, mo] = np.asarray(ii["inter_ct_b"])
    w["l2"] = [
        ("wih", wih2, BF16),
        ("whh", whh2, BF16),
        ("bih", bih2, F32),
        ("ctw", ct2.reshape(2, 2, 128, 128).transpose(2, 0, 1, 3)
         .reshape(128, 1, 2, 2, 128), BF16),
        ("ctb", ctb2, F32),
    ]
    # ---- L3a ----
    qg = _uniform(ii["q_g"]); kg = _uniform(ii["k_g"]); vg = _uniform(ii["v_g"])
    assert _uniform(ii["q_bt"]) == 0 and _uniform(ii["k_bt"]) == 0
    assert _uniform(ii["v_bt"]) == 0
    wall = np.zeros((64, 96), np.float32)
    bias96 = np.zeros((96,), np.float32)
    alpha96 = np.zeros((96,), np.float32)
    cnt96 = np.zeros((96,), np.float32)
    gs96 = np.zeros((96,), np.float32)
    grp = np.zeros((96,), np.int32)
    for h in range(NH):
        wall[:, h * 4:h * 4 + 4] = np.asarray(ii["q_w"][h]).T
        wall[:, 16 + h * 4:16 + h * 4 + 4] = np.asarray(ii["k_w"][h]).T
        wall[:, 32 + h * 16:32 + h * 16 + 16] = np.asarray(ii["v_w"][h]).T
        bias96[h * 4:h * 4 + 4] = np.asarray(ii["q_b"][h])
        bias96[16 + h * 4:16 + h * 4 + 4] = np.asarray(ii["k_b"][h])
        alpha96[h * 4:h * 4 + 4] = float(ii["q_p"][h])
        alpha96[16 + h * 4:16 + h * 4 + 4] = float(ii["k_p"][h])
        alpha96[32 + h * 16:32 + h * 16 + 16] = float(ii["v_p"][h])
        cnt96[h * 4:h * 4 + 4] = 1.0 / (E * Q)
        cnt96[16 + h * 4:16 + h * 4 + 4] = 1.0 / (E * Q)
        cnt96[32 + h * 16:32 + h * 16 + 16] = 1.0 / (Dv * Q)
        gs96[h * 4:h * 4 + 4] = qg / np.sqrt(E * Q)
        gs96[16 + h * 4:16 + h * 4 + 4] = kg
        gs96[32 + h * 16:32 + h * 16 + 16] = vg
        grp[h * 4:h * 4 + 4] = h
        grp[16 + h * 4:16 + h * 4 + 4] = 4 + h
        grp[32 + h * 16:32 + h * 16 + 16] = 8 + h
    gmat = (grp[:, None] == grp[None, :]).astype(np.float32)
    bs96 = np.stack([bias96, alpha96, cnt96, gs96], axis=1)
    w["l3a"] = [("wall", wall, BF16), ("bs", bs96, F32), ("gmat", gmat, BF16)]
    # ---- L3b mask ----
    mask = np.triu(np.full((128, 128), -1e9, np.float32), 1)
    w["msk"] = mask
    # ---- L3c ----
    assert _uniform(ii["proj_g"]) == 1.0 and _uniform(ii["proj_bt"]) == 0.0
    pw = np.ascontiguousarray(np.asarray(ii["proj_w"], np.float32).T)
    pb3 = np.zeros((64, 3), np.float32)
    pb3[:, 0] = np.asarray(ii["proj_b"])
    pb3[:, 1] = float(ii["proj_p"])
    w["l3c"] = [("pw", pw, BF16), ("pb", pb3, F32)]
    return w


# ======================= glue (device jnp) =======================

BF = jnp.bfloat16


def _posmajor_j(v):
    """[8, NPOS, 64] (NPOS=17000) -> [1024, 133, 64] pos-major tiles."""
    v = jnp.concatenate([v, jnp.zeros((8, G1 * 128 - NP1, 64), v.dtype)], 1)
    return v.reshape(8, G1, 128, 64).transpose(0, 2, 1, 3).reshape(
        8 * 128, G1, 64)


def glue0_fn(xg):
    """xg [8, 64, 250, 68] f16 -> (x_pm bf16 [1024,133,64],
    x_u f32 [1024,2,4250])."""
    v32 = xg.astype(jnp.float32)
    pm = _posmajor_j(v32.transpose(0, 2, 3, 1).reshape(8, NP1, 64)
                     .astype(BF))
    xu = v32.reshape(8, 64, 250, 17, 2, 2).transpose(0, 4, 5, 1, 3, 2)
    xu = xu.reshape(8, 2, 128, NT1).transpose(0, 2, 1, 3).reshape(
        1024, 2, NT1)
    return pm, xu


GROUPS = [[0, 1, 2, 3], [4, 5, 6, 7]]


def _a2a(x):
    """all-to-all within the 4-core b-group along dim0 (size 4)."""
    return jax.lax.all_to_all(x, "core", 0, 0, tiled=True,
                              axis_index_groups=GROUPS)


def _posmajor_l(rows_pc):
    """local [NPOS=17000, 64] -> [128, 133, 64] pos-major tiles."""
    v = jnp.concatenate(
        [rows_pc, jnp.zeros((G1 * 128 - NP1, 64), rows_pc.dtype)], 0)
    return v.reshape(G1, 128, 64).transpose(1, 0, 2)


def glue1_local(o1l):
    """per-core l1 outu [128,2,4250] f32 -> l2 (x_pm, x_u) local blocks.

    Core (b,tc) holds intra t-chunk; l2 wants q-chunks: all-to-all
    within the 4-core b-group."""
    ou = o1l.reshape(2, 64, 2, 17, 250)          # [kp, c, mo, l, t]
    intra = ou.transpose(1, 4, 3, 2, 0).reshape(64, 250, 68)
    s4 = intra.reshape(64, 250, 4, 17).transpose(2, 0, 1, 3)
    rcv = _a2a(s4)                               # [tc', 64, 250, 17]
    rows = rcv.transpose(1, 3, 0, 2).reshape(64, 17, 1000)  # [c, r, t]
    pm2 = _posmajor_l(rows.transpose(1, 2, 0).reshape(NP1, 64).astype(BF))
    xu2 = rows.reshape(64, 17, 250, 2, 2).transpose(3, 4, 0, 2, 1)
    xu2 = xu2.reshape(2, 128, NT2).transpose(1, 0, 2)
    return pm2, xu2


def glue2_local(o2l):
    """per-core l2 outu [128,2,4250] f32 -> (icm bf16, icm f32) local."""
    ou = o2l.reshape(2, 64, 2, 250, 17)          # [kp, c, mo, l, r]
    rows = ou.transpose(1, 4, 3, 2, 0).reshape(64, 17, 1000)  # [c, r, t]
    s4 = rows.reshape(64, 17, 4, 250).transpose(2, 0, 1, 3)
    rcv = _a2a(s4)                               # [qc', 64, 17, 250]
    tloc = rcv.transpose(1, 3, 0, 2).reshape(64, 250, 68)     # [c, t', q]
    tloc = jnp.concatenate(
        [tloc[:, :, :Q], jnp.zeros((64, 250, Qp - Q), tloc.dtype)], 2)
    return tloc.astype(BF), tloc


def glue3_local(qkvl):
    """per-core l3a qkvo [96,250,68] bf16 -> (qT, kT, vm) local."""
    qs = qkvl[0:16].reshape(4, 4, 250, 68)
    ks_ = qkvl[16:32].reshape(4, 4, 250, 68)
    vs = qkvl[32:96].reshape(4, 16, 250, 68)
    s4 = jnp.concatenate([qs, ks_, vs], axis=1)  # [h, 24, 250, 68]
    rcv = _a2a(s4)                               # [tc', 24, 250, 68]
    qkvh = rcv.transpose(1, 0, 2, 3).reshape(24, 1000, 68)

    def fm(a):                                   # [4, 1000, 68] -> ef-major
        t = a.transpose(0, 2, 1).reshape(272, 1000)
        t = jnp.concatenate([t, jnp.zeros((112, 1000), a.dtype)], 0)
        return t.reshape(3, 128, 1000).transpose(1, 0, 2)

    qT = fm(qkvh[0:4])
    kT = fm(qkvh[4:8])
    vmm = qkvh[8:24].transpose(1, 0, 2).reshape(1000, Dv * Qp)
    vmm = jnp.concatenate(
        [vmm, jnp.zeros((24, Dv * Qp), vmm.dtype)], 0)
    vmm = vmm.reshape(8, 128, Dv * Qp).transpose(1, 0, 2)
    return qT, kT, vmm


def glue4_local(avol):
    """per-core l3b avo [128,8,1088] bf16 -> avf [64,250,68] local."""
    a = avol.transpose(1, 0, 2).reshape(1024, Dv, Qp)[:1000]
    s4 = a.reshape(4, 250, Dv, Qp)               # [tc, t', d, f]
    rcv = _a2a(s4)                               # [h', 250, 16, 68]
    return rcv.transpose(0, 2, 1, 3).reshape(64, 250, 68)


def glue5_fn(outo):
    return outo.astype(jnp.float16)


# ---- two-pass (per-b) pipelined variants ----
PAIRS = [[0, 4], [1, 5], [2, 6], [3, 7]]


def glue0p_local(xl):
    """pass-mode input glue. xl local [32, 250, 65] f16: core r holds
    channel-half r//4 of t-chunk r%4 of this pass's b. Pair all-gather
    rebuilds the full [64, 250, 65] chunk on both group members, then
    the usual pos-major + unfold transforms."""
    full = jax.lax.all_gather(xl, "core", axis_index_groups=PAIRS)
    v32 = full.reshape(64, 250, Q).astype(jnp.float32)
    v32 = jnp.concatenate(
        [v32, jnp.zeros((64, 250, Qp - Q), jnp.float32)], 2)
    pm = _posmajor_l(v32.transpose(1, 2, 0).reshape(NP1, 64).astype(BF))
    xu = v32.reshape(64, 250, 17, 2, 2).transpose(3, 4, 0, 2, 1)
    xu = xu.reshape(2, 128, NT1).transpose(1, 0, 2)
    return pm, xu


def glue5p_local(outol):
    """pass-mode output glue. outol local [64, 250, 65] f32; core r
    returns channel-half r//4 so the global D2H is half-sized."""
    idx = jax.lax.axis_index("core")
    m = (idx >= 4).astype(jnp.float16)
    lo = outol[0:32].astype(jnp.float16)
    hi = outol[32:64].astype(jnp.float16)
    return lo * (1 - m) + hi * m


def zeros_fn():
    return (jnp.zeros((1024, 2, NT1), jnp.float32),
            jnp.zeros((1024, 2, NT2), jnp.float32),
            jnp.zeros((768, 250, 68), BF),
            jnp.zeros((1024, 8, Dv * Qp), BF),
            jnp.zeros((512, 250, Q), jnp.float32))


# ======================= stage runner =======================

class _StageRunner:
    """jit(shard_map(bass_exec)) built once per stage, reused every call."""

    def __init__(self, nc, mesh, n_cores=NCORES):
        install_neuronx_cc_hook()
        self.nc = nc
        partition_name = (nc.partition_id_tensor.name
                          if nc.partition_id_tensor else None)
        dbg_name = nc.dbg_addr.name if nc.dbg_addr is not None else None
        assert not nc.dbg_callbacks
        in_names, out_names, out_avals = [], [], []
        for alloc in nc.m.functions[0].allocations:
            if not isinstance(alloc, mybir.MemoryLocationSet):
                continue
            name = alloc.memorylocations[0].name
            if alloc.kind == "ExternalInput":
                if name != partition_name:
                    in_names.append(name)
            elif alloc.kind == "ExternalOutput":
                out_names.append(name)
                out_avals.append(jax.core.ShapedArray(
                    tuple(alloc.tensor_shape), mybir.dt.np(alloc.dtype)))
        assert dbg_name is None or dbg_name in in_names
        self.in_names, self.out_names = in_names, out_names
        n_params, n_outs = len(in_names), len(out_names)
        all_names = list(in_names) + list(out_names)
        if partition_name is not None:
            all_names.append(partition_name)

        def _body(*args):
            operands = list(args)
            if partition_name is not None:
                operands.append(partition_id_tensor())
            outs = _bass_exec_p.bind(
                *operands,
                out_avals=tuple(out_avals),
                in_names=tuple(all_names),
                out_names=tuple(out_names),
                lowering_input_output_aliases=(),
                sim_require_finite=True,
                sim_require_nnan=True,
                nc=nc,
            )
            return tuple(outs)

        in_specs = (PartitionSpec("core"),) * (n_params + n_outs)
        out_specs = (PartitionSpec("core"),) * n_outs
        donate = tuple(range(n_params, n_params + n_outs))
        self.fn = jax.jit(
            shard_map(_body, mesh=mesh, in_specs=in_specs,
                      out_specs=out_specs, check_rep=False),
            donate_argnums=donate, keep_unused=True)

    def __call__(self, arrs, zeros):
        return self.fn(*arrs, *zeros)


# ======================= host orchestration =======================

_C = {}


def _wfingerprint(ii):
    keys = ["intra_wih", "intra_whh", "inter_wih", "inter_whh", "q_w",
            "k_w", "v_w", "proj_w", "intra_ct_w", "inter_ct_w"]
    return tuple(float(np.asarray(ii[k]).sum()) for k in keys)


def _ensure(ii):
    if "mesh" not in _C:
        devs = jax.devices()[:NCORES]
        _C["mesh"] = Mesh(np.asarray(devs), ("core",))
        _C["shard"] = NamedSharding(_C["mesh"], PartitionSpec("core"))
    mesh, shard = _C["mesh"], _C["shard"]
    if "l1" not in _C:
        _C["l1"] = _StageRunner(build_lstm_launch("intra"), mesh)
        _C["l2"] = _StageRunner(build_lstm_launch("inter"), mesh)
        _C["l3a"] = _StageRunner(build_l3a(), mesh)
        _C["l3b"] = _StageRunner(build_l3b(), mesh)
        _C["l3c"] = _StageRunner(build_l3c(), mesh)
        jt = lambda f: jax.jit(f, out_shardings=shard)
        P = PartitionSpec("core")

        def sm(f, nin, nout):
            return jax.jit(shard_map(
                f, mesh=mesh, in_specs=(P,) * nin,
                out_specs=(P,) * nout if nout > 1 else P, check_rep=False))

        _C["g0"] = jt(glue0_fn)
        _g0p_sm = shard_map(glue0p_local, mesh=mesh, in_specs=(P,),
                            out_specs=(P, P), check_rep=False)

        def _g0pz(xl):
            pm, xu = _g0p_sm(xl)
            return (pm, xu) + zeros_fn()

        _C["g0pz"] = jax.jit(_g0pz, out_shardings=shard)
        _C["g1"] = sm(glue1_local, 1, 2)
        _C["g2"] = sm(glue2_local, 1, 2)
        _C["g3"] = sm(glue3_local, 1, 3)
        _C["g4"] = sm(glue4_local, 1, 1)
        _C["g5"] = jt(glue5_fn)
        _C["g5p"] = sm(glue5p_local, 1, 1)
        _C["zeros"] = jax.jit(zeros_fn, out_shardings=shard)
    fp = _wfingerprint(ii)
    if _C.get("wfp") != fp:
        w = _build_weight_arrays(ii)
        put = lambda a: jax.device_put(
            np.ascontiguousarray(np.tile(np.asarray(a, np.float32),
                                         (NCORES,) + (1,) * (a.ndim - 1))),
            _C["shard"])
        devw = {}
        for stage in ["l1", "l2", "l3a", "l3c"]:
            names, arrs, dts = zip(*w[stage])
            f32d = [put(a) for a in arrs]
            castfn = jax.jit(
                lambda *xs, dts=dts: tuple(
                    x.astype(jnp.bfloat16) if dt == BF16 else x
                    for x, dt in zip(xs, dts)),
                out_shardings=_C["shard"])
            casted = castfn(*f32d)
            devw[stage] = dict(zip(names, casted))
        devw["msk"] = jax.device_put(
            np.tile(w["msk"], (NCORES, 1)), _C["shard"])
        _C["w"] = devw
        _C["wfp"] = fp


def _stage_inputs(runner, data, weights):
    m = dict(data)
    m.update(weights)
    return [m[n] for n in runner.in_names]


TWO_PASS = os.environ.get("K_TWO_PASS", "1") == "1"


def _run_stages(pm1, xu1, zs):
    w = _C["w"]
    z1, z2, z3a, z3b, z3c = zs
    (o1,) = _C["l1"](_stage_inputs(_C["l1"], {"x_pm": pm1, "x_u": xu1},
                                   w["l1"]), [z1])
    pm2, xu2 = _C["g1"](o1)
    (o2,) = _C["l2"](_stage_inputs(_C["l2"], {"x_pm": pm2, "x_u": xu2},
                                   w["l2"]), [z2])
    icm_bf, icm_f = _C["g2"](o2)
    (qkv,) = _C["l3a"](_stage_inputs(_C["l3a"], {"icm": icm_bf}, w["l3a"]),
                       [z3a])
    qT, kT, vm = _C["g3"](qkv)
    (avo,) = _C["l3b"](_stage_inputs(
        _C["l3b"], {"qT": qT, "kT": kT, "vm": vm, "msk": w["msk"]}, {}),
        [z3b])
    avf = _C["g4"](avo)
    (outo,) = _C["l3c"](_stage_inputs(
        _C["l3c"], {"avf": avf, "icm": icm_f}, w["l3c"]), [z3c])
    return outo


def kernel(**inputs):
    ii = {k: np.asarray(v) for k, v in inputs.items()}
    _ensure(ii)
    x = np.asarray(ii["x"], np.float32)

    if not TWO_PASS:
        xg = np.zeros((2, 4, 64, 250, 68), np.float16)
        xg[..., :Q] = x.reshape(2, 64, 4, 250, Q).transpose(0, 2, 1, 3, 4)
        xd = jax.device_put(xg.reshape(8, 64, 250, 68), _C["shard"])
        zs = _C["zeros"]()
        pm1, xu1 = _C["g0"](xd)
        of = _C["g5"](_run_stages(pm1, xu1, zs))
        oh = np.asarray(of)     # [512, 250, 65] f16
        out = oh.reshape(2, 4, 64, 250, Q).transpose(0, 2, 1, 3, 4)
        return np.ascontiguousarray(out.reshape(2, 64, 1000, Q)
                                    .astype(np.float32))

    # two-pass pipelined: pass b uploads half the bytes (channel-halves
    # across core pairs), both 4-core groups compute that b, pass 0's
    # download overlaps pass 1's upload (the tunnel is full duplex).
    import threading
    out = np.empty((2, 64, 1000, Q), np.float32)

    def pack(b):
        g = x[b].reshape(2, 32, 4, 250, Q).transpose(0, 2, 1, 3, 4)
        return np.ascontiguousarray(g, dtype=np.float16).reshape(
            8, 32, 250, Q)

    def run_pass(xg):
        xd = jax.device_put(xg, _C["shard"])
        pm1, xu1, *zs = _C["g0pz"](xd)
        return _C["g5p"](_run_stages(pm1, xu1, zs))

    def drain(b, of):
        oh = np.asarray(of)     # [256, 250, 65] f16, rows (half, chunk)
        o = oh.reshape(2, 4, 32, 250, Q).transpose(0, 2, 1, 3, 4)
        out[b] = o.reshape(64, 1000, Q).astype(np.float32)

    of0 = run_pass(pack(0))
    th = threading.Thread(target=drain, args=(0, of0))
    th.start()
    of1 = run_pass(pack(1))
    drain(1, of1)
    th.join()
    return out
